# revision 85
# baseline (speedup 1.0000x reference)
"""Multi-head attention (B=4, N=2048, DM=1024, H=16, DH=64) on 8 trn2 cores.

Sharding: core c -> (batch b = c//2, head-group hg = c%2 of 8 heads).

Live-token compaction: the pair mask only keeps (i,j) score pairs where
both tokens are live, and every dead query row of the reference output is
the SAME uniform average over all value tokens.  So the host gathers the
~NL live tokens of each batch into a compact [DM, NLP] x^T (NLP = padded
live count, multiple of 128, >= NL+1), the device runs attention on live
tokens only, and the host scatters rows back, filling dead rows with the
row produced by the first zero-padded query column.

Per-column semantics on device (q = x@Wq, k = x@Wk, scaled, no masking):
  - live i, live j: t = exp(q_i.k_j) -- the real softmax numerator.
  - pad i (x=0 -> q=0): t = 1 for all j, plus a rank-1 correction row
    (+padrow_i * dvec, dvec = [sum_dead v | N_dead] from the host) makes
    pv = [sum_all v | N], i.e. exactly the reference's uniform dead row.
  - pad j (x=0 -> k=v=0): t = exp(0) = 1 but vplus rows are zero (the
    denominator ones-column is L=live-indicator, not constant 1), so pads
    contribute nothing.

Device layout mirrors the dense kernel: feature-major q/k ([64,NLP] per
head), token-major v with an appended denominator column, scores
transposed [j,i] so PV needs no transpose, exp on ACT, a small [N,64]
transpose per head for the output projection.

Scheduling: heads run as a software pipeline -- window W(h) emits scores
+exp for head h, PV for head h-1, the batched last-i-tile ("tail") scores
for head h-1 and PV for head h-2, plus deadline-scheduled filler chunks
(later pairs' qk projections, then partial output chunks).  The output
projection is split into two half-contractions written to separate DRAM
tensors (outA = pairs 0-1 in f32 overlapped with the mid loop, outB =
pairs 2-3 in bf16 to halve the drain DMA); the host sums them.
"""

import sys

sys.path.insert(0, "/opt/trn_rl_repo")

import numpy as np
import ml_dtypes

B, N, DM, H, DH = 4, 2048, 1024, 16, 64
SCALE = DH**-0.5
NCORES = 8
HG = 2  # head groups (tensor-parallel factor)
HL = H // HG  # 8 heads per core
NP = HL // 2  # 4 head pairs
FQK = HL * 2 * DH  # 1024 qk features per core
FV = HL * DH  # 512 v features per core
P = 128
DMT = DM // P  # 8 dm tiles
VW = DH + 1  # 65: v columns + denominator column
VROW = HL * VW  # 520
HT = FV // P  # 4 head-dim tiles for the projection

_CACHE = {}


def _build_program(NLP):
    import concourse.mybir as mybir
    import concourse.tile as tile
    from concourse import bacc
    from concourse.masks import make_identity

    bf = mybir.dt.bfloat16
    f32 = mybir.dt.float32
    EXP = mybir.ActivationFunctionType.Exp
    COPY = mybir.ActivationFunctionType.Copy

    NTL = (NLP + P - 1) // P  # live token tiles (last may be partial)
    TW = NLP - (NTL - 1) * P  # width of the last tile
    # i-span structure: one wide main span (software-pipelined head loop),
    # remaining tiles handled in the pipelined tail path (w==1) or a slow
    # generic path (w>1, only for much larger masks).
    spans = []
    off = 0
    while NTL - off > 0:
        w = min(8, NTL - off)
        spans.append((off, w))
        off += w
    off0, w0 = spans[0]
    tail1 = [s for s in spans[1:] if s[1] == 1]
    slow_tails = [s for s in spans[1:] if s[1] > 1]

    def IW(it):
        return TW if it == NTL - 1 else P

    # qk_all block stride: a (P-TW)-col pad after each feature block so the
    # full-width kT reads of the partial last j-tile never touch another
    # block's (possibly unwritten) data
    BS = NLP + (P - TW) if TW < P else NLP
    # qk-projection column chunks (psum-bank sized)
    qk_chunks = [(c0, min(512, NLP - c0)) for c0 in range(0, NLP, 512)]

    nc = bacc.Bacc(
        "TRN2", target_bir_lowering=False, debug=False, num_devices=NCORES
    )
    xTl = nc.dram_tensor("xTl", [DM, NLP], bf, kind="ExternalInput")
    wqk = nc.dram_tensor("wqk", [DM, FQK], bf, kind="ExternalInput")
    wv = nc.dram_tensor("wv", [DM, FV], bf, kind="ExternalInput")
    wout = nc.dram_tensor("wout", [FV, DM], bf, kind="ExternalInput")
    padrow = nc.dram_tensor("padrow", [1, NLP], bf, kind="ExternalInput")
    lind = nc.dram_tensor("lind", [P, NTL], f32, kind="ExternalInput")
    dvec = nc.dram_tensor("dvec", [1, VROW], bf, kind="ExternalInput")
    outA = nc.dram_tensor("outA", [NLP, DM], f32, kind="ExternalOutput")
    outB = nc.dram_tensor("outB", [NLP, DM], bf, kind="ExternalOutput")

    with tile.TileContext(nc) as tc:
        with tc.tile_pool(name="const", bufs=1) as cp:
            xTl_sb = cp.tile([P, DMT * NLP], bf, tag="xTl")
            wqk_sb = cp.tile([P, DMT * FQK], bf, tag="wqk")
            wv_sb = cp.tile([P, DMT * FV], bf, tag="wv")
            wout_sb = cp.tile([P, HT * DM], bf, tag="wout")
            padrow_sb = cp.tile([1, NLP], bf, tag="padrow")
            lind_sb = cp.tile([P, NTL], f32, tag="lind")
            dvec_sb = cp.tile([1, VROW], bf, tag="dvec")
            ident = cp.tile([P, P], bf, tag="ident")
            zeros8 = cp.tile([P, HL], bf, tag="zeros8")
            vplus = cp.tile([P, NTL * VROW], bf, tag="vplus")
            qk_all = cp.tile([P, HL * BS], bf, tag="qkall")
            attT = cp.tile([P, HT * NLP], bf, tag="attT")

            # DMA order mirrors consumption: per-dm-tile x^T plus the
            # pair-0 qk weight columns first (feeds the first projection),
            # then v weights (needed by the head-1 window), small tensors,
            # then the remaining qk weight columns and w_out.
            for dmt in range(DMT):
                nc.sync.dma_start(
                    out=wqk_sb[:, dmt * FQK : dmt * FQK + 3 * P],
                    in_=wqk[dmt * P : (dmt + 1) * P, 0 : 3 * P],
                )
                nc.sync.dma_start(
                    out=xTl_sb[:, dmt * NLP : (dmt + 1) * NLP],
                    in_=xTl[dmt * P : (dmt + 1) * P, :],
                )
            nc.sync.dma_start(
                out=wv_sb[:, :].rearrange("p (d f) -> p d f", d=DMT, f=FV),
                in_=wv[:, :].rearrange("(d p) c -> p d c", p=P),
            )
            nc.sync.dma_start(out=lind_sb[:, :], in_=lind[:, :])
            nc.sync.dma_start(out=padrow_sb[:, :], in_=padrow[:, :])
            nc.sync.dma_start(out=dvec_sb[:, :], in_=dvec[:, :])
            for dmt in range(DMT):
                nc.sync.dma_start(
                    out=wqk_sb[:, dmt * FQK + 3 * P : (dmt + 1) * FQK],
                    in_=wqk[dmt * P : (dmt + 1) * P, 3 * P :],
                )
            for ht in range(HT):
                nc.sync.dma_start(
                    out=wout_sb[:, ht * DM : (ht + 1) * DM],
                    in_=wout[ht * P : (ht + 1) * P, :],
                )
            make_identity(nc, ident)
            nc.gpsimd.memset(zeros8[:, :], 0.0)
            # zero the pad margin after each feature block (spill target of
            # the full-width kT reads of the partial last j-tile)
            if TW < P:
                for f in range(HL):
                    nc.gpsimd.memset(qk_all[:, f * BS + NLP : (f + 1) * BS], 0.0)
            if TW < P:
                # rows of the partial last j-tile beyond the live+pad range
                # never get v written; zero the whole block up front (the v
                # eviction later overwrites rows [0:TW]) so spilled-garbage
                # exp rows contract against zeros
                nc.gpsimd.memset(vplus[:, (NTL - 1) * VROW : NTL * VROW], 0.0)

            vp4 = vplus.rearrange("p (t g c) -> p t g c", t=NTL, g=HL, c=VW)

            with (
                tc.tile_pool(name="psqk", bufs=2, space="PSUM") as pqk,
                tc.tile_pool(name="pss", bufs=2, space="PSUM") as pss,
                tc.tile_pool(name="psa", bufs=1, space="PSUM") as psa,
                tc.tile_pool(name="tpool", bufs=20) as tp,
                tc.tile_pool(name="ttpool", bufs=3) as ttp,
                tc.tile_pool(name="appool", bufs=2) as app,
                tc.tile_pool(name="spool", bufs=4) as sp,
            ):
                PSW = min(w0 * P, 1024)

                def emit_qk_chunk(f, c0, cw):
                    ps = pqk.tile([P, 512], f32, tag="qk", name="ps_qk")
                    for dmt in range(DMT):
                        nc.tensor.matmul(
                            ps[:, :cw],
                            wqk_sb[:, dmt * FQK + f * P : dmt * FQK + (f + 1) * P],
                            xTl_sb[:, dmt * NLP + c0 : dmt * NLP + c0 + cw],
                            start=(dmt == 0),
                            stop=(dmt == DMT - 1),
                        )
                    nc.vector.tensor_copy(
                        qk_all[:, f * BS + c0 : f * BS + c0 + cw], ps[:, :cw]
                    )

                def emit_v(tt):
                    W = IW(tt)
                    ps = pqk.tile([P, FV], f32, tag="qk", name="ps_v")
                    for dmt in range(DMT):
                        nc.tensor.matmul(
                            ps[0:W, :],
                            xTl_sb[:, dmt * NLP + tt * P : dmt * NLP + tt * P + W],
                            wv_sb[:, dmt * FV : (dmt + 1) * FV],
                            start=(dmt == 0),
                            stop=(dmt == DMT - 1),
                        )
                    nc.vector.tensor_copy(
                        vp4[0:W, tt, :, 0:DH],
                        ps[0:W].rearrange("p (g c) -> p g c", g=HL, c=DH),
                    )
                    # denominator column = live indicator (0 for pad rows)
                    nc.vector.tensor_scalar_add(
                        vp4[0:W, tt, :, DH],
                        zeros8[0:W, :],
                        lind_sb[0:W, tt : tt + 1],
                    )

                def sc_wide(h, off, w, jt):
                    pair, hh = h // 2, h % 2
                    p0 = hh * 64
                    qc = (2 * pair) * BS + off * P
                    kc = (2 * pair + 1) * BS
                    ps_s = pss.tile([P, PSW], f32, tag="s", name="ps_s")
                    for c0 in range(0, w * P, 512):
                        cw = min(512, w * P - c0)
                        nc.tensor.matmul(
                            ps_s[:, c0 : c0 + cw],
                            qk_all[p0 : p0 + 64, kc + jt * P : kc + (jt + 1) * P],
                            qk_all[p0 : p0 + 64, qc + c0 : qc + c0 + cw],
                            start=True,
                            stop=True,
                        )
                    t_sb = tp.tile([P, PSW], bf, tag="t", name="t_sb")
                    nc.scalar.activation(t_sb[:, : w * P], ps_s[:, : w * P], EXP)
                    return t_sb

                def pv_wide(h, w, jt, t_sb, pa):
                    vsl = vplus[:, jt * VROW + h * VW : jt * VROW + (h + 1) * VW]
                    for it in range(w):
                        nc.tensor.matmul(
                            pa[:, it * P : it * P + VW],
                            t_sb[:, it * P : (it + 1) * P],
                            vsl,
                            start=(jt == 0 and it % 4 == 0),
                            stop=False,
                        )

                def corr_wide(h, off, w, pa):
                    for it in range(w):
                        nc.tensor.matmul(
                            pa[:, it * P : it * P + VW],
                            padrow_sb[:, (off + it) * P : (off + it + 1) * P],
                            dvec_sb[:, h * VW : (h + 1) * VW],
                            start=False,
                            stop=(it % 4 == 3 or it == w - 1),
                        )

                def norm_wide(h, w, pa, ap):
                    p0 = (h % 2) * 64
                    r_sb = sp.tile([P, 8], f32, tag="r", name="r_sb")
                    pa3 = pa.rearrange("p (t c) -> p t c", t=PSW // P, c=P)
                    nc.vector.reciprocal(r_sb[:, :w], pa3[:, :w, DH])
                    for it in range(w):
                        nc.vector.tensor_scalar_mul(
                            ap[:, it * P + p0 : it * P + p0 + DH],
                            pa[:, it * P : it * P + DH],
                            r_sb[:, it : it + 1],
                        )

                def pv_slot(h, off, w, it, pa, t_list, ap):
                    # slot-major: finish output tile `it` for head h in one
                    # go (all-j PV + correction + normalize), so downstream
                    # per-tile work pipelines inside the window
                    p0 = (h % 2) * 64
                    vcol = h * VW
                    for jt in range(NTL):
                        nc.tensor.matmul(
                            pa[:, it * P : it * P + VW],
                            t_list[jt][:, it * P : (it + 1) * P],
                            vplus[:, jt * VROW + vcol : jt * VROW + vcol + VW],
                            start=(jt == 0 and it % 4 == 0),
                            stop=False,
                        )
                    nc.tensor.matmul(
                        pa[:, it * P : it * P + VW],
                        padrow_sb[:, (off + it) * P : (off + it + 1) * P],
                        dvec_sb[:, h * VW : (h + 1) * VW],
                        start=False,
                        stop=(it % 4 == 3 or it == w - 1),
                    )
                    r1 = sp.tile([P, 8], f32, tag="r", name="r1")
                    nc.vector.reciprocal(
                        r1[:, 0:1], pa[:, it * P + DH : it * P + DH + 1]
                    )
                    nc.vector.tensor_scalar_mul(
                        ap[:, it * P + p0 : it * P + p0 + DH],
                        pa[:, it * P : it * P + DH],
                        r1[:, 0:1],
                    )

                def transpose_it(pair, off, it, ap):
                    ps_tr = pqk.tile([P, P], bf, tag="qk", name="ps_tr")
                    nc.tensor.transpose(
                        ps_tr[:, :], ap[:, it * P : (it + 1) * P], ident
                    )
                    dst = attT[
                        :,
                        pair * NLP + (off + it) * P : pair * NLP
                        + (off + it + 1) * P,
                    ]
                    nc.vector.tensor_copy(dst, ps_tr[:, :])

                def transpose_pair(pair, off, w, ap):
                    # batch 4 transposes per 1-bank slot -> one eviction copy
                    it = 0
                    while it < w:
                        nb = min(4, w - it)
                        if IW(off + it + nb - 1) != P:
                            nb = 1
                        ps_tr = pqk.tile([P, 512], bf, tag="qk", name="ps_tr")
                        wtot = 0
                        for k in range(nb):
                            W = IW(off + it + k)
                            nc.tensor.transpose(
                                ps_tr[:, k * P : k * P + W],
                                ap[0:W, (it + k) * P : (it + k + 1) * P],
                                ident[0:W, 0:W],
                            )
                            wtot = k * P + W
                        nc.vector.tensor_copy(
                            attT[
                                :,
                                pair * NLP + (off + it) * P : pair * NLP
                                + (off + it) * P
                                + wtot,
                            ],
                            ps_tr[:, 0:wtot],
                        )
                        it += nb

                def s_tail(h, off):
                    # batched narrow-tail scores: the TW-wide last i-tile for
                    # all j-tiles, packed into as few psum banks / exp
                    # instructions as possible
                    pair, hh = h // 2, h % 2
                    p0 = hh * 64
                    qc = (2 * pair) * BS + off * P
                    kc = (2 * pair + 1) * BS
                    bpb = max(1, 512 // TW)  # batches per psum bank
                    t_t = ttp.tile([P, NTL * TW], bf, tag="tt", name="t_tail")
                    done = 0
                    while done < NTL:
                        nb = min(2 * bpb, NTL - done)  # one 2-bank slot
                        ps_s = pss.tile([P, PSW], f32, tag="s", name="ps_st")
                        for g in range(nb):
                            jt = done + g
                            pos = (g // bpb) * 512 + (g % bpb) * TW
                            nc.tensor.matmul(
                                ps_s[:, pos : pos + TW],
                                qk_all[p0 : p0 + 64, kc + jt * P : kc + jt * P + P],
                                qk_all[p0 : p0 + 64, qc : qc + TW],
                                start=True,
                                stop=True,
                            )
                        for bk in range((nb + bpb - 1) // bpb):
                            cnt = min(bpb, nb - bk * bpb)
                            nc.scalar.activation(
                                t_t[
                                    :,
                                    (done + bk * bpb) * TW : (done + bk * bpb + cnt)
                                    * TW,
                                ],
                                ps_s[:, bk * 512 : bk * 512 + cnt * TW],
                                EXP,
                            )
                        done += nb
                    return t_t

                def p_tail(h, off, t_t, ap):
                    # 65-col accumulator lives in a 1-bank "qk" slot so it
                    # never waits on the main PV accumulator (deadlock risk)
                    p0 = (h % 2) * 64
                    pa = pqk.tile([P, 512], f32, tag="qk", name="pa_t")
                    for jt in range(NTL):
                        nc.tensor.matmul(
                            pa[0:TW, 0:VW],
                            t_t[:, jt * TW : (jt + 1) * TW],
                            vplus[:, jt * VROW + h * VW : jt * VROW + (h + 1) * VW],
                            start=(jt == 0),
                            stop=False,
                        )
                    nc.tensor.matmul(
                        pa[0:TW, 0:VW],
                        padrow_sb[:, off * P : off * P + TW],
                        dvec_sb[:, h * VW : (h + 1) * VW],
                        start=False,
                        stop=True,
                    )
                    r_sb = sp.tile([P, 8], f32, tag="r", name="r_t")
                    nc.vector.reciprocal(r_sb[0:TW, 0:1], pa[0:TW, DH : DH + 1])
                    nc.vector.tensor_scalar_mul(
                        ap[0:TW, p0 : p0 + DH], pa[0:TW, 0:DH], r_sb[0:TW, 0:1]
                    )

                nout = [0]

                def emit_half(it, ch, half):
                    # half 0: pairs 0-1 -> outA f32; half 1: pairs 2-3 -> outB bf16
                    W = IW(it)
                    ps_o = pqk.tile([P, 512], f32, tag="qk", name="ps_o")
                    for ht in (0, 1) if half == 0 else (2, 3):
                        nc.tensor.matmul(
                            ps_o[0:W, :],
                            attT[:, ht * NLP + it * P : ht * NLP + it * P + W],
                            wout_sb[:, ht * DM + ch * 512 : ht * DM + (ch + 1) * 512],
                            start=(ht % 2 == 0),
                            stop=(ht % 2 == 1),
                        )
                    dt = f32 if half == 0 else bf
                    o_sb = sp.tile([P, 512], dt, tag="obA" if half == 0 else "obB",
                                   name="o_sb")
                    # A-half evictions stay off ACT (it paces mid-loop exps);
                    # B-half runs in the drain where ACT is idle.
                    if half == 1:
                        nc.scalar.activation(o_sb[0:W, :], ps_o[0:W, :], COPY)
                    else:
                        nc.vector.tensor_copy(o_sb[0:W, :], ps_o[0:W, :])
                    nout[0] += 1
                    dst = outA if half == 0 else outB
                    nc.sync.dma_start(
                        out=dst[it * P : it * P + W, ch * 512 : (ch + 1) * 512],
                        in_=o_sb[0:W, :],
                    )

                # ---- filler queues ----
                # qk chunks for pairs 1..3: pair p before head 2p's scores.
                fast_start = len(qk_chunks) == 3
                fill_units = [
                    (f, c0, cw)
                    for pair in range(1, NP)
                    for f in (2 * pair, 2 * pair + 1)
                    for (c0, cw) in qk_chunks
                    if not (fast_start and f == 2 and c0 < qk_chunks[2][0])
                ]
                n_units = len(fill_units)
                cpp = 2 * len(qk_chunks)
                fill_pos = [0]
                cpair1 = cpp - (2 if fast_start else 0)

                def emit_fill_to(tgt):
                    k = fill_pos[0]
                    for u in fill_units[k : min(n_units, tgt)]:
                        emit_qk_chunk(*u)
                    fill_pos[0] = max(k, min(n_units, tgt))

                def needed_before(h):
                    p = max(0, h // 2)
                    return min(n_units, cpair1 if p == 1 else
                               cpair1 + (p - 1) * cpp if p > 1 else 0)

                # A half-chunks (pairs 0-1): ready once pair-1 main+tail
                # transposes are done (end of window 5); fill windows 6-7.
                a_units = [
                    (it, ch) for it in range(off0, off0 + w0) for ch in range(2)
                ]
                nA = len(a_units)
                a_pos = [0]

                def emit_a_to(tgt):
                    k = a_pos[0]
                    for u in a_units[k : min(nA, tgt)]:
                        emit_half(u[0], u[1], 0)
                    a_pos[0] = max(k, min(nA, tgt))

                # ---- window 0: pair-0 projections + head-0 scores,
                #      pair-1 qk chunks interleaved ----
                if len(qk_chunks) == 3:
                    # dmt-outer interleave across the f0/f1 chunks plus
                    # pair-1's first q chunks keeps the PE fed at DMA
                    # arrival pace (chunk-slots borrowed from the idle
                    # pss/psa pools + pqk)
                    sA = pss.tile([P, PSW], f32, tag="s", name="ps_q0")
                    sB = pss.tile([P, PSW], f32, tag="s", name="ps_q1")
                    qA = pqk.tile([P, 512], f32, tag="qk", name="ps_q2")
                    qB = pqk.tile([P, 512], f32, tag="qk", name="ps_q3")
                    aA = psa.tile([P, PSW], f32, tag="att", name="ps_q4")
                    units = [
                        (0, qk_chunks[0][0], qk_chunks[0][1], sA, 0),
                        (0, qk_chunks[1][0], qk_chunks[1][1], sA, 512),
                        (0, qk_chunks[2][0], qk_chunks[2][1], sB, 0),
                        (1, qk_chunks[0][0], qk_chunks[0][1], sB, 512),
                        (1, qk_chunks[1][0], qk_chunks[1][1], qA, 0),
                        (1, qk_chunks[2][0], qk_chunks[2][1], qB, 0),
                        (2, qk_chunks[0][0], qk_chunks[0][1], aA, 0),
                        (2, qk_chunks[1][0], qk_chunks[1][1], aA, 512),
                    ]
                    for dmt in range(DMT):
                        for f, c0, cw, ps, so in units:
                            nc.tensor.matmul(
                                ps[:, so : so + cw],
                                wqk_sb[
                                    :, dmt * FQK + f * P : dmt * FQK + (f + 1) * P
                                ],
                                xTl_sb[:, dmt * NLP + c0 : dmt * NLP + c0 + cw],
                                start=(dmt == 0),
                                stop=(dmt == DMT - 1),
                            )
                    for f, c0, cw, ps, so in units:
                        nc.vector.tensor_copy(
                            qk_all[:, f * BS + c0 : f * BS + c0 + cw],
                            ps[:, so : so + cw],
                        )
                else:
                    for f in (0, 1):
                        for c0, cw in qk_chunks:
                            emit_qk_chunk(f, c0, cw)
                t_store = {0: []}
                tt_store = {}
                for jt in range(NTL):
                    t_store[0].append(sc_wide(0, off0, w0, jt))
                if tail1:
                    # pair-0 tail scores cover the wv DMA wait; their spill
                    # reads (pair-1 q block) are written by the fast startup
                    tt_store[0] = s_tail(0, tail1[0][0])
                    tt_store[1] = s_tail(1, tail1[0][0])
                for tt in range(min(3, NTL)):
                    emit_v(tt)
                emit_fill_to(cpair1)

                # ---- windows 1..7: S(h) || PV(h-1) || tail(h-1 scores,
                #      h-2 PV) || fillers ----
                ap_cur = None
                apt_cur = None
                for h in range(1, HL):
                    emit_fill_to(needed_before(h))
                    fprev = fill_pos[0]
                    fth = n_units if h >= 5 else max(
                        needed_before(h + 1), (n_units * h + 4) // 5
                    )
                    aprev = a_pos[0]
                    ath = {5: 6, 6: 14}.get(h, 0 if h < 5 else nA)
                    t_store[h] = []
                    pa = psa.tile([P, PSW], f32, tag="att", name="pa")
                    for jt in range(NTL):
                        t_store[h].append(sc_wide(h, off0, w0, jt))
                        if h == 1 and jt >= 3:
                            emit_v(jt)
                        emit_fill_to(fprev + ((fth - fprev) * (jt + 1)) // NTL)
                        emit_a_to(aprev + ((ath - aprev) * (jt + 1)) // NTL)
                        pv_wide(h - 1, w0, jt, t_store[h - 1][jt], pa)
                        t_store[h - 1][jt] = None
                        if jt == 2 and h >= 2 and tail1 and (h - 2) in tt_store:
                            # lag-2 tail PV for head h-2
                            offt = tail1[0][0]
                            if (h - 2) % 2 == 0:
                                apt_cur = app.tile([P, P], bf, tag="apt", name="apt")
                            p_tail(h - 2, offt, tt_store.pop(h - 2), apt_cur)
                            if (h - 2) % 2 == 1:
                                transpose_pair((h - 2) // 2, offt, 1, apt_cur)
                    if tail1 and h >= 3:
                        tt_store[h - 1] = s_tail(h - 1, tail1[0][0])
                    if tail1 and h == HL - 1:
                        tt_store[h] = s_tail(h, tail1[0][0])
                    corr_wide(h - 1, off0, w0, pa)
                    if (h - 1) % 2 == 0:
                        ap_cur = app.tile([P, PSW], bf, tag="ap", name="ap")
                    norm_wide(h - 1, w0, pa, ap_cur)
                    if (h - 1) % 2 == 1:
                        transpose_pair((h - 1) // 2, off0, w0, ap_cur)

                # ---- epilogue: PV(7), tails 6-7, pipelined pair-3 finish ----
                def emit_outB_big(it, split=False):
                    # whole-row B chunk: pairs 2-3 for both DM halves in one
                    # 2-bank pss slot, one eviction, one outB DMA
                    W = IW(it)
                    ps_o = pss.tile([P, PSW], f32, tag="s", name="ps_b")
                    for ch in range(2):
                        for ht in (2, 3):
                            nc.tensor.matmul(
                                ps_o[0:W, ch * 512 : (ch + 1) * 512],
                                attT[:, ht * NLP + it * P : ht * NLP + it * P + W],
                                wout_sb[
                                    :, ht * DM + ch * 512 : ht * DM + (ch + 1) * 512
                                ],
                                start=(ht == 2),
                                stop=(ht == 3),
                            )
                    o_sb = sp.tile([P, 1024], bf, tag="obB", name="o_sbB")
                    if split:
                        # parallel eviction halves (ACT + DVE): shallow drain
                        nc.scalar.activation(
                            o_sb[0:W, 0:512], ps_o[0:W, 0:512], COPY
                        )
                        nc.vector.tensor_copy(
                            o_sb[0:W, 512:1024], ps_o[0:W, 512:1024]
                        )
                        nc.sync.dma_start(
                            out=outB[it * P : it * P + W, 0:512],
                            in_=o_sb[0:W, 0:512],
                        )
                        nc.sync.dma_start(
                            out=outB[it * P : it * P + W, 512:1024],
                            in_=o_sb[0:W, 512:1024],
                        )
                        return
                    if nout[0] % 2 == 0:
                        nc.scalar.activation(o_sb[0:W, :], ps_o[0:W, :1024], COPY)
                    else:
                        nc.vector.tensor_copy(o_sb[0:W, :], ps_o[0:W, :1024])
                    nout[0] += 1
                    nc.sync.dma_start(
                        out=outB[it * P : it * P + W, :], in_=o_sb[0:W, :]
                    )

                if tail1 and (HL - 1) not in tt_store:
                    tt_store[HL - 1] = s_tail(HL - 1, tail1[0][0])
                emit_fill_to(n_units)
                aprev = a_pos[0]
                pa = psa.tile([P, PSW], f32, tag="att", name="pa")
                for jt in range(NTL):
                    emit_a_to(aprev + ((nA - aprev) * (jt + 1)) // NTL)
                    pv_wide(HL - 1, w0, jt, t_store[HL - 1][jt], pa)
                    if jt == 2 and tail1 and (HL - 2) in tt_store:
                        offt = tail1[0][0]
                        apt_cur = app.tile([P, P], bf, tag="apt", name="apt")
                        p_tail(HL - 2, offt, tt_store.pop(HL - 2), apt_cur)
                    if jt == 4 and tail1 and (HL - 1) in tt_store:
                        offt = tail1[0][0]
                        p_tail(HL - 1, offt, tt_store.pop(HL - 1), apt_cur)
                        transpose_pair(NP - 1, offt, 1, apt_cur)
                    if jt == 0 and tail1:
                        # tail i-tile outA rows (pairs 0-1 only): PE work to
                        # cover the window-boundary normalize wait
                        offt = tail1[0][0]
                        emit_half(offt, 0, 0)
                        emit_half(offt, 1, 0)

                corr_wide(HL - 1, off0, w0, pa)
                emit_a_to(nA)
                # pipelined pair-3 finish: normalize slot -> transpose ->
                # previous tile's whole-row B chunk (hides eviction latency)
                p0e = ((HL - 1) % 2) * 64
                r_sb = sp.tile([P, 8], f32, tag="r", name="r_e")
                pa3 = pa.rearrange("p (t c) -> p t c", t=PSW // P, c=P)
                nc.vector.reciprocal(r_sb[:, :w0], pa3[:, :w0, DH])
                for it in range(w0):
                    nc.vector.tensor_scalar_mul(
                        ap_cur[:, it * P + p0e : it * P + p0e + DH],
                        pa[:, it * P : it * P + DH],
                        r_sb[:, it : it + 1],
                    )
                    ps_tr = pqk.tile([P, P], bf, tag="qk", name="ps_tr")
                    nc.tensor.transpose(
                        ps_tr[:, :], ap_cur[:, it * P : (it + 1) * P], ident
                    )
                    nc.vector.tensor_copy(
                        attT[
                            :,
                            (NP - 1) * NLP + (off0 + it) * P : (NP - 1) * NLP
                            + (off0 + it + 1) * P,
                        ],
                        ps_tr[:, :],
                    )
                    if it >= 1:
                        emit_outB_big(off0 + it - 1, split=(it == w0 - 1))
                emit_outB_big(off0 + w0 - 1, split=True)
                if tail1:
                    # tail i-tile outB last: its 24-row eviction/DMAs drain
                    # far faster than a full 128-row tile's
                    emit_outB_big(tail1[0][0], split=True)

                # ---- slow generic path for wide tail spans (NTL > 9) ----
                for offt, wt in slow_tails:
                    for pair in range(NP):
                        for hh in range(2):
                            h = 2 * pair + hh
                            pa = psa.tile([P, PSW], f32, tag="att", name="pa_w")
                            tw = [sc_wide(h, offt, wt, jt) for jt in range(NTL)]
                            for jt in range(NTL):
                                pv_wide(h, wt, jt, tw[jt], pa)
                            corr_wide(h, offt, wt, pa)
                            if hh == 0:
                                ap_t = app.tile([P, PSW], bf, tag="ap", name="ap_w")
                            norm_wide(h, wt, pa, ap_t)
                        transpose_pair(pair, offt, wt, ap_t)
                    for it in range(offt, offt + wt):
                        for ch in range(2):
                            emit_half(it, ch, 0)

                # B halves for slow-tail tiles (already emitted for the rest)
                for offt, wt in slow_tails:
                    for it in range(offt, offt + wt):
                        for ch in range(2):
                            emit_half(it, ch, 1)

    nc.compile()
    return nc


def _shard_inputs(x, w_qkv, w_out, b_out, mask):
    """Host-side live-token gather + per-core layout prep."""
    bf = ml_dtypes.bfloat16
    x = np.asarray(x, dtype=np.float32)
    w_qkv = np.asarray(w_qkv, dtype=np.float32)
    w_out = np.asarray(w_out, dtype=np.float32)
    mask = np.asarray(mask)

    NLs = [int(mask[b].sum()) for b in range(B)]
    NLP = int(np.ceil((max(NLs) + 1) / 8) * 8)
    NTL = (NLP + P - 1) // P

    w3 = w_qkv.reshape(DM, H, 3, DH)
    in_maps = []
    for c in range(NCORES):
        b, hg = c // HG, c % HG
        nl = NLs[b]
        live = np.nonzero(mask[b])[0]
        dead = np.nonzero(mask[b] == 0)[0]

        wq = w3[:, hg * HL : (hg + 1) * HL, 0, :].reshape(DM, FV) * SCALE
        wk = w3[:, hg * HL : (hg + 1) * HL, 1, :].reshape(DM, FV)
        # pair-major column layout: [q_p0 | k_p0 | q_p1 | k_p1 | ...]
        wqk_c = np.empty((DM, FQK), np.float32)
        for p in range(NP):
            wqk_c[:, p * 256 : p * 256 + 128] = wq[:, p * 128 : (p + 1) * 128]
            wqk_c[:, p * 256 + 128 : (p + 1) * 256] = wk[:, p * 128 : (p + 1) * 128]
        wv_c = np.ascontiguousarray(
            w3[:, hg * HL : (hg + 1) * HL, 2, :].reshape(DM, FV)
        )
        wout_c = np.ascontiguousarray(w_out[hg * FV : (hg + 1) * FV, :])

        xTl_c = np.zeros((DM, NLP), np.float32)
        xTl_c[:, :nl] = x[b].T[:, live]

        padrow_c = np.zeros((1, NLP), np.float32)
        padrow_c[0, nl:] = 1.0
        lind_c = np.zeros(NTL * P, np.float32)
        lind_c[:nl] = 1.0
        lind_c = np.ascontiguousarray(lind_c.reshape(NTL, P).T)

        # dvec: per head [sum_dead v_h | N_dead]
        xs = x[b][dead].sum(axis=0) if len(dead) else np.zeros(DM, np.float32)
        dv = (xs @ wv_c).reshape(HL, DH)
        dvec_c = np.empty((1, VROW), np.float32)
        for h in range(HL):
            dvec_c[0, h * VW : h * VW + DH] = dv[h]
            dvec_c[0, h * VW + DH] = float(len(dead))

        in_maps.append(
            {
                "xTl": xTl_c.astype(bf),
                "wqk": wqk_c.astype(bf),
                "wv": wv_c.astype(bf),
                "wout": wout_c.astype(bf),
                "padrow": padrow_c.astype(bf),
                "lind": lind_c.astype(np.float32),
                "dvec": dvec_c.astype(bf),
            }
        )
    return in_maps, NLP, NLs


def kernel(x, w_qkv, w_out, b_out, mask):
    from concourse.bass_utils import run_bass_kernel_spmd

    in_maps, NLP, NLs = _shard_inputs(x, w_qkv, w_out, b_out, mask)
    if NLP not in _CACHE:
        _CACHE[NLP] = _build_program(NLP)
    nc = _CACHE[NLP]

    res = run_bass_kernel_spmd(nc, in_maps, list(range(NCORES))).results

    mask = np.asarray(mask)
    b_out = np.asarray(b_out, dtype=np.float32)
    out = np.empty((B, N, DM), np.float32)
    for b in range(B):
        nl = NLs[b]
        live = np.nonzero(mask[b])[0]
        dead = np.nonzero(mask[b] == 0)[0]
        part = (
            res[HG * b]["outA"]
            + res[HG * b]["outB"].astype(np.float32)
            + res[HG * b + 1]["outA"]
            + res[HG * b + 1]["outB"].astype(np.float32)
        )
        out[b, live] = part[:nl]
        if len(dead):
            out[b, dead] = part[nl]
        out[b] += b_out[None, :]
    return out


# revision 89
# speedup vs baseline: 1.0018x; 1.0018x over previous
"""Multi-head attention (B=4, N=2048, DM=1024, H=16, DH=64) on 8 trn2 cores.

Sharding: core c -> (batch b = c//2, head-group hg = c%2 of 8 heads).

Live-token compaction: the pair mask only keeps (i,j) score pairs where
both tokens are live, and every dead query row of the reference output is
the SAME uniform average over all value tokens.  So the host gathers the
~NL live tokens of each batch into a compact [DM, NLP] x^T (NLP = padded
live count, multiple of 128, >= NL+1), the device runs attention on live
tokens only, and the host scatters rows back, filling dead rows with the
row produced by the first zero-padded query column.

Per-column semantics on device (q = x@Wq, k = x@Wk, scaled, no masking):
  - live i, live j: t = exp(q_i.k_j) -- the real softmax numerator.
  - pad i (x=0 -> q=0): t = 1 for all j, plus a rank-1 correction row
    (+padrow_i * dvec, dvec = [sum_dead v | N_dead] from the host) makes
    pv = [sum_all v | N], i.e. exactly the reference's uniform dead row.
  - pad j (x=0 -> k=v=0): t = exp(0) = 1 but vplus rows are zero (the
    denominator ones-column is L=live-indicator, not constant 1), so pads
    contribute nothing.

Device layout mirrors the dense kernel: feature-major q/k ([64,NLP] per
head), token-major v with an appended denominator column, scores
transposed [j,i] so PV needs no transpose, exp on ACT, a small [N,64]
transpose per head for the output projection.

Scheduling: heads run as a software pipeline -- window W(h) emits scores
+exp for head h, PV for head h-1, the batched last-i-tile ("tail") scores
for head h-1 and PV for head h-2, plus deadline-scheduled filler chunks
(later pairs' qk projections, then partial output chunks).  The output
projection is split into two half-contractions written to separate DRAM
tensors (outA = pairs 0-1 in f32 overlapped with the mid loop, outB =
pairs 2-3 in bf16 to halve the drain DMA); the host sums them.
"""

import sys

sys.path.insert(0, "/opt/trn_rl_repo")

import numpy as np
import ml_dtypes

B, N, DM, H, DH = 4, 2048, 1024, 16, 64
SCALE = DH**-0.5
NCORES = 8
HG = 2  # head groups (tensor-parallel factor)
HL = H // HG  # 8 heads per core
NP = HL // 2  # 4 head pairs
FQK = HL * 2 * DH  # 1024 qk features per core
FV = HL * DH  # 512 v features per core
P = 128
DMT = DM // P  # 8 dm tiles
VW = DH + 1  # 65: v columns + denominator column
VROW = HL * VW  # 520
HT = FV // P  # 4 head-dim tiles for the projection

_CACHE = {}


def _build_program(NLP):
    import concourse.mybir as mybir
    import concourse.tile as tile
    from concourse import bacc
    from concourse.masks import make_identity

    bf = mybir.dt.bfloat16
    f32 = mybir.dt.float32
    EXP = mybir.ActivationFunctionType.Exp
    COPY = mybir.ActivationFunctionType.Copy

    NTL = (NLP + P - 1) // P  # live token tiles (last may be partial)
    TW = NLP - (NTL - 1) * P  # width of the last tile
    # i-span structure: one wide main span (software-pipelined head loop),
    # remaining tiles handled in the pipelined tail path (w==1) or a slow
    # generic path (w>1, only for much larger masks).
    spans = []
    off = 0
    while NTL - off > 0:
        w = min(8, NTL - off)
        spans.append((off, w))
        off += w
    off0, w0 = spans[0]
    tail1 = [s for s in spans[1:] if s[1] == 1]
    slow_tails = [s for s in spans[1:] if s[1] > 1]

    def IW(it):
        return TW if it == NTL - 1 else P

    # qk_all block stride: a (P-TW)-col pad after each feature block so the
    # full-width kT reads of the partial last j-tile never touch another
    # block's (possibly unwritten) data
    BS = NLP + (P - TW) if TW < P else NLP
    # qk-projection column chunks (psum-bank sized)
    qk_chunks = [(c0, min(512, NLP - c0)) for c0 in range(0, NLP, 512)]

    nc = bacc.Bacc(
        "TRN2", target_bir_lowering=False, debug=False, num_devices=NCORES
    )
    xTl = nc.dram_tensor("xTl", [DM, NLP], bf, kind="ExternalInput")
    wqk = nc.dram_tensor("wqk", [DM, FQK], bf, kind="ExternalInput")
    wv = nc.dram_tensor("wv", [DM, FV], bf, kind="ExternalInput")
    wout = nc.dram_tensor("wout", [FV, DM], bf, kind="ExternalInput")
    padrow = nc.dram_tensor("padrow", [1, NLP], bf, kind="ExternalInput")
    lind = nc.dram_tensor("lind", [P, NTL], f32, kind="ExternalInput")
    dvec = nc.dram_tensor("dvec", [1, VROW], bf, kind="ExternalInput")
    outA = nc.dram_tensor("outA", [NLP, DM], f32, kind="ExternalOutput")
    outB = nc.dram_tensor("outB", [NLP, DM], bf, kind="ExternalOutput")

    with tile.TileContext(nc) as tc:
        with tc.tile_pool(name="const", bufs=1) as cp:
            xTl_sb = cp.tile([P, DMT * NLP], bf, tag="xTl")
            wqk_sb = cp.tile([P, DMT * FQK], bf, tag="wqk")
            wv_sb = cp.tile([P, DMT * FV], bf, tag="wv")
            wout_sb = cp.tile([P, HT * DM], bf, tag="wout")
            padrow_sb = cp.tile([1, NLP], bf, tag="padrow")
            lind_sb = cp.tile([P, NTL], f32, tag="lind")
            dvec_sb = cp.tile([1, VROW], bf, tag="dvec")
            ident = cp.tile([P, P], bf, tag="ident")
            zeros8 = cp.tile([P, HL], bf, tag="zeros8")
            vplus = cp.tile([P, NTL * VROW], bf, tag="vplus")
            qk_all = cp.tile([P, HL * BS], bf, tag="qkall")
            attT = cp.tile([P, HT * NLP], bf, tag="attT")

            # DMA order mirrors consumption: per-dm-tile x^T plus the
            # pair-0 qk weight columns first (feeds the first projection),
            # then v weights (needed by the head-1 window), small tensors,
            # then the remaining qk weight columns and w_out.
            for dmt in range(DMT):
                nc.sync.dma_start(
                    out=wqk_sb[:, dmt * FQK : dmt * FQK + 3 * P],
                    in_=wqk[dmt * P : (dmt + 1) * P, 0 : 3 * P],
                )
                nc.sync.dma_start(
                    out=xTl_sb[:, dmt * NLP : (dmt + 1) * NLP],
                    in_=xTl[dmt * P : (dmt + 1) * P, :],
                )
            nc.sync.dma_start(
                out=wv_sb[:, :].rearrange("p (d f) -> p d f", d=DMT, f=FV),
                in_=wv[:, :].rearrange("(d p) c -> p d c", p=P),
            )
            nc.sync.dma_start(out=lind_sb[:, :], in_=lind[:, :])
            nc.sync.dma_start(out=padrow_sb[:, :], in_=padrow[:, :])
            nc.sync.dma_start(out=dvec_sb[:, :], in_=dvec[:, :])
            for dmt in range(DMT):
                nc.sync.dma_start(
                    out=wqk_sb[:, dmt * FQK + 3 * P : (dmt + 1) * FQK],
                    in_=wqk[dmt * P : (dmt + 1) * P, 3 * P :],
                )
            for ht in range(HT):
                nc.sync.dma_start(
                    out=wout_sb[:, ht * DM : (ht + 1) * DM],
                    in_=wout[ht * P : (ht + 1) * P, :],
                )
            make_identity(nc, ident)
            nc.gpsimd.memset(zeros8[:, :], 0.0)
            # zero the pad margin after each feature block (spill target of
            # the full-width kT reads of the partial last j-tile)
            if TW < P:
                for f in range(HL):
                    nc.gpsimd.memset(qk_all[:, f * BS + NLP : (f + 1) * BS], 0.0)
            if TW < P:
                # rows of the partial last j-tile beyond the live+pad range
                # never get v written; zero the whole block up front (the v
                # eviction later overwrites rows [0:TW]) so spilled-garbage
                # exp rows contract against zeros
                nc.gpsimd.memset(vplus[:, (NTL - 1) * VROW : NTL * VROW], 0.0)

            vp4 = vplus.rearrange("p (t g c) -> p t g c", t=NTL, g=HL, c=VW)

            with (
                tc.tile_pool(name="psqk", bufs=2, space="PSUM") as pqk,
                tc.tile_pool(name="pss", bufs=2, space="PSUM") as pss,
                tc.tile_pool(name="psa", bufs=1, space="PSUM") as psa,
                tc.tile_pool(name="tpool", bufs=20) as tp,
                tc.tile_pool(name="ttpool", bufs=3) as ttp,
                tc.tile_pool(name="appool", bufs=2) as app,
                tc.tile_pool(name="spool", bufs=4) as sp,
            ):
                PSW = min(w0 * P, 1024)

                def emit_qk_chunk(f, c0, cw):
                    ps = pqk.tile([P, 512], f32, tag="qk", name="ps_qk")
                    for dmt in range(DMT):
                        nc.tensor.matmul(
                            ps[:, :cw],
                            wqk_sb[:, dmt * FQK + f * P : dmt * FQK + (f + 1) * P],
                            xTl_sb[:, dmt * NLP + c0 : dmt * NLP + c0 + cw],
                            start=(dmt == 0),
                            stop=(dmt == DMT - 1),
                        )
                    nc.vector.tensor_copy(
                        qk_all[:, f * BS + c0 : f * BS + c0 + cw], ps[:, :cw]
                    )

                def emit_v(tt):
                    W = IW(tt)
                    ps = pqk.tile([P, FV], f32, tag="qk", name="ps_v")
                    for dmt in range(DMT):
                        nc.tensor.matmul(
                            ps[0:W, :],
                            xTl_sb[:, dmt * NLP + tt * P : dmt * NLP + tt * P + W],
                            wv_sb[:, dmt * FV : (dmt + 1) * FV],
                            start=(dmt == 0),
                            stop=(dmt == DMT - 1),
                        )
                    nc.vector.tensor_copy(
                        vp4[0:W, tt, :, 0:DH],
                        ps[0:W].rearrange("p (g c) -> p g c", g=HL, c=DH),
                    )
                    # denominator column = live indicator (0 for pad rows)
                    nc.vector.tensor_scalar_add(
                        vp4[0:W, tt, :, DH],
                        zeros8[0:W, :],
                        lind_sb[0:W, tt : tt + 1],
                    )

                def sc_wide(h, off, w, jt):
                    pair, hh = h // 2, h % 2
                    p0 = hh * 64
                    qc = (2 * pair) * BS + off * P
                    kc = (2 * pair + 1) * BS
                    ps_s = pss.tile([P, PSW], f32, tag="s", name="ps_s")
                    for c0 in range(0, w * P, 512):
                        cw = min(512, w * P - c0)
                        nc.tensor.matmul(
                            ps_s[:, c0 : c0 + cw],
                            qk_all[p0 : p0 + 64, kc + jt * P : kc + (jt + 1) * P],
                            qk_all[p0 : p0 + 64, qc + c0 : qc + c0 + cw],
                            start=True,
                            stop=True,
                        )
                    t_sb = tp.tile([P, PSW], bf, tag="t", name="t_sb")
                    nc.scalar.activation(t_sb[:, : w * P], ps_s[:, : w * P], EXP)
                    return t_sb

                def pv_wide(h, w, jt, t_sb, pa):
                    vsl = vplus[:, jt * VROW + h * VW : jt * VROW + (h + 1) * VW]
                    for it in range(w):
                        nc.tensor.matmul(
                            pa[:, it * P : it * P + VW],
                            t_sb[:, it * P : (it + 1) * P],
                            vsl,
                            start=(jt == 0 and it % 4 == 0),
                            stop=False,
                        )

                def corr_wide(h, off, w, pa):
                    for it in range(w):
                        nc.tensor.matmul(
                            pa[:, it * P : it * P + VW],
                            padrow_sb[:, (off + it) * P : (off + it + 1) * P],
                            dvec_sb[:, h * VW : (h + 1) * VW],
                            start=False,
                            stop=(it % 4 == 3 or it == w - 1),
                        )

                def norm_wide(h, w, pa, ap):
                    p0 = (h % 2) * 64
                    r_sb = sp.tile([P, 8], f32, tag="r", name="r_sb")
                    pa3 = pa.rearrange("p (t c) -> p t c", t=PSW // P, c=P)
                    nc.vector.reciprocal(r_sb[:, :w], pa3[:, :w, DH])
                    for it in range(w):
                        nc.vector.tensor_scalar_mul(
                            ap[:, it * P + p0 : it * P + p0 + DH],
                            pa[:, it * P : it * P + DH],
                            r_sb[:, it : it + 1],
                        )

                def pv_slot(h, off, w, it, pa, t_list, ap):
                    # slot-major: finish output tile `it` for head h in one
                    # go (all-j PV + correction + normalize), so downstream
                    # per-tile work pipelines inside the window
                    p0 = (h % 2) * 64
                    vcol = h * VW
                    for jt in range(NTL):
                        nc.tensor.matmul(
                            pa[:, it * P : it * P + VW],
                            t_list[jt][:, it * P : (it + 1) * P],
                            vplus[:, jt * VROW + vcol : jt * VROW + vcol + VW],
                            start=(jt == 0 and it % 4 == 0),
                            stop=False,
                        )
                    nc.tensor.matmul(
                        pa[:, it * P : it * P + VW],
                        padrow_sb[:, (off + it) * P : (off + it + 1) * P],
                        dvec_sb[:, h * VW : (h + 1) * VW],
                        start=False,
                        stop=(it % 4 == 3 or it == w - 1),
                    )
                    r1 = sp.tile([P, 8], f32, tag="r", name="r1")
                    nc.vector.reciprocal(
                        r1[:, 0:1], pa[:, it * P + DH : it * P + DH + 1]
                    )
                    nc.vector.tensor_scalar_mul(
                        ap[:, it * P + p0 : it * P + p0 + DH],
                        pa[:, it * P : it * P + DH],
                        r1[:, 0:1],
                    )

                def transpose_it(pair, off, it, ap):
                    ps_tr = pqk.tile([P, P], bf, tag="qk", name="ps_tr")
                    nc.tensor.transpose(
                        ps_tr[:, :], ap[:, it * P : (it + 1) * P], ident
                    )
                    dst = attT[
                        :,
                        pair * NLP + (off + it) * P : pair * NLP
                        + (off + it + 1) * P,
                    ]
                    nc.vector.tensor_copy(dst, ps_tr[:, :])

                def transpose_pair(pair, off, w, ap):
                    # batch 4 transposes per 1-bank slot -> one eviction copy
                    it = 0
                    while it < w:
                        nb = min(4, w - it)
                        if IW(off + it + nb - 1) != P:
                            nb = 1
                        ps_tr = pqk.tile([P, 512], bf, tag="qk", name="ps_tr")
                        wtot = 0
                        for k in range(nb):
                            W = IW(off + it + k)
                            nc.tensor.transpose(
                                ps_tr[:, k * P : k * P + W],
                                ap[0:W, (it + k) * P : (it + k + 1) * P],
                                ident[0:W, 0:W],
                            )
                            wtot = k * P + W
                        nc.vector.tensor_copy(
                            attT[
                                :,
                                pair * NLP + (off + it) * P : pair * NLP
                                + (off + it) * P
                                + wtot,
                            ],
                            ps_tr[:, 0:wtot],
                        )
                        it += nb

                def s_tail(h, off):
                    # batched narrow-tail scores: the TW-wide last i-tile for
                    # all j-tiles, packed into as few psum banks / exp
                    # instructions as possible
                    pair, hh = h // 2, h % 2
                    p0 = hh * 64
                    qc = (2 * pair) * BS + off * P
                    kc = (2 * pair + 1) * BS
                    bpb = max(1, 512 // TW)  # batches per psum bank
                    t_t = ttp.tile([P, NTL * TW], bf, tag="tt", name="t_tail")
                    done = 0
                    while done < NTL:
                        nb = min(2 * bpb, NTL - done)  # one 2-bank slot
                        ps_s = pss.tile([P, PSW], f32, tag="s", name="ps_st")
                        for g in range(nb):
                            jt = done + g
                            pos = (g // bpb) * 512 + (g % bpb) * TW
                            nc.tensor.matmul(
                                ps_s[:, pos : pos + TW],
                                qk_all[p0 : p0 + 64, kc + jt * P : kc + jt * P + P],
                                qk_all[p0 : p0 + 64, qc : qc + TW],
                                start=True,
                                stop=True,
                            )
                        for bk in range((nb + bpb - 1) // bpb):
                            cnt = min(bpb, nb - bk * bpb)
                            nc.scalar.activation(
                                t_t[
                                    :,
                                    (done + bk * bpb) * TW : (done + bk * bpb + cnt)
                                    * TW,
                                ],
                                ps_s[:, bk * 512 : bk * 512 + cnt * TW],
                                EXP,
                            )
                        done += nb
                    return t_t

                def p_tail(h, off, t_t, ap):
                    # 65-col accumulator lives in a 1-bank "qk" slot so it
                    # never waits on the main PV accumulator (deadlock risk)
                    p0 = (h % 2) * 64
                    pa = pqk.tile([P, 512], f32, tag="qk", name="pa_t")
                    for jt in range(NTL):
                        nc.tensor.matmul(
                            pa[0:TW, 0:VW],
                            t_t[:, jt * TW : (jt + 1) * TW],
                            vplus[:, jt * VROW + h * VW : jt * VROW + (h + 1) * VW],
                            start=(jt == 0),
                            stop=False,
                        )
                    nc.tensor.matmul(
                        pa[0:TW, 0:VW],
                        padrow_sb[:, off * P : off * P + TW],
                        dvec_sb[:, h * VW : (h + 1) * VW],
                        start=False,
                        stop=True,
                    )
                    r_sb = sp.tile([P, 8], f32, tag="r", name="r_t")
                    nc.vector.reciprocal(r_sb[0:TW, 0:1], pa[0:TW, DH : DH + 1])
                    nc.vector.tensor_scalar_mul(
                        ap[0:TW, p0 : p0 + DH], pa[0:TW, 0:DH], r_sb[0:TW, 0:1]
                    )

                nout = [0]

                def emit_half(it, ch, half):
                    # half 0: pairs 0-1 -> outA f32; half 1: pairs 2-3 -> outB bf16
                    W = IW(it)
                    ps_o = pqk.tile([P, 512], f32, tag="qk", name="ps_o")
                    for ht in (0, 1) if half == 0 else (2, 3):
                        nc.tensor.matmul(
                            ps_o[0:W, :],
                            attT[:, ht * NLP + it * P : ht * NLP + it * P + W],
                            wout_sb[:, ht * DM + ch * 512 : ht * DM + (ch + 1) * 512],
                            start=(ht % 2 == 0),
                            stop=(ht % 2 == 1),
                        )
                    dt = f32 if half == 0 else bf
                    o_sb = sp.tile([P, 512], dt, tag="obA" if half == 0 else "obB",
                                   name="o_sb")
                    # A-half evictions stay off ACT (it paces mid-loop exps);
                    # B-half runs in the drain where ACT is idle.
                    if half == 1:
                        nc.scalar.activation(o_sb[0:W, :], ps_o[0:W, :], COPY)
                    else:
                        nc.vector.tensor_copy(o_sb[0:W, :], ps_o[0:W, :])
                    nout[0] += 1
                    dst = outA if half == 0 else outB
                    nc.sync.dma_start(
                        out=dst[it * P : it * P + W, ch * 512 : (ch + 1) * 512],
                        in_=o_sb[0:W, :],
                    )

                # ---- filler queues ----
                # qk chunks for pairs 1..3: pair p before head 2p's scores.
                fast_start = len(qk_chunks) == 3
                fill_units = [
                    (f, c0, cw)
                    for pair in range(1, NP)
                    for f in (2 * pair, 2 * pair + 1)
                    for (c0, cw) in qk_chunks
                    if not (fast_start and f == 2 and c0 < qk_chunks[2][0])
                ]
                n_units = len(fill_units)
                cpp = 2 * len(qk_chunks)
                fill_pos = [0]
                cpair1 = cpp - (2 if fast_start else 0)

                def emit_fill_to(tgt):
                    k = fill_pos[0]
                    for u in fill_units[k : min(n_units, tgt)]:
                        emit_qk_chunk(*u)
                    fill_pos[0] = max(k, min(n_units, tgt))

                def needed_before(h):
                    p = max(0, h // 2)
                    return min(n_units, cpair1 if p == 1 else
                               cpair1 + (p - 1) * cpp if p > 1 else 0)

                # A half-chunks (pairs 0-1): ready once pair-1 main+tail
                # transposes are done (end of window 5); fill windows 6-7.
                a_units = [
                    (it, ch) for it in range(off0, off0 + w0) for ch in range(2)
                ]
                nA = len(a_units)
                a_pos = [0]

                def emit_a_to(tgt):
                    k = a_pos[0]
                    for u in a_units[k : min(nA, tgt)]:
                        emit_half(u[0], u[1], 0)
                    a_pos[0] = max(k, min(nA, tgt))

                # ---- window 0: pair-0 projections + head-0 scores,
                #      pair-1 qk chunks interleaved ----
                if len(qk_chunks) == 3:
                    # dmt-outer interleave across the f0/f1 chunks plus
                    # pair-1's first q chunks keeps the PE fed at DMA
                    # arrival pace (chunk-slots borrowed from the idle
                    # pss/psa pools + pqk)
                    sA = pss.tile([P, PSW], f32, tag="s", name="ps_q0")
                    sB = pss.tile([P, PSW], f32, tag="s", name="ps_q1")
                    qA = pqk.tile([P, 512], f32, tag="qk", name="ps_q2")
                    qB = pqk.tile([P, 512], f32, tag="qk", name="ps_q3")
                    aA = psa.tile([P, PSW], f32, tag="att", name="ps_q4")
                    units = [
                        (0, qk_chunks[0][0], qk_chunks[0][1], sA, 0),
                        (0, qk_chunks[1][0], qk_chunks[1][1], sA, 512),
                        (0, qk_chunks[2][0], qk_chunks[2][1], sB, 0),
                        (1, qk_chunks[0][0], qk_chunks[0][1], sB, 512),
                        (1, qk_chunks[1][0], qk_chunks[1][1], qA, 0),
                        (1, qk_chunks[2][0], qk_chunks[2][1], qB, 0),
                        (2, qk_chunks[0][0], qk_chunks[0][1], aA, 0),
                        (2, qk_chunks[1][0], qk_chunks[1][1], aA, 512),
                    ]
                    for dmt in range(DMT):
                        for f, c0, cw, ps, so in units:
                            nc.tensor.matmul(
                                ps[:, so : so + cw],
                                wqk_sb[
                                    :, dmt * FQK + f * P : dmt * FQK + (f + 1) * P
                                ],
                                xTl_sb[:, dmt * NLP + c0 : dmt * NLP + c0 + cw],
                                start=(dmt == 0),
                                stop=(dmt == DMT - 1),
                            )
                    for f, c0, cw, ps, so in units:
                        nc.vector.tensor_copy(
                            qk_all[:, f * BS + c0 : f * BS + c0 + cw],
                            ps[:, so : so + cw],
                        )
                else:
                    for f in (0, 1):
                        for c0, cw in qk_chunks:
                            emit_qk_chunk(f, c0, cw)
                t_store = {0: []}
                tt_store = {}
                for jt in range(NTL):
                    t_store[0].append(sc_wide(0, off0, w0, jt))
                if tail1:
                    # pair-0 tail scores cover the wv DMA wait; their spill
                    # reads (pair-1 q block) are written by the fast startup
                    tt_store[0] = s_tail(0, tail1[0][0])
                    tt_store[1] = s_tail(1, tail1[0][0])
                for tt in range(min(3, NTL)):
                    emit_v(tt)
                emit_fill_to(cpair1)

                # ---- windows 1..7: S(h) || PV(h-1) || tail(h-1 scores,
                #      h-2 PV) || fillers ----
                ap_cur = None
                apt_cur = None
                for h in range(1, HL):
                    emit_fill_to(needed_before(h))
                    fprev = fill_pos[0]
                    fth = n_units if h >= 5 else max(
                        needed_before(h + 1), (n_units * h + 4) // 5
                    )
                    aprev = a_pos[0]
                    ath = {5: 6, 6: 14}.get(h, 0 if h < 5 else nA)
                    t_store[h] = []
                    pa = psa.tile([P, PSW], f32, tag="att", name="pa")
                    for jt in range(NTL):
                        t_store[h].append(sc_wide(h, off0, w0, jt))
                        if h == 1 and jt >= 3:
                            emit_v(jt)
                        emit_a_to(aprev + ((ath - aprev) * (jt + 1)) // NTL)
                        emit_fill_to(fprev + ((fth - fprev) * (jt + 1)) // NTL)
                        pv_wide(h - 1, w0, jt, t_store[h - 1][jt], pa)
                        t_store[h - 1][jt] = None
                        if jt == 2 and h >= 2 and tail1 and (h - 2) in tt_store:
                            # lag-2 tail PV for head h-2
                            offt = tail1[0][0]
                            if (h - 2) % 2 == 0:
                                apt_cur = app.tile([P, P], bf, tag="apt", name="apt")
                            p_tail(h - 2, offt, tt_store.pop(h - 2), apt_cur)
                            if (h - 2) % 2 == 1:
                                transpose_pair((h - 2) // 2, offt, 1, apt_cur)
                    if tail1 and h >= 3:
                        tt_store[h - 1] = s_tail(h - 1, tail1[0][0])
                    if tail1 and h == HL - 1:
                        tt_store[h] = s_tail(h, tail1[0][0])
                    corr_wide(h - 1, off0, w0, pa)
                    if (h - 1) % 2 == 0:
                        ap_cur = app.tile([P, PSW], bf, tag="ap", name="ap")
                    norm_wide(h - 1, w0, pa, ap_cur)
                    if (h - 1) % 2 == 1:
                        transpose_pair((h - 1) // 2, off0, w0, ap_cur)

                # ---- epilogue: PV(7), tails 6-7, pipelined pair-3 finish ----
                def emit_outB_big(it, split=False):
                    # whole-row B chunk: pairs 2-3 for both DM halves in one
                    # 2-bank pss slot, one eviction, one outB DMA
                    W = IW(it)
                    ps_o = pss.tile([P, PSW], f32, tag="s", name="ps_b")
                    for ch in range(2):
                        for ht in (2, 3):
                            nc.tensor.matmul(
                                ps_o[0:W, ch * 512 : (ch + 1) * 512],
                                attT[:, ht * NLP + it * P : ht * NLP + it * P + W],
                                wout_sb[
                                    :, ht * DM + ch * 512 : ht * DM + (ch + 1) * 512
                                ],
                                start=(ht == 2),
                                stop=(ht == 3),
                            )
                    o_sb = sp.tile([P, 1024], bf, tag="obB", name="o_sbB")
                    if split:
                        # parallel eviction halves (ACT + DVE): shallow drain
                        nc.scalar.activation(
                            o_sb[0:W, 0:512], ps_o[0:W, 0:512], COPY
                        )
                        nc.vector.tensor_copy(
                            o_sb[0:W, 512:1024], ps_o[0:W, 512:1024]
                        )
                        nc.sync.dma_start(
                            out=outB[it * P : it * P + W, 0:512],
                            in_=o_sb[0:W, 0:512],
                        )
                        nc.sync.dma_start(
                            out=outB[it * P : it * P + W, 512:1024],
                            in_=o_sb[0:W, 512:1024],
                        )
                        return
                    if nout[0] % 2 == 0:
                        nc.scalar.activation(o_sb[0:W, :], ps_o[0:W, :1024], COPY)
                    else:
                        nc.vector.tensor_copy(o_sb[0:W, :], ps_o[0:W, :1024])
                    nout[0] += 1
                    nc.sync.dma_start(
                        out=outB[it * P : it * P + W, :], in_=o_sb[0:W, :]
                    )

                if tail1 and (HL - 1) not in tt_store:
                    tt_store[HL - 1] = s_tail(HL - 1, tail1[0][0])
                emit_fill_to(n_units)
                aprev = a_pos[0]
                pa = psa.tile([P, PSW], f32, tag="att", name="pa")
                for jt in range(NTL):
                    emit_a_to(aprev + ((nA - aprev) * (jt + 1)) // NTL)
                    pv_wide(HL - 1, w0, jt, t_store[HL - 1][jt], pa)
                    if jt == 2 and tail1 and (HL - 2) in tt_store:
                        offt = tail1[0][0]
                        apt_cur = app.tile([P, P], bf, tag="apt", name="apt")
                        p_tail(HL - 2, offt, tt_store.pop(HL - 2), apt_cur)
                    if jt == 4 and tail1 and (HL - 1) in tt_store:
                        offt = tail1[0][0]
                        p_tail(HL - 1, offt, tt_store.pop(HL - 1), apt_cur)
                        transpose_pair(NP - 1, offt, 1, apt_cur)
                    if jt == 0 and tail1:
                        # tail i-tile outA rows (pairs 0-1 only): PE work to
                        # cover the window-boundary normalize wait
                        offt = tail1[0][0]
                        emit_half(offt, 0, 0)
                        emit_half(offt, 1, 0)

                corr_wide(HL - 1, off0, w0, pa)
                emit_a_to(nA)
                # pipelined pair-3 finish: normalize slot -> transpose ->
                # previous tile's whole-row B chunk (hides eviction latency)
                p0e = ((HL - 1) % 2) * 64
                r_sb = sp.tile([P, 8], f32, tag="r", name="r_e")
                pa3 = pa.rearrange("p (t c) -> p t c", t=PSW // P, c=P)
                nc.vector.reciprocal(r_sb[:, :w0], pa3[:, :w0, DH])
                for it in range(w0):
                    nc.vector.tensor_scalar_mul(
                        ap_cur[:, it * P + p0e : it * P + p0e + DH],
                        pa[:, it * P : it * P + DH],
                        r_sb[:, it : it + 1],
                    )
                    ps_tr = pqk.tile([P, P], bf, tag="qk", name="ps_tr")
                    nc.tensor.transpose(
                        ps_tr[:, :], ap_cur[:, it * P : (it + 1) * P], ident
                    )
                    nc.vector.tensor_copy(
                        attT[
                            :,
                            (NP - 1) * NLP + (off0 + it) * P : (NP - 1) * NLP
                            + (off0 + it + 1) * P,
                        ],
                        ps_tr[:, :],
                    )
                    if it >= 1:
                        emit_outB_big(off0 + it - 1, split=(it == w0 - 1))
                emit_outB_big(off0 + w0 - 1, split=True)
                if tail1:
                    # tail i-tile outB last: its 24-row eviction/DMAs drain
                    # far faster than a full 128-row tile's
                    emit_outB_big(tail1[0][0], split=True)

                # ---- slow generic path for wide tail spans (NTL > 9) ----
                for offt, wt in slow_tails:
                    for pair in range(NP):
                        for hh in range(2):
                            h = 2 * pair + hh
                            pa = psa.tile([P, PSW], f32, tag="att", name="pa_w")
                            tw = [sc_wide(h, offt, wt, jt) for jt in range(NTL)]
                            for jt in range(NTL):
                                pv_wide(h, wt, jt, tw[jt], pa)
                            corr_wide(h, offt, wt, pa)
                            if hh == 0:
                                ap_t = app.tile([P, PSW], bf, tag="ap", name="ap_w")
                            norm_wide(h, wt, pa, ap_t)
                        transpose_pair(pair, offt, wt, ap_t)
                    for it in range(offt, offt + wt):
                        for ch in range(2):
                            emit_half(it, ch, 0)

                # B halves for slow-tail tiles (already emitted for the rest)
                for offt, wt in slow_tails:
                    for it in range(offt, offt + wt):
                        for ch in range(2):
                            emit_half(it, ch, 1)

    nc.compile()
    return nc


def _shard_inputs(x, w_qkv, w_out, b_out, mask):
    """Host-side live-token gather + per-core layout prep."""
    bf = ml_dtypes.bfloat16
    x = np.asarray(x, dtype=np.float32)
    w_qkv = np.asarray(w_qkv, dtype=np.float32)
    w_out = np.asarray(w_out, dtype=np.float32)
    mask = np.asarray(mask)

    NLs = [int(mask[b].sum()) for b in range(B)]
    NLP = int(np.ceil((max(NLs) + 1) / 8) * 8)
    NTL = (NLP + P - 1) // P

    w3 = w_qkv.reshape(DM, H, 3, DH)
    in_maps = []
    for c in range(NCORES):
        b, hg = c // HG, c % HG
        nl = NLs[b]
        live = np.nonzero(mask[b])[0]
        dead = np.nonzero(mask[b] == 0)[0]

        wq = w3[:, hg * HL : (hg + 1) * HL, 0, :].reshape(DM, FV) * SCALE
        wk = w3[:, hg * HL : (hg + 1) * HL, 1, :].reshape(DM, FV)
        # pair-major column layout: [q_p0 | k_p0 | q_p1 | k_p1 | ...]
        wqk_c = np.empty((DM, FQK), np.float32)
        for p in range(NP):
            wqk_c[:, p * 256 : p * 256 + 128] = wq[:, p * 128 : (p + 1) * 128]
            wqk_c[:, p * 256 + 128 : (p + 1) * 256] = wk[:, p * 128 : (p + 1) * 128]
        wv_c = np.ascontiguousarray(
            w3[:, hg * HL : (hg + 1) * HL, 2, :].reshape(DM, FV)
        )
        wout_c = np.ascontiguousarray(w_out[hg * FV : (hg + 1) * FV, :])

        xTl_c = np.zeros((DM, NLP), np.float32)
        xTl_c[:, :nl] = x[b].T[:, live]

        padrow_c = np.zeros((1, NLP), np.float32)
        padrow_c[0, nl:] = 1.0
        lind_c = np.zeros(NTL * P, np.float32)
        lind_c[:nl] = 1.0
        lind_c = np.ascontiguousarray(lind_c.reshape(NTL, P).T)

        # dvec: per head [sum_dead v_h | N_dead]
        xs = x[b][dead].sum(axis=0) if len(dead) else np.zeros(DM, np.float32)
        dv = (xs @ wv_c).reshape(HL, DH)
        dvec_c = np.empty((1, VROW), np.float32)
        for h in range(HL):
            dvec_c[0, h * VW : h * VW + DH] = dv[h]
            dvec_c[0, h * VW + DH] = float(len(dead))

        in_maps.append(
            {
                "xTl": xTl_c.astype(bf),
                "wqk": wqk_c.astype(bf),
                "wv": wv_c.astype(bf),
                "wout": wout_c.astype(bf),
                "padrow": padrow_c.astype(bf),
                "lind": lind_c.astype(np.float32),
                "dvec": dvec_c.astype(bf),
            }
        )
    return in_maps, NLP, NLs


def kernel(x, w_qkv, w_out, b_out, mask):
    from concourse.bass_utils import run_bass_kernel_spmd

    in_maps, NLP, NLs = _shard_inputs(x, w_qkv, w_out, b_out, mask)
    if NLP not in _CACHE:
        _CACHE[NLP] = _build_program(NLP)
    nc = _CACHE[NLP]

    res = run_bass_kernel_spmd(nc, in_maps, list(range(NCORES))).results

    mask = np.asarray(mask)
    b_out = np.asarray(b_out, dtype=np.float32)
    out = np.empty((B, N, DM), np.float32)
    for b in range(B):
        nl = NLs[b]
        live = np.nonzero(mask[b])[0]
        dead = np.nonzero(mask[b] == 0)[0]
        part = (
            res[HG * b]["outA"]
            + res[HG * b]["outB"].astype(np.float32)
            + res[HG * b + 1]["outA"]
            + res[HG * b + 1]["outB"].astype(np.float32)
        )
        out[b, live] = part[:nl]
        if len(dead):
            out[b, dead] = part[nl]
        out[b] += b_out[None, :]
    return out


# revision 92
# speedup vs baseline: 1.0018x; 1.0000x over previous
"""Multi-head attention (B=4, N=2048, DM=1024, H=16, DH=64) on 8 trn2 cores.

Sharding: core c -> (batch b = c//2, head-group hg = c%2 of 8 heads).

Live-token compaction: the pair mask only keeps (i,j) score pairs where
both tokens are live, and every dead query row of the reference output is
the SAME uniform average over all value tokens.  So the host gathers the
~NL live tokens of each batch into a compact [DM, NLP] x^T (NLP = padded
live count, multiple of 128, >= NL+1), the device runs attention on live
tokens only, and the host scatters rows back, filling dead rows with the
row produced by the first zero-padded query column.

Per-column semantics on device (q = x@Wq, k = x@Wk, scaled, no masking):
  - live i, live j: t = exp(q_i.k_j) -- the real softmax numerator.
  - pad i (x=0 -> q=0): t = 1 for all j, plus a rank-1 correction row
    (+padrow_i * dvec, dvec = [sum_dead v | N_dead] from the host) makes
    pv = [sum_all v | N], i.e. exactly the reference's uniform dead row.
  - pad j (x=0 -> k=v=0): t = exp(0) = 1 but vplus rows are zero (the
    denominator ones-column is L=live-indicator, not constant 1), so pads
    contribute nothing.

Device layout mirrors the dense kernel: feature-major q/k ([64,NLP] per
head), token-major v with an appended denominator column, scores
transposed [j,i] so PV needs no transpose, exp on ACT, a small [N,64]
transpose per head for the output projection.

Scheduling: heads run as a software pipeline -- window W(h) emits scores
+exp for head h, PV for head h-1, the batched last-i-tile ("tail") scores
for head h-1 and PV for head h-2, plus deadline-scheduled filler chunks
(later pairs' qk projections, then partial output chunks).  The output
projection is split into two half-contractions written to separate DRAM
tensors (outA = pairs 0-1 in f32 overlapped with the mid loop, outB =
pairs 2-3 in bf16 to halve the drain DMA); the host sums them.
"""

import sys

sys.path.insert(0, "/opt/trn_rl_repo")

import numpy as np
import ml_dtypes

B, N, DM, H, DH = 4, 2048, 1024, 16, 64
SCALE = DH**-0.5
NCORES = 8
HG = 2  # head groups (tensor-parallel factor)
HL = H // HG  # 8 heads per core
NP = HL // 2  # 4 head pairs
FQK = HL * 2 * DH  # 1024 qk features per core
FV = HL * DH  # 512 v features per core
P = 128
DMT = DM // P  # 8 dm tiles
VW = DH + 1  # 65: v columns + denominator column
VROW = HL * VW  # 520
HT = FV // P  # 4 head-dim tiles for the projection

_CACHE = {}


def _build_program(NLP):
    import concourse.mybir as mybir
    import concourse.tile as tile
    from concourse import bacc
    from concourse.masks import make_identity

    bf = mybir.dt.bfloat16
    f32 = mybir.dt.float32
    EXP = mybir.ActivationFunctionType.Exp
    COPY = mybir.ActivationFunctionType.Copy

    NTL = (NLP + P - 1) // P  # live token tiles (last may be partial)
    TW = NLP - (NTL - 1) * P  # width of the last tile
    # i-span structure: one wide main span (software-pipelined head loop),
    # remaining tiles handled in the pipelined tail path (w==1) or a slow
    # generic path (w>1, only for much larger masks).
    spans = []
    off = 0
    while NTL - off > 0:
        w = min(8, NTL - off)
        spans.append((off, w))
        off += w
    off0, w0 = spans[0]
    tail1 = [s for s in spans[1:] if s[1] == 1]
    slow_tails = [s for s in spans[1:] if s[1] > 1]

    def IW(it):
        return TW if it == NTL - 1 else P

    # qk_all block stride: a (P-TW)-col pad after each feature block so the
    # full-width kT reads of the partial last j-tile never touch another
    # block's (possibly unwritten) data
    BS = NLP + (P - TW) if TW < P else NLP
    # qk-projection column chunks (psum-bank sized)
    qk_chunks = [(c0, min(512, NLP - c0)) for c0 in range(0, NLP, 512)]

    nc = bacc.Bacc(
        "TRN2", target_bir_lowering=False, debug=False, num_devices=NCORES
    )
    xTl = nc.dram_tensor("xTl", [DM, NLP], bf, kind="ExternalInput")
    wqk = nc.dram_tensor("wqk", [DM, FQK], bf, kind="ExternalInput")
    wv = nc.dram_tensor("wv", [DM, FV], bf, kind="ExternalInput")
    wout = nc.dram_tensor("wout", [FV, DM], bf, kind="ExternalInput")
    padrow = nc.dram_tensor("padrow", [1, NLP], bf, kind="ExternalInput")
    lind = nc.dram_tensor("lind", [P, NTL], f32, kind="ExternalInput")
    dvec = nc.dram_tensor("dvec", [1, VROW], bf, kind="ExternalInput")
    outA = nc.dram_tensor("outA", [NLP, DM], f32, kind="ExternalOutput")
    outB = nc.dram_tensor("outB", [NLP, DM], bf, kind="ExternalOutput")

    with tile.TileContext(nc) as tc:
        with tc.tile_pool(name="const", bufs=1) as cp:
            xTl_sb = cp.tile([P, DMT * NLP], bf, tag="xTl")
            wqk_sb = cp.tile([P, DMT * FQK], bf, tag="wqk")
            wv_sb = cp.tile([P, DMT * FV], bf, tag="wv")
            wout_sb = cp.tile([P, HT * DM], bf, tag="wout")
            padrow_sb = cp.tile([1, NLP], bf, tag="padrow")
            lind_sb = cp.tile([P, NTL], f32, tag="lind")
            dvec_sb = cp.tile([1, VROW], bf, tag="dvec")
            ident = cp.tile([P, P], bf, tag="ident")
            zeros8 = cp.tile([P, HL], bf, tag="zeros8")
            vplus = cp.tile([P, NTL * VROW], bf, tag="vplus")
            qk_all = cp.tile([P, HL * BS], bf, tag="qkall")
            attT = cp.tile([P, HT * NLP], bf, tag="attT")

            # DMA order mirrors consumption: per-dm-tile x^T plus the
            # pair-0 qk weight columns first (feeds the first projection),
            # then v weights (needed by the head-1 window), small tensors,
            # then the remaining qk weight columns and w_out.
            for dmt in range(DMT):
                nc.sync.dma_start(
                    out=wqk_sb[:, dmt * FQK : dmt * FQK + 3 * P],
                    in_=wqk[dmt * P : (dmt + 1) * P, 0 : 3 * P],
                )
                nc.sync.dma_start(
                    out=xTl_sb[:, dmt * NLP : (dmt + 1) * NLP],
                    in_=xTl[dmt * P : (dmt + 1) * P, :],
                )
            nc.sync.dma_start(
                out=wv_sb[:, :].rearrange("p (d f) -> p d f", d=DMT, f=FV),
                in_=wv[:, :].rearrange("(d p) c -> p d c", p=P),
            )
            nc.sync.dma_start(out=lind_sb[:, :], in_=lind[:, :])
            nc.sync.dma_start(out=padrow_sb[:, :], in_=padrow[:, :])
            nc.sync.dma_start(out=dvec_sb[:, :], in_=dvec[:, :])
            for dmt in range(DMT):
                nc.sync.dma_start(
                    out=wqk_sb[:, dmt * FQK + 3 * P : (dmt + 1) * FQK],
                    in_=wqk[dmt * P : (dmt + 1) * P, 3 * P :],
                )
            for ht in range(HT):
                nc.sync.dma_start(
                    out=wout_sb[:, ht * DM : (ht + 1) * DM],
                    in_=wout[ht * P : (ht + 1) * P, :],
                )
            make_identity(nc, ident)
            nc.gpsimd.memset(zeros8[:, :], 0.0)
            # zero the pad margin after each feature block (spill target of
            # the full-width kT reads of the partial last j-tile)
            if TW < P:
                for f in range(HL):
                    nc.gpsimd.memset(qk_all[:, f * BS + NLP : (f + 1) * BS], 0.0)
            if TW < P:
                # rows of the partial last j-tile beyond the live+pad range
                # never get v written; zero the whole block up front (the v
                # eviction later overwrites rows [0:TW]) so spilled-garbage
                # exp rows contract against zeros
                nc.gpsimd.memset(vplus[:, (NTL - 1) * VROW : NTL * VROW], 0.0)

            vp4 = vplus.rearrange("p (t g c) -> p t g c", t=NTL, g=HL, c=VW)

            with (
                tc.tile_pool(name="psqk", bufs=2, space="PSUM") as pqk,
                tc.tile_pool(name="pss", bufs=2, space="PSUM") as pss,
                tc.tile_pool(name="psa", bufs=1, space="PSUM") as psa,
                tc.tile_pool(name="tpool", bufs=20) as tp,
                tc.tile_pool(name="ttpool", bufs=3) as ttp,
                tc.tile_pool(name="appool", bufs=2) as app,
                tc.tile_pool(name="spool", bufs=4) as sp,
            ):
                PSW = min(w0 * P, 1024)

                def emit_qk_chunk(f, c0, cw):
                    ps = pqk.tile([P, 512], f32, tag="qk", name="ps_qk")
                    for dmt in range(DMT):
                        nc.tensor.matmul(
                            ps[:, :cw],
                            wqk_sb[:, dmt * FQK + f * P : dmt * FQK + (f + 1) * P],
                            xTl_sb[:, dmt * NLP + c0 : dmt * NLP + c0 + cw],
                            start=(dmt == 0),
                            stop=(dmt == DMT - 1),
                        )
                    nc.vector.tensor_copy(
                        qk_all[:, f * BS + c0 : f * BS + c0 + cw], ps[:, :cw]
                    )

                def emit_v(tt):
                    W = IW(tt)
                    ps = pqk.tile([P, FV], f32, tag="qk", name="ps_v")
                    for dmt in range(DMT):
                        nc.tensor.matmul(
                            ps[0:W, :],
                            xTl_sb[:, dmt * NLP + tt * P : dmt * NLP + tt * P + W],
                            wv_sb[:, dmt * FV : (dmt + 1) * FV],
                            start=(dmt == 0),
                            stop=(dmt == DMT - 1),
                        )
                    nc.vector.tensor_copy(
                        vp4[0:W, tt, :, 0:DH],
                        ps[0:W].rearrange("p (g c) -> p g c", g=HL, c=DH),
                    )
                    # denominator column = live indicator (0 for pad rows)
                    nc.vector.tensor_scalar_add(
                        vp4[0:W, tt, :, DH],
                        zeros8[0:W, :],
                        lind_sb[0:W, tt : tt + 1],
                    )

                def sc_wide(h, off, w, jt):
                    pair, hh = h // 2, h % 2
                    p0 = hh * 64
                    qc = (2 * pair) * BS + off * P
                    kc = (2 * pair + 1) * BS
                    ps_s = pss.tile([P, PSW], f32, tag="s", name="ps_s")
                    for c0 in range(0, w * P, 512):
                        cw = min(512, w * P - c0)
                        nc.tensor.matmul(
                            ps_s[:, c0 : c0 + cw],
                            qk_all[p0 : p0 + 64, kc + jt * P : kc + (jt + 1) * P],
                            qk_all[p0 : p0 + 64, qc + c0 : qc + c0 + cw],
                            start=True,
                            stop=True,
                        )
                    t_sb = tp.tile([P, PSW], bf, tag="t", name="t_sb")
                    nc.scalar.activation(t_sb[:, : w * P], ps_s[:, : w * P], EXP)
                    return t_sb

                def pv_wide(h, w, jt, t_sb, pa):
                    vsl = vplus[:, jt * VROW + h * VW : jt * VROW + (h + 1) * VW]
                    for it in range(w):
                        nc.tensor.matmul(
                            pa[:, it * P : it * P + VW],
                            t_sb[:, it * P : (it + 1) * P],
                            vsl,
                            start=(jt == 0 and it % 4 == 0),
                            stop=False,
                        )

                def corr_wide(h, off, w, pa):
                    for it in range(w):
                        nc.tensor.matmul(
                            pa[:, it * P : it * P + VW],
                            padrow_sb[:, (off + it) * P : (off + it + 1) * P],
                            dvec_sb[:, h * VW : (h + 1) * VW],
                            start=False,
                            stop=(it % 4 == 3 or it == w - 1),
                        )

                def norm_wide(h, w, pa, ap):
                    p0 = (h % 2) * 64
                    r_sb = sp.tile([P, 8], f32, tag="r", name="r_sb")
                    pa3 = pa.rearrange("p (t c) -> p t c", t=PSW // P, c=P)
                    nc.vector.reciprocal(r_sb[:, :w], pa3[:, :w, DH])
                    for it in range(w):
                        nc.vector.tensor_scalar_mul(
                            ap[:, it * P + p0 : it * P + p0 + DH],
                            pa[:, it * P : it * P + DH],
                            r_sb[:, it : it + 1],
                        )

                def pv_slot(h, off, w, it, pa, t_list, ap):
                    # slot-major: finish output tile `it` for head h in one
                    # go (all-j PV + correction + normalize), so downstream
                    # per-tile work pipelines inside the window
                    p0 = (h % 2) * 64
                    vcol = h * VW
                    for jt in range(NTL):
                        nc.tensor.matmul(
                            pa[:, it * P : it * P + VW],
                            t_list[jt][:, it * P : (it + 1) * P],
                            vplus[:, jt * VROW + vcol : jt * VROW + vcol + VW],
                            start=(jt == 0 and it % 4 == 0),
                            stop=False,
                        )
                    nc.tensor.matmul(
                        pa[:, it * P : it * P + VW],
                        padrow_sb[:, (off + it) * P : (off + it + 1) * P],
                        dvec_sb[:, h * VW : (h + 1) * VW],
                        start=False,
                        stop=(it % 4 == 3 or it == w - 1),
                    )
                    r1 = sp.tile([P, 8], f32, tag="r", name="r1")
                    nc.vector.reciprocal(
                        r1[:, 0:1], pa[:, it * P + DH : it * P + DH + 1]
                    )
                    nc.vector.tensor_scalar_mul(
                        ap[:, it * P + p0 : it * P + p0 + DH],
                        pa[:, it * P : it * P + DH],
                        r1[:, 0:1],
                    )

                def transpose_it(pair, off, it, ap):
                    ps_tr = pqk.tile([P, P], bf, tag="qk", name="ps_tr")
                    nc.tensor.transpose(
                        ps_tr[:, :], ap[:, it * P : (it + 1) * P], ident
                    )
                    dst = attT[
                        :,
                        pair * NLP + (off + it) * P : pair * NLP
                        + (off + it + 1) * P,
                    ]
                    nc.vector.tensor_copy(dst, ps_tr[:, :])

                def transpose_pair(pair, off, w, ap):
                    # batch 4 transposes per 1-bank slot -> one eviction copy
                    it = 0
                    while it < w:
                        nb = min(4, w - it)
                        if IW(off + it + nb - 1) != P:
                            nb = 1
                        ps_tr = pqk.tile([P, 512], bf, tag="qk", name="ps_tr")
                        wtot = 0
                        for k in range(nb):
                            W = IW(off + it + k)
                            nc.tensor.transpose(
                                ps_tr[:, k * P : k * P + W],
                                ap[0:W, (it + k) * P : (it + k + 1) * P],
                                ident[0:W, 0:W],
                            )
                            wtot = k * P + W
                        nc.vector.tensor_copy(
                            attT[
                                :,
                                pair * NLP + (off + it) * P : pair * NLP
                                + (off + it) * P
                                + wtot,
                            ],
                            ps_tr[:, 0:wtot],
                        )
                        it += nb

                def s_tail(h, off):
                    # batched narrow-tail scores: the TW-wide last i-tile for
                    # all j-tiles, packed into as few psum banks / exp
                    # instructions as possible
                    pair, hh = h // 2, h % 2
                    p0 = hh * 64
                    qc = (2 * pair) * BS + off * P
                    kc = (2 * pair + 1) * BS
                    bpb = max(1, 512 // TW)  # batches per psum bank
                    t_t = ttp.tile([P, NTL * TW], bf, tag="tt", name="t_tail")
                    done = 0
                    while done < NTL:
                        nb = min(2 * bpb, NTL - done)  # one 2-bank slot
                        ps_s = pss.tile([P, PSW], f32, tag="s", name="ps_st")
                        for g in range(nb):
                            jt = done + g
                            pos = (g // bpb) * 512 + (g % bpb) * TW
                            nc.tensor.matmul(
                                ps_s[:, pos : pos + TW],
                                qk_all[p0 : p0 + 64, kc + jt * P : kc + jt * P + P],
                                qk_all[p0 : p0 + 64, qc : qc + TW],
                                start=True,
                                stop=True,
                            )
                        for bk in range((nb + bpb - 1) // bpb):
                            cnt = min(bpb, nb - bk * bpb)
                            nc.scalar.activation(
                                t_t[
                                    :,
                                    (done + bk * bpb) * TW : (done + bk * bpb + cnt)
                                    * TW,
                                ],
                                ps_s[:, bk * 512 : bk * 512 + cnt * TW],
                                EXP,
                            )
                        done += nb
                    return t_t

                def p_tail(h, off, t_t, ap):
                    # 65-col accumulator lives in a 1-bank "qk" slot so it
                    # never waits on the main PV accumulator (deadlock risk)
                    p0 = (h % 2) * 64
                    pa = pqk.tile([P, 512], f32, tag="qk", name="pa_t")
                    for jt in range(NTL):
                        nc.tensor.matmul(
                            pa[0:TW, 0:VW],
                            t_t[:, jt * TW : (jt + 1) * TW],
                            vplus[:, jt * VROW + h * VW : jt * VROW + (h + 1) * VW],
                            start=(jt == 0),
                            stop=False,
                        )
                    nc.tensor.matmul(
                        pa[0:TW, 0:VW],
                        padrow_sb[:, off * P : off * P + TW],
                        dvec_sb[:, h * VW : (h + 1) * VW],
                        start=False,
                        stop=True,
                    )
                    r_sb = sp.tile([P, 8], f32, tag="r", name="r_t")
                    nc.vector.reciprocal(r_sb[0:TW, 0:1], pa[0:TW, DH : DH + 1])
                    nc.vector.tensor_scalar_mul(
                        ap[0:TW, p0 : p0 + DH], pa[0:TW, 0:DH], r_sb[0:TW, 0:1]
                    )

                nout = [0]

                def emit_half(it, ch, half):
                    # half 0: pairs 0-1 -> outA f32; half 1: pairs 2-3 -> outB bf16
                    W = IW(it)
                    ps_o = pqk.tile([P, 512], f32, tag="qk", name="ps_o")
                    for ht in (0, 1) if half == 0 else (2, 3):
                        nc.tensor.matmul(
                            ps_o[0:W, :],
                            attT[:, ht * NLP + it * P : ht * NLP + it * P + W],
                            wout_sb[:, ht * DM + ch * 512 : ht * DM + (ch + 1) * 512],
                            start=(ht % 2 == 0),
                            stop=(ht % 2 == 1),
                        )
                    dt = f32 if half == 0 else bf
                    o_sb = sp.tile([P, 512], dt, tag="obA" if half == 0 else "obB",
                                   name="o_sb")
                    # A-half evictions stay off ACT (it paces mid-loop exps);
                    # B-half runs in the drain where ACT is idle.
                    if half == 1:
                        nc.scalar.activation(o_sb[0:W, :], ps_o[0:W, :], COPY)
                    else:
                        nc.vector.tensor_copy(o_sb[0:W, :], ps_o[0:W, :])
                    nout[0] += 1
                    dst = outA if half == 0 else outB
                    nc.sync.dma_start(
                        out=dst[it * P : it * P + W, ch * 512 : (ch + 1) * 512],
                        in_=o_sb[0:W, :],
                    )

                # ---- filler queues ----
                # qk chunks for pairs 1..3: pair p before head 2p's scores.
                fast_start = len(qk_chunks) == 3
                fill_units = [
                    (f, c0, cw)
                    for pair in range(1, NP)
                    for f in (2 * pair, 2 * pair + 1)
                    for (c0, cw) in qk_chunks
                    if not (fast_start and f == 2 and c0 < qk_chunks[2][0])
                ]
                n_units = len(fill_units)
                cpp = 2 * len(qk_chunks)
                fill_pos = [0]
                cpair1 = cpp - (2 if fast_start else 0)

                def emit_fill_to(tgt):
                    k = fill_pos[0]
                    for u in fill_units[k : min(n_units, tgt)]:
                        emit_qk_chunk(*u)
                    fill_pos[0] = max(k, min(n_units, tgt))

                def needed_before(h):
                    p = max(0, h // 2)
                    return min(n_units, cpair1 if p == 1 else
                               cpair1 + (p - 1) * cpp if p > 1 else 0)

                # A half-chunks (pairs 0-1): ready once pair-1 main+tail
                # transposes are done (end of window 5); fill windows 6-7.
                a_units = [
                    (it, ch) for it in range(off0, off0 + w0) for ch in range(2)
                ]
                nA = len(a_units)
                a_pos = [0]

                def emit_a_to(tgt):
                    k = a_pos[0]
                    for u in a_units[k : min(nA, tgt)]:
                        emit_half(u[0], u[1], 0)
                    a_pos[0] = max(k, min(nA, tgt))

                # ---- window 0: pair-0 projections + head-0 scores,
                #      pair-1 qk chunks interleaved ----
                if len(qk_chunks) == 3:
                    # dmt-outer interleave across the f0/f1 chunks plus
                    # pair-1's first q chunks keeps the PE fed at DMA
                    # arrival pace (chunk-slots borrowed from the idle
                    # pss/psa pools + pqk)
                    sA = pss.tile([P, PSW], f32, tag="s", name="ps_q0")
                    sB = pss.tile([P, PSW], f32, tag="s", name="ps_q1")
                    qA = pqk.tile([P, 512], f32, tag="qk", name="ps_q2")
                    qB = pqk.tile([P, 512], f32, tag="qk", name="ps_q3")
                    aA = psa.tile([P, PSW], f32, tag="att", name="ps_q4")
                    units = [
                        (0, qk_chunks[0][0], qk_chunks[0][1], sA, 0),
                        (0, qk_chunks[1][0], qk_chunks[1][1], sA, 512),
                        (0, qk_chunks[2][0], qk_chunks[2][1], sB, 0),
                        (1, qk_chunks[0][0], qk_chunks[0][1], sB, 512),
                        (1, qk_chunks[1][0], qk_chunks[1][1], qA, 0),
                        (1, qk_chunks[2][0], qk_chunks[2][1], qB, 0),
                        (2, qk_chunks[0][0], qk_chunks[0][1], aA, 0),
                        (2, qk_chunks[1][0], qk_chunks[1][1], aA, 512),
                    ]
                    for dmt in range(DMT):
                        for f, c0, cw, ps, so in units:
                            nc.tensor.matmul(
                                ps[:, so : so + cw],
                                wqk_sb[
                                    :, dmt * FQK + f * P : dmt * FQK + (f + 1) * P
                                ],
                                xTl_sb[:, dmt * NLP + c0 : dmt * NLP + c0 + cw],
                                start=(dmt == 0),
                                stop=(dmt == DMT - 1),
                            )
                    for f, c0, cw, ps, so in units:
                        nc.vector.tensor_copy(
                            qk_all[:, f * BS + c0 : f * BS + c0 + cw],
                            ps[:, so : so + cw],
                        )
                else:
                    for f in (0, 1):
                        for c0, cw in qk_chunks:
                            emit_qk_chunk(f, c0, cw)
                t_store = {0: []}
                tt_store = {}
                for jt in range(NTL):
                    t_store[0].append(sc_wide(0, off0, w0, jt))
                if tail1:
                    # pair-0 tail scores cover the wv DMA wait; their spill
                    # reads (pair-1 q block) are written by the fast startup
                    tt_store[0] = s_tail(0, tail1[0][0])
                    tt_store[1] = s_tail(1, tail1[0][0])
                for tt in range(min(3, NTL)):
                    emit_v(tt)
                emit_fill_to(cpair1)

                # ---- windows 1..7: S(h) || PV(h-1) || tail(h-1 scores,
                #      h-2 PV) || fillers ----
                ap_cur = None
                apt_cur = None
                for h in range(1, HL):
                    emit_fill_to(needed_before(h))
                    fprev = fill_pos[0]
                    fth = n_units if h >= 5 else max(
                        needed_before(h + 1), (n_units * h + 4) // 5
                    )
                    aprev = a_pos[0]
                    ath = {5: 6, 6: 14}.get(h, 0 if h < 5 else nA)
                    t_store[h] = []
                    pa = psa.tile([P, PSW], f32, tag="att", name="pa")
                    for jt in range(NTL):
                        t_store[h].append(sc_wide(h, off0, w0, jt))
                        if h == 1 and jt >= 3:
                            emit_v(jt)
                        emit_a_to(aprev + ((ath - aprev) * (jt + 1)) // NTL)
                        pv_wide(h - 1, w0, jt, t_store[h - 1][jt], pa)
                        emit_fill_to(fprev + ((fth - fprev) * (jt + 1)) // NTL)
                        t_store[h - 1][jt] = None
                        if jt == 2 and h >= 2 and tail1 and (h - 2) in tt_store:
                            # lag-2 tail PV for head h-2
                            offt = tail1[0][0]
                            if (h - 2) % 2 == 0:
                                apt_cur = app.tile([P, P], bf, tag="apt", name="apt")
                            p_tail(h - 2, offt, tt_store.pop(h - 2), apt_cur)
                            if (h - 2) % 2 == 1:
                                transpose_pair((h - 2) // 2, offt, 1, apt_cur)
                    if tail1 and h >= 3:
                        tt_store[h - 1] = s_tail(h - 1, tail1[0][0])
                    if tail1 and h == HL - 1:
                        tt_store[h] = s_tail(h, tail1[0][0])
                    corr_wide(h - 1, off0, w0, pa)
                    if (h - 1) % 2 == 0:
                        ap_cur = app.tile([P, PSW], bf, tag="ap", name="ap")
                    norm_wide(h - 1, w0, pa, ap_cur)
                    if (h - 1) % 2 == 1:
                        transpose_pair((h - 1) // 2, off0, w0, ap_cur)

                # ---- epilogue: PV(7), tails 6-7, pipelined pair-3 finish ----
                def emit_outB_big(it, split=False):
                    # whole-row B chunk: pairs 2-3 for both DM halves in one
                    # 2-bank pss slot, one eviction, one outB DMA
                    W = IW(it)
                    ps_o = pss.tile([P, PSW], f32, tag="s", name="ps_b")
                    for ch in range(2):
                        for ht in (2, 3):
                            nc.tensor.matmul(
                                ps_o[0:W, ch * 512 : (ch + 1) * 512],
                                attT[:, ht * NLP + it * P : ht * NLP + it * P + W],
                                wout_sb[
                                    :, ht * DM + ch * 512 : ht * DM + (ch + 1) * 512
                                ],
                                start=(ht == 2),
                                stop=(ht == 3),
                            )
                    o_sb = sp.tile([P, 1024], bf, tag="obB", name="o_sbB")
                    if split:
                        # parallel eviction halves (ACT + DVE): shallow drain
                        nc.scalar.activation(
                            o_sb[0:W, 0:512], ps_o[0:W, 0:512], COPY
                        )
                        nc.vector.tensor_copy(
                            o_sb[0:W, 512:1024], ps_o[0:W, 512:1024]
                        )
                        nc.sync.dma_start(
                            out=outB[it * P : it * P + W, 0:512],
                            in_=o_sb[0:W, 0:512],
                        )
                        nc.sync.dma_start(
                            out=outB[it * P : it * P + W, 512:1024],
                            in_=o_sb[0:W, 512:1024],
                        )
                        return
                    if nout[0] % 2 == 0:
                        nc.scalar.activation(o_sb[0:W, :], ps_o[0:W, :1024], COPY)
                    else:
                        nc.vector.tensor_copy(o_sb[0:W, :], ps_o[0:W, :1024])
                    nout[0] += 1
                    nc.sync.dma_start(
                        out=outB[it * P : it * P + W, :], in_=o_sb[0:W, :]
                    )

                if tail1 and (HL - 1) not in tt_store:
                    tt_store[HL - 1] = s_tail(HL - 1, tail1[0][0])
                emit_fill_to(n_units)
                aprev = a_pos[0]
                pa = psa.tile([P, PSW], f32, tag="att", name="pa")
                for jt in range(NTL):
                    emit_a_to(aprev + ((nA - aprev) * (jt + 1)) // NTL)
                    pv_wide(HL - 1, w0, jt, t_store[HL - 1][jt], pa)
                    if jt == 2 and tail1 and (HL - 2) in tt_store:
                        offt = tail1[0][0]
                        apt_cur = app.tile([P, P], bf, tag="apt", name="apt")
                        p_tail(HL - 2, offt, tt_store.pop(HL - 2), apt_cur)
                    if jt == 4 and tail1 and (HL - 1) in tt_store:
                        offt = tail1[0][0]
                        p_tail(HL - 1, offt, tt_store.pop(HL - 1), apt_cur)
                        transpose_pair(NP - 1, offt, 1, apt_cur)
                    if jt == 0 and tail1:
                        # tail i-tile outA rows (pairs 0-1 only): PE work to
                        # cover the window-boundary normalize wait
                        offt = tail1[0][0]
                        emit_half(offt, 0, 0)
                        emit_half(offt, 1, 0)

                corr_wide(HL - 1, off0, w0, pa)
                emit_a_to(nA)
                # pipelined pair-3 finish: normalize slot -> transpose ->
                # previous tile's whole-row B chunk (hides eviction latency)
                p0e = ((HL - 1) % 2) * 64
                r_sb = sp.tile([P, 8], f32, tag="r", name="r_e")
                pa3 = pa.rearrange("p (t c) -> p t c", t=PSW // P, c=P)
                nc.vector.reciprocal(r_sb[:, :w0], pa3[:, :w0, DH])
                for it in range(w0):
                    nc.vector.tensor_scalar_mul(
                        ap_cur[:, it * P + p0e : it * P + p0e + DH],
                        pa[:, it * P : it * P + DH],
                        r_sb[:, it : it + 1],
                    )
                    ps_tr = pqk.tile([P, P], bf, tag="qk", name="ps_tr")
                    nc.tensor.transpose(
                        ps_tr[:, :], ap_cur[:, it * P : (it + 1) * P], ident
                    )
                    nc.vector.tensor_copy(
                        attT[
                            :,
                            (NP - 1) * NLP + (off0 + it) * P : (NP - 1) * NLP
                            + (off0 + it + 1) * P,
                        ],
                        ps_tr[:, :],
                    )
                    if it >= 1:
                        emit_outB_big(off0 + it - 1, split=(it == w0 - 1))
                emit_outB_big(off0 + w0 - 1, split=True)
                if tail1:
                    # tail i-tile outB last: its 24-row eviction/DMAs drain
                    # far faster than a full 128-row tile's
                    emit_outB_big(tail1[0][0], split=True)

                # ---- slow generic path for wide tail spans (NTL > 9) ----
                for offt, wt in slow_tails:
                    for pair in range(NP):
                        for hh in range(2):
                            h = 2 * pair + hh
                            pa = psa.tile([P, PSW], f32, tag="att", name="pa_w")
                            tw = [sc_wide(h, offt, wt, jt) for jt in range(NTL)]
                            for jt in range(NTL):
                                pv_wide(h, wt, jt, tw[jt], pa)
                            corr_wide(h, offt, wt, pa)
                            if hh == 0:
                                ap_t = app.tile([P, PSW], bf, tag="ap", name="ap_w")
                            norm_wide(h, wt, pa, ap_t)
                        transpose_pair(pair, offt, wt, ap_t)
                    for it in range(offt, offt + wt):
                        for ch in range(2):
                            emit_half(it, ch, 0)

                # B halves for slow-tail tiles (already emitted for the rest)
                for offt, wt in slow_tails:
                    for it in range(offt, offt + wt):
                        for ch in range(2):
                            emit_half(it, ch, 1)

    nc.compile()
    return nc


def _shard_inputs(x, w_qkv, w_out, b_out, mask):
    """Host-side live-token gather + per-core layout prep."""
    bf = ml_dtypes.bfloat16
    x = np.asarray(x, dtype=np.float32)
    w_qkv = np.asarray(w_qkv, dtype=np.float32)
    w_out = np.asarray(w_out, dtype=np.float32)
    mask = np.asarray(mask)

    NLs = [int(mask[b].sum()) for b in range(B)]
    NLP = int(np.ceil((max(NLs) + 1) / 8) * 8)
    NTL = (NLP + P - 1) // P

    w3 = w_qkv.reshape(DM, H, 3, DH)
    in_maps = []
    for c in range(NCORES):
        b, hg = c // HG, c % HG
        nl = NLs[b]
        live = np.nonzero(mask[b])[0]
        dead = np.nonzero(mask[b] == 0)[0]

        wq = w3[:, hg * HL : (hg + 1) * HL, 0, :].reshape(DM, FV) * SCALE
        wk = w3[:, hg * HL : (hg + 1) * HL, 1, :].reshape(DM, FV)
        # pair-major column layout: [q_p0 | k_p0 | q_p1 | k_p1 | ...]
        wqk_c = np.empty((DM, FQK), np.float32)
        for p in range(NP):
            wqk_c[:, p * 256 : p * 256 + 128] = wq[:, p * 128 : (p + 1) * 128]
            wqk_c[:, p * 256 + 128 : (p + 1) * 256] = wk[:, p * 128 : (p + 1) * 128]
        wv_c = np.ascontiguousarray(
            w3[:, hg * HL : (hg + 1) * HL, 2, :].reshape(DM, FV)
        )
        wout_c = np.ascontiguousarray(w_out[hg * FV : (hg + 1) * FV, :])

        xTl_c = np.zeros((DM, NLP), np.float32)
        xTl_c[:, :nl] = x[b].T[:, live]

        padrow_c = np.zeros((1, NLP), np.float32)
        padrow_c[0, nl:] = 1.0
        lind_c = np.zeros(NTL * P, np.float32)
        lind_c[:nl] = 1.0
        lind_c = np.ascontiguousarray(lind_c.reshape(NTL, P).T)

        # dvec: per head [sum_dead v_h | N_dead]
        xs = x[b][dead].sum(axis=0) if len(dead) else np.zeros(DM, np.float32)
        dv = (xs @ wv_c).reshape(HL, DH)
        dvec_c = np.empty((1, VROW), np.float32)
        for h in range(HL):
            dvec_c[0, h * VW : h * VW + DH] = dv[h]
            dvec_c[0, h * VW + DH] = float(len(dead))

        in_maps.append(
            {
                "xTl": xTl_c.astype(bf),
                "wqk": wqk_c.astype(bf),
                "wv": wv_c.astype(bf),
                "wout": wout_c.astype(bf),
                "padrow": padrow_c.astype(bf),
                "lind": lind_c.astype(np.float32),
                "dvec": dvec_c.astype(bf),
            }
        )
    return in_maps, NLP, NLs


def kernel(x, w_qkv, w_out, b_out, mask):
    from concourse.bass_utils import run_bass_kernel_spmd

    in_maps, NLP, NLs = _shard_inputs(x, w_qkv, w_out, b_out, mask)
    if NLP not in _CACHE:
        _CACHE[NLP] = _build_program(NLP)
    nc = _CACHE[NLP]

    res = run_bass_kernel_spmd(nc, in_maps, list(range(NCORES))).results

    mask = np.asarray(mask)
    b_out = np.asarray(b_out, dtype=np.float32)
    out = np.empty((B, N, DM), np.float32)
    for b in range(B):
        nl = NLs[b]
        live = np.nonzero(mask[b])[0]
        dead = np.nonzero(mask[b] == 0)[0]
        part = (
            res[HG * b]["outA"]
            + res[HG * b]["outB"].astype(np.float32)
            + res[HG * b + 1]["outA"]
            + res[HG * b + 1]["outB"].astype(np.float32)
        )
        out[b, live] = part[:nl]
        if len(dead):
            out[b, dead] = part[nl]
        out[b] += b_out[None, :]
    return out


# revision 96
# speedup vs baseline: 1.0064x; 1.0046x over previous
"""Multi-head attention (B=4, N=2048, DM=1024, H=16, DH=64) on 8 trn2 cores.

Sharding: core c -> (batch b = c//2, head-group hg = c%2 of 8 heads).

Live-token compaction: the pair mask only keeps (i,j) score pairs where
both tokens are live, and every dead query row of the reference output is
the SAME uniform average over all value tokens.  So the host gathers the
~NL live tokens of each batch into a compact [DM, NLP] x^T (NLP = padded
live count, multiple of 128, >= NL+1), the device runs attention on live
tokens only, and the host scatters rows back, filling dead rows with the
row produced by the first zero-padded query column.

Per-column semantics on device (q = x@Wq, k = x@Wk, scaled, no masking):
  - live i, live j: t = exp(q_i.k_j) -- the real softmax numerator.
  - pad i (x=0 -> q=0): t = 1 for all j, plus a rank-1 correction row
    (+padrow_i * dvec, dvec = [sum_dead v | N_dead] from the host) makes
    pv = [sum_all v | N], i.e. exactly the reference's uniform dead row.
  - pad j (x=0 -> k=v=0): t = exp(0) = 1 but vplus rows are zero (the
    denominator ones-column is L=live-indicator, not constant 1), so pads
    contribute nothing.

Device layout mirrors the dense kernel: feature-major q/k ([64,NLP] per
head), token-major v with an appended denominator column, scores
transposed [j,i] so PV needs no transpose, exp on ACT, a small [N,64]
transpose per head for the output projection.

Scheduling: heads run as a software pipeline -- window W(h) emits scores
+exp for head h, PV for head h-1, the batched last-i-tile ("tail") scores
for head h-1 and PV for head h-2, plus deadline-scheduled filler chunks
(later pairs' qk projections, then partial output chunks).  The output
projection is split into two half-contractions written to separate DRAM
tensors (outA = pairs 0-1 in f32 overlapped with the mid loop, outB =
pairs 2-3 in bf16 to halve the drain DMA); the host sums them.
"""

import sys

sys.path.insert(0, "/opt/trn_rl_repo")

import numpy as np
import ml_dtypes

B, N, DM, H, DH = 4, 2048, 1024, 16, 64
SCALE = DH**-0.5
NCORES = 8
HG = 2  # head groups (tensor-parallel factor)
HL = H // HG  # 8 heads per core
NP = HL // 2  # 4 head pairs
FQK = HL * 2 * DH  # 1024 qk features per core
FV = HL * DH  # 512 v features per core
P = 128
DMT = DM // P  # 8 dm tiles
VW = DH + 1  # 65: v columns + denominator column
VROW = HL * VW  # 520
HT = FV // P  # 4 head-dim tiles for the projection

_CACHE = {}


def _build_program(NLP):
    import concourse.mybir as mybir
    import concourse.tile as tile
    from concourse import bacc
    from concourse.masks import make_identity

    bf = mybir.dt.bfloat16
    f32 = mybir.dt.float32
    EXP = mybir.ActivationFunctionType.Exp
    COPY = mybir.ActivationFunctionType.Copy

    NTL = (NLP + P - 1) // P  # live token tiles (last may be partial)
    TW = NLP - (NTL - 1) * P  # width of the last tile
    # i-span structure: one wide main span (software-pipelined head loop),
    # remaining tiles handled in the pipelined tail path (w==1) or a slow
    # generic path (w>1, only for much larger masks).
    spans = []
    off = 0
    while NTL - off > 0:
        w = min(8, NTL - off)
        spans.append((off, w))
        off += w
    off0, w0 = spans[0]
    tail1 = [s for s in spans[1:] if s[1] == 1]
    slow_tails = [s for s in spans[1:] if s[1] > 1]

    def IW(it):
        return TW if it == NTL - 1 else P

    # qk_all block stride: a (P-TW)-col pad after each feature block so the
    # full-width kT reads of the partial last j-tile never touch another
    # block's (possibly unwritten) data
    BS = NLP + (P - TW) if TW < P else NLP
    # qk-projection column chunks (psum-bank sized)
    qk_chunks = [(c0, min(512, NLP - c0)) for c0 in range(0, NLP, 512)]

    nc = bacc.Bacc(
        "TRN2", target_bir_lowering=False, debug=False, num_devices=NCORES
    )
    xTl = nc.dram_tensor("xTl", [DM, NLP], bf, kind="ExternalInput")
    wqk = nc.dram_tensor("wqk", [DM, FQK], bf, kind="ExternalInput")
    wv = nc.dram_tensor("wv", [DM, FV], bf, kind="ExternalInput")
    wout = nc.dram_tensor("wout", [FV, DM], bf, kind="ExternalInput")
    padrow = nc.dram_tensor("padrow", [1, NLP], bf, kind="ExternalInput")
    lind = nc.dram_tensor("lind", [P, NTL], f32, kind="ExternalInput")
    dvec = nc.dram_tensor("dvec", [1, VROW], bf, kind="ExternalInput")
    outA = nc.dram_tensor("outA", [NLP, DM], f32, kind="ExternalOutput")
    outB = nc.dram_tensor("outB", [NLP, DM], bf, kind="ExternalOutput")

    with tile.TileContext(nc) as tc:
        with tc.tile_pool(name="const", bufs=1) as cp:
            xTl_sb = cp.tile([P, DMT * NLP], bf, tag="xTl")
            wqk_sb = cp.tile([P, DMT * FQK], bf, tag="wqk")
            wv_sb = cp.tile([P, DMT * FV], bf, tag="wv")
            wout_sb = cp.tile([P, HT * DM], bf, tag="wout")
            padrow_sb = cp.tile([1, NLP], bf, tag="padrow")
            lind_sb = cp.tile([P, NTL], f32, tag="lind")
            dvec_sb = cp.tile([1, VROW], bf, tag="dvec")
            ident = cp.tile([P, P], bf, tag="ident")
            zeros8 = cp.tile([P, HL], bf, tag="zeros8")
            vplus = cp.tile([P, NTL * VROW], bf, tag="vplus")
            qk_all = cp.tile([P, HL * BS], bf, tag="qkall")
            attT = cp.tile([P, HT * NLP], bf, tag="attT")

            # DMA order mirrors consumption: per-dm-tile x^T plus the
            # pair-0 qk weight columns first (feeds the first projection),
            # then v weights (needed by the head-1 window), small tensors,
            # then the remaining qk weight columns and w_out.
            for dmt in range(DMT):
                nc.sync.dma_start(
                    out=wqk_sb[:, dmt * FQK : dmt * FQK + 3 * P],
                    in_=wqk[dmt * P : (dmt + 1) * P, 0 : 3 * P],
                )
                nc.sync.dma_start(
                    out=xTl_sb[:, dmt * NLP : (dmt + 1) * NLP],
                    in_=xTl[dmt * P : (dmt + 1) * P, :],
                )
            nc.sync.dma_start(
                out=wv_sb[:, :].rearrange("p (d f) -> p d f", d=DMT, f=FV),
                in_=wv[:, :].rearrange("(d p) c -> p d c", p=P),
            )
            nc.sync.dma_start(out=lind_sb[:, :], in_=lind[:, :])
            nc.sync.dma_start(out=padrow_sb[:, :], in_=padrow[:, :])
            nc.sync.dma_start(out=dvec_sb[:, :], in_=dvec[:, :])
            for dmt in range(DMT):
                nc.sync.dma_start(
                    out=wqk_sb[:, dmt * FQK + 3 * P : (dmt + 1) * FQK],
                    in_=wqk[dmt * P : (dmt + 1) * P, 3 * P :],
                )
            for ht in range(HT):
                nc.sync.dma_start(
                    out=wout_sb[:, ht * DM : (ht + 1) * DM],
                    in_=wout[ht * P : (ht + 1) * P, :],
                )
            make_identity(nc, ident)
            nc.gpsimd.memset(zeros8[:, :], 0.0)
            # zero the pad margin after each feature block (spill target of
            # the full-width kT reads of the partial last j-tile)
            if TW < P:
                for f in range(HL):
                    nc.gpsimd.memset(qk_all[:, f * BS + NLP : (f + 1) * BS], 0.0)
            if TW < P:
                # rows of the partial last j-tile beyond the live+pad range
                # never get v written; zero the whole block up front (the v
                # eviction later overwrites rows [0:TW]) so spilled-garbage
                # exp rows contract against zeros
                nc.gpsimd.memset(vplus[:, (NTL - 1) * VROW : NTL * VROW], 0.0)

            vp4 = vplus.rearrange("p (t g c) -> p t g c", t=NTL, g=HL, c=VW)

            with (
                tc.tile_pool(name="psqk", bufs=2, space="PSUM") as pqk,
                tc.tile_pool(name="pss", bufs=2, space="PSUM") as pss,
                tc.tile_pool(name="psa", bufs=1, space="PSUM") as psa,
                tc.tile_pool(name="tpool", bufs=20) as tp,
                tc.tile_pool(name="ttpool", bufs=3) as ttp,
                tc.tile_pool(name="appool", bufs=2) as app,
                tc.tile_pool(name="spool", bufs=4) as sp,
            ):
                PSW = min(w0 * P, 1024)

                def emit_qk_chunk(f, c0, cw):
                    ps = pqk.tile([P, 512], f32, tag="qk", name="ps_qk")
                    for dmt in range(DMT):
                        nc.tensor.matmul(
                            ps[:, :cw],
                            wqk_sb[:, dmt * FQK + f * P : dmt * FQK + (f + 1) * P],
                            xTl_sb[:, dmt * NLP + c0 : dmt * NLP + c0 + cw],
                            start=(dmt == 0),
                            stop=(dmt == DMT - 1),
                        )
                    nc.vector.tensor_copy(
                        qk_all[:, f * BS + c0 : f * BS + c0 + cw], ps[:, :cw]
                    )

                def emit_v(tt):
                    W = IW(tt)
                    ps = pqk.tile([P, FV], f32, tag="qk", name="ps_v")
                    for dmt in range(DMT):
                        nc.tensor.matmul(
                            ps[0:W, :],
                            xTl_sb[:, dmt * NLP + tt * P : dmt * NLP + tt * P + W],
                            wv_sb[:, dmt * FV : (dmt + 1) * FV],
                            start=(dmt == 0),
                            stop=(dmt == DMT - 1),
                        )
                    nc.vector.tensor_copy(
                        vp4[0:W, tt, :, 0:DH],
                        ps[0:W].rearrange("p (g c) -> p g c", g=HL, c=DH),
                    )
                    # denominator column = live indicator (0 for pad rows)
                    nc.vector.tensor_scalar_add(
                        vp4[0:W, tt, :, DH],
                        zeros8[0:W, :],
                        lind_sb[0:W, tt : tt + 1],
                    )

                def sc_wide(h, off, w, jt):
                    pair, hh = h // 2, h % 2
                    p0 = hh * 64
                    qc = (2 * pair) * BS + off * P
                    kc = (2 * pair + 1) * BS
                    ps_s = pss.tile([P, PSW], f32, tag="s", name="ps_s")
                    for c0 in range(0, w * P, 512):
                        cw = min(512, w * P - c0)
                        nc.tensor.matmul(
                            ps_s[:, c0 : c0 + cw],
                            qk_all[p0 : p0 + 64, kc + jt * P : kc + (jt + 1) * P],
                            qk_all[p0 : p0 + 64, qc + c0 : qc + c0 + cw],
                            start=True,
                            stop=True,
                        )
                    t_sb = tp.tile([P, PSW], bf, tag="t", name="t_sb")
                    nc.scalar.activation(t_sb[:, : w * P], ps_s[:, : w * P], EXP)
                    return t_sb

                def pv_wide(h, w, jt, t_sb, pa):
                    vsl = vplus[:, jt * VROW + h * VW : jt * VROW + (h + 1) * VW]
                    for it in range(w):
                        nc.tensor.matmul(
                            pa[:, it * P : it * P + VW],
                            t_sb[:, it * P : (it + 1) * P],
                            vsl,
                            start=(jt == 0 and it % 4 == 0),
                            stop=False,
                        )

                def corr_wide(h, off, w, pa):
                    for it in range(w):
                        nc.tensor.matmul(
                            pa[:, it * P : it * P + VW],
                            padrow_sb[:, (off + it) * P : (off + it + 1) * P],
                            dvec_sb[:, h * VW : (h + 1) * VW],
                            start=False,
                            stop=(it % 4 == 3 or it == w - 1),
                        )

                def norm_wide(h, w, pa, ap):
                    p0 = (h % 2) * 64
                    r_sb = sp.tile([P, 8], f32, tag="r", name="r_sb")
                    pa3 = pa.rearrange("p (t c) -> p t c", t=PSW // P, c=P)
                    nc.vector.reciprocal(r_sb[:, :w], pa3[:, :w, DH])
                    for it in range(w):
                        nc.vector.tensor_scalar_mul(
                            ap[:, it * P + p0 : it * P + p0 + DH],
                            pa[:, it * P : it * P + DH],
                            r_sb[:, it : it + 1],
                        )

                def pv_slot(h, off, w, it, pa, t_list, ap):
                    # slot-major: finish output tile `it` for head h in one
                    # go (all-j PV + correction + normalize), so downstream
                    # per-tile work pipelines inside the window
                    p0 = (h % 2) * 64
                    vcol = h * VW
                    for jt in range(NTL):
                        nc.tensor.matmul(
                            pa[:, it * P : it * P + VW],
                            t_list[jt][:, it * P : (it + 1) * P],
                            vplus[:, jt * VROW + vcol : jt * VROW + vcol + VW],
                            start=(jt == 0 and it % 4 == 0),
                            stop=False,
                        )
                    nc.tensor.matmul(
                        pa[:, it * P : it * P + VW],
                        padrow_sb[:, (off + it) * P : (off + it + 1) * P],
                        dvec_sb[:, h * VW : (h + 1) * VW],
                        start=False,
                        stop=(it % 4 == 3 or it == w - 1),
                    )
                    r1 = sp.tile([P, 8], f32, tag="r", name="r1")
                    nc.vector.reciprocal(
                        r1[:, 0:1], pa[:, it * P + DH : it * P + DH + 1]
                    )
                    nc.vector.tensor_scalar_mul(
                        ap[:, it * P + p0 : it * P + p0 + DH],
                        pa[:, it * P : it * P + DH],
                        r1[:, 0:1],
                    )

                def transpose_it(pair, off, it, ap):
                    ps_tr = pqk.tile([P, P], bf, tag="qk", name="ps_tr")
                    nc.tensor.transpose(
                        ps_tr[:, :], ap[:, it * P : (it + 1) * P], ident
                    )
                    dst = attT[
                        :,
                        pair * NLP + (off + it) * P : pair * NLP
                        + (off + it + 1) * P,
                    ]
                    nc.vector.tensor_copy(dst, ps_tr[:, :])

                def transpose_pair(pair, off, w, ap):
                    # batch 4 transposes per 1-bank slot -> one eviction copy
                    it = 0
                    while it < w:
                        nb = min(4, w - it)
                        if IW(off + it + nb - 1) != P:
                            nb = 1
                        ps_tr = pqk.tile([P, 512], bf, tag="qk", name="ps_tr")
                        wtot = 0
                        for k in range(nb):
                            W = IW(off + it + k)
                            nc.tensor.transpose(
                                ps_tr[:, k * P : k * P + W],
                                ap[0:W, (it + k) * P : (it + k + 1) * P],
                                ident[0:W, 0:W],
                            )
                            wtot = k * P + W
                        nc.vector.tensor_copy(
                            attT[
                                :,
                                pair * NLP + (off + it) * P : pair * NLP
                                + (off + it) * P
                                + wtot,
                            ],
                            ps_tr[:, 0:wtot],
                        )
                        it += nb

                def s_tail(h, off):
                    # batched narrow-tail scores: the TW-wide last i-tile for
                    # all j-tiles, packed into as few psum banks / exp
                    # instructions as possible
                    pair, hh = h // 2, h % 2
                    p0 = hh * 64
                    qc = (2 * pair) * BS + off * P
                    kc = (2 * pair + 1) * BS
                    bpb = max(1, 512 // TW)  # batches per psum bank
                    t_t = ttp.tile([P, NTL * TW], bf, tag="tt", name="t_tail")
                    done = 0
                    while done < NTL:
                        nb = min(2 * bpb, NTL - done)  # one 2-bank slot
                        ps_s = pss.tile([P, PSW], f32, tag="s", name="ps_st")
                        for g in range(nb):
                            jt = done + g
                            pos = (g // bpb) * 512 + (g % bpb) * TW
                            nc.tensor.matmul(
                                ps_s[:, pos : pos + TW],
                                qk_all[p0 : p0 + 64, kc + jt * P : kc + jt * P + P],
                                qk_all[p0 : p0 + 64, qc : qc + TW],
                                start=True,
                                stop=True,
                            )
                        for bk in range((nb + bpb - 1) // bpb):
                            cnt = min(bpb, nb - bk * bpb)
                            nc.scalar.activation(
                                t_t[
                                    :,
                                    (done + bk * bpb) * TW : (done + bk * bpb + cnt)
                                    * TW,
                                ],
                                ps_s[:, bk * 512 : bk * 512 + cnt * TW],
                                EXP,
                            )
                        done += nb
                    return t_t

                def p_tail(h, off, t_t, ap):
                    # 65-col accumulator lives in a 1-bank "qk" slot so it
                    # never waits on the main PV accumulator (deadlock risk)
                    p0 = (h % 2) * 64
                    pa = pqk.tile([P, 512], f32, tag="qk", name="pa_t")
                    for jt in range(NTL):
                        nc.tensor.matmul(
                            pa[0:TW, 0:VW],
                            t_t[:, jt * TW : (jt + 1) * TW],
                            vplus[:, jt * VROW + h * VW : jt * VROW + (h + 1) * VW],
                            start=(jt == 0),
                            stop=False,
                        )
                    nc.tensor.matmul(
                        pa[0:TW, 0:VW],
                        padrow_sb[:, off * P : off * P + TW],
                        dvec_sb[:, h * VW : (h + 1) * VW],
                        start=False,
                        stop=True,
                    )
                    r_sb = sp.tile([P, 8], f32, tag="r", name="r_t")
                    nc.vector.reciprocal(r_sb[0:TW, 0:1], pa[0:TW, DH : DH + 1])
                    nc.vector.tensor_scalar_mul(
                        ap[0:TW, p0 : p0 + DH], pa[0:TW, 0:DH], r_sb[0:TW, 0:1]
                    )

                nout = [0]
                nbig = [0]

                def emit_half(it, ch, half):
                    # half 0: pairs 0-1 -> outA f32; half 1: pairs 2-3 -> outB bf16
                    W = IW(it)
                    ps_o = pqk.tile([P, 512], f32, tag="qk", name="ps_o")
                    for ht in (0, 1) if half == 0 else (2, 3):
                        nc.tensor.matmul(
                            ps_o[0:W, :],
                            attT[:, ht * NLP + it * P : ht * NLP + it * P + W],
                            wout_sb[:, ht * DM + ch * 512 : ht * DM + (ch + 1) * 512],
                            start=(ht % 2 == 0),
                            stop=(ht % 2 == 1),
                        )
                    dt = f32 if half == 0 else bf
                    o_sb = sp.tile([P, 512], dt, tag="obA" if half == 0 else "obB",
                                   name="o_sb")
                    # A-half evictions stay off ACT (it paces mid-loop exps);
                    # B-half runs in the drain where ACT is idle.
                    if half == 1:
                        nc.scalar.activation(o_sb[0:W, :], ps_o[0:W, :], COPY)
                    else:
                        nc.vector.tensor_copy(o_sb[0:W, :], ps_o[0:W, :])
                    nout[0] += 1
                    dst = outA if half == 0 else outB
                    nc.sync.dma_start(
                        out=dst[it * P : it * P + W, ch * 512 : (ch + 1) * 512],
                        in_=o_sb[0:W, :],
                    )

                # ---- filler queues ----
                # qk chunks for pairs 1..3: pair p before head 2p's scores.
                fast_start = len(qk_chunks) == 3
                fill_units = [
                    (f, c0, cw)
                    for pair in range(1, NP)
                    for f in (2 * pair, 2 * pair + 1)
                    for (c0, cw) in qk_chunks
                    if not (fast_start and f == 2 and c0 < qk_chunks[2][0])
                ]
                n_units = len(fill_units)
                cpp = 2 * len(qk_chunks)
                fill_pos = [0]
                cpair1 = cpp - (2 if fast_start else 0)

                def emit_fill_to(tgt):
                    k = fill_pos[0]
                    for u in fill_units[k : min(n_units, tgt)]:
                        emit_qk_chunk(*u)
                    fill_pos[0] = max(k, min(n_units, tgt))

                def needed_before(h):
                    p = max(0, h // 2)
                    return min(n_units, cpair1 if p == 1 else
                               cpair1 + (p - 1) * cpp if p > 1 else 0)

                # A half-chunks (pairs 0-1): ready once pair-1 main+tail
                # transposes are done (end of window 5); fill windows 6-7.
                a_units = [
                    (it, ch) for it in range(off0, off0 + w0) for ch in range(2)
                ]
                nA = len(a_units)
                a_pos = [0]

                def emit_a_to(tgt):
                    k = a_pos[0]
                    for u in a_units[k : min(nA, tgt)]:
                        emit_half(u[0], u[1], 0)
                    a_pos[0] = max(k, min(nA, tgt))

                # ---- window 0: pair-0 projections + head-0 scores,
                #      pair-1 qk chunks interleaved ----
                if len(qk_chunks) == 3:
                    # dmt-outer interleave across the f0/f1 chunks plus
                    # pair-1's first q chunks keeps the PE fed at DMA
                    # arrival pace (chunk-slots borrowed from the idle
                    # pss/psa pools + pqk)
                    sA = pss.tile([P, PSW], f32, tag="s", name="ps_q0")
                    sB = pss.tile([P, PSW], f32, tag="s", name="ps_q1")
                    qA = pqk.tile([P, 512], f32, tag="qk", name="ps_q2")
                    qB = pqk.tile([P, 512], f32, tag="qk", name="ps_q3")
                    aA = psa.tile([P, PSW], f32, tag="att", name="ps_q4")
                    units = [
                        (0, qk_chunks[0][0], qk_chunks[0][1], sA, 0),
                        (0, qk_chunks[1][0], qk_chunks[1][1], sA, 512),
                        (0, qk_chunks[2][0], qk_chunks[2][1], sB, 0),
                        (1, qk_chunks[0][0], qk_chunks[0][1], sB, 512),
                        (1, qk_chunks[1][0], qk_chunks[1][1], qA, 0),
                        (1, qk_chunks[2][0], qk_chunks[2][1], qB, 0),
                        (2, qk_chunks[0][0], qk_chunks[0][1], aA, 0),
                        (2, qk_chunks[1][0], qk_chunks[1][1], aA, 512),
                    ]
                    for dmt in range(DMT):
                        for f, c0, cw, ps, so in units:
                            nc.tensor.matmul(
                                ps[:, so : so + cw],
                                wqk_sb[
                                    :, dmt * FQK + f * P : dmt * FQK + (f + 1) * P
                                ],
                                xTl_sb[:, dmt * NLP + c0 : dmt * NLP + c0 + cw],
                                start=(dmt == 0),
                                stop=(dmt == DMT - 1),
                            )
                    for f, c0, cw, ps, so in units:
                        nc.vector.tensor_copy(
                            qk_all[:, f * BS + c0 : f * BS + c0 + cw],
                            ps[:, so : so + cw],
                        )
                else:
                    for f in (0, 1):
                        for c0, cw in qk_chunks:
                            emit_qk_chunk(f, c0, cw)
                t_store = {0: []}
                tt_store = {}
                for jt in range(NTL):
                    t_store[0].append(sc_wide(0, off0, w0, jt))
                if tail1:
                    # pair-0 tail scores cover the wv DMA wait; their spill
                    # reads (pair-1 q block) are written by the fast startup
                    tt_store[0] = s_tail(0, tail1[0][0])
                    tt_store[1] = s_tail(1, tail1[0][0])
                for tt in range(min(3, NTL)):
                    emit_v(tt)
                emit_fill_to(cpair1)

                # ---- windows 1..7: S(h) || PV(h-1) || tail(h-1 scores,
                #      h-2 PV) || fillers ----
                ap_cur = None
                apt_cur = None
                for h in range(1, HL):
                    emit_fill_to(needed_before(h))
                    fprev = fill_pos[0]
                    fth = n_units if h >= 5 else max(
                        needed_before(h + 1), (n_units * h * 2 + 8) // 9
                    )
                    aprev = a_pos[0]
                    ath = {5: 6, 6: 14}.get(h, 0 if h < 5 else nA)
                    t_store[h] = []
                    pa = psa.tile([P, PSW], f32, tag="att", name="pa")
                    for jt in range(NTL):
                        t_store[h].append(sc_wide(h, off0, w0, jt))
                        if h == 1 and jt >= 3:
                            emit_v(jt)
                        emit_a_to(aprev + ((ath - aprev) * (jt + 1)) // NTL)
                        pv_wide(h - 1, w0, jt, t_store[h - 1][jt], pa)
                        emit_fill_to(fprev + ((fth - fprev) * (jt + 1)) // NTL)
                        t_store[h - 1][jt] = None
                        if jt == 2 and h >= 2 and tail1 and (h - 2) in tt_store:
                            # lag-2 tail PV for head h-2
                            offt = tail1[0][0]
                            if (h - 2) % 2 == 0:
                                apt_cur = app.tile([P, P], bf, tag="apt", name="apt")
                            p_tail(h - 2, offt, tt_store.pop(h - 2), apt_cur)
                            if (h - 2) % 2 == 1:
                                transpose_pair((h - 2) // 2, offt, 1, apt_cur)
                    if tail1 and h >= 3:
                        tt_store[h - 1] = s_tail(h - 1, tail1[0][0])
                    if tail1 and h == HL - 1:
                        tt_store[h] = s_tail(h, tail1[0][0])
                    corr_wide(h - 1, off0, w0, pa)
                    if (h - 1) % 2 == 0:
                        ap_cur = app.tile([P, PSW], bf, tag="ap", name="ap")
                    norm_wide(h - 1, w0, pa, ap_cur)
                    if (h - 1) % 2 == 1:
                        transpose_pair((h - 1) // 2, off0, w0, ap_cur)

                # ---- epilogue: PV(7), tails 6-7, pipelined pair-3 finish ----
                def emit_outB_big(it, split=False):
                    # whole-row B chunk: pairs 2-3 for both DM halves in one
                    # 2-bank pss slot, one eviction, one outB DMA
                    W = IW(it)
                    ps_o = pss.tile([P, PSW], f32, tag="s", name="ps_b")
                    for ch in range(2):
                        for ht in (2, 3):
                            nc.tensor.matmul(
                                ps_o[0:W, ch * 512 : (ch + 1) * 512],
                                attT[:, ht * NLP + it * P : ht * NLP + it * P + W],
                                wout_sb[
                                    :, ht * DM + ch * 512 : ht * DM + (ch + 1) * 512
                                ],
                                start=(ht == 2),
                                stop=(ht == 3),
                            )
                    o_sb = sp.tile([P, 1024], bf, tag="obB", name="o_sbB")
                    nbig[0] += 1
                    if split:
                        # parallel eviction halves (ACT + DVE): shallow drain
                        nc.scalar.activation(
                            o_sb[0:W, 0:512], ps_o[0:W, 0:512], COPY
                        )
                        nc.vector.tensor_copy(
                            o_sb[0:W, 512:1024], ps_o[0:W, 512:1024]
                        )
                        nc.sync.dma_start(
                            out=outB[it * P : it * P + W, 0:512],
                            in_=o_sb[0:W, 0:512],
                        )
                        nc.sync.dma_start(
                            out=outB[it * P : it * P + W, 512:1024],
                            in_=o_sb[0:W, 512:1024],
                        )
                        return
                    if nbig[0] % 2 == 1:
                        nc.scalar.activation(o_sb[0:W, :], ps_o[0:W, :1024], COPY)
                    else:
                        nc.vector.tensor_copy(o_sb[0:W, :], ps_o[0:W, :1024])
                    nc.sync.dma_start(
                        out=outB[it * P : it * P + W, :], in_=o_sb[0:W, :]
                    )

                if tail1 and (HL - 1) not in tt_store:
                    tt_store[HL - 1] = s_tail(HL - 1, tail1[0][0])
                emit_fill_to(n_units)
                aprev = a_pos[0]
                pa = psa.tile([P, PSW], f32, tag="att", name="pa")
                for jt in range(NTL):
                    emit_a_to(aprev + ((nA - aprev) * (jt + 1)) // NTL)
                    pv_wide(HL - 1, w0, jt, t_store[HL - 1][jt], pa)
                    if jt == 2 and tail1 and (HL - 2) in tt_store:
                        offt = tail1[0][0]
                        apt_cur = app.tile([P, P], bf, tag="apt", name="apt")
                        p_tail(HL - 2, offt, tt_store.pop(HL - 2), apt_cur)
                    if jt == 4 and tail1 and (HL - 1) in tt_store:
                        offt = tail1[0][0]
                        p_tail(HL - 1, offt, tt_store.pop(HL - 1), apt_cur)
                        transpose_pair(NP - 1, offt, 1, apt_cur)
                    if jt == 0 and tail1:
                        # tail i-tile outA rows (pairs 0-1 only): PE work to
                        # cover the window-boundary normalize wait
                        offt = tail1[0][0]
                        emit_half(offt, 0, 0)
                        emit_half(offt, 1, 0)

                corr_wide(HL - 1, off0, w0, pa)
                emit_a_to(nA)
                # pipelined pair-3 finish: normalize slot -> transpose ->
                # previous tile's whole-row B chunk (hides eviction latency)
                p0e = ((HL - 1) % 2) * 64
                r_sb = sp.tile([P, 8], f32, tag="r", name="r_e")
                pa3 = pa.rearrange("p (t c) -> p t c", t=PSW // P, c=P)
                nc.vector.reciprocal(r_sb[:, :w0], pa3[:, :w0, DH])
                for it in range(w0):
                    nc.vector.tensor_scalar_mul(
                        ap_cur[:, it * P + p0e : it * P + p0e + DH],
                        pa[:, it * P : it * P + DH],
                        r_sb[:, it : it + 1],
                    )
                    ps_tr = pqk.tile([P, P], bf, tag="qk", name="ps_tr")
                    nc.tensor.transpose(
                        ps_tr[:, :], ap_cur[:, it * P : (it + 1) * P], ident
                    )
                    nc.vector.tensor_copy(
                        attT[
                            :,
                            (NP - 1) * NLP + (off0 + it) * P : (NP - 1) * NLP
                            + (off0 + it + 1) * P,
                        ],
                        ps_tr[:, :],
                    )
                    if it >= 1:
                        emit_outB_big(off0 + it - 1, split=(it == w0 - 1))
                emit_outB_big(off0 + w0 - 1, split=True)
                if tail1:
                    # tail i-tile outB last: its 24-row eviction/DMAs drain
                    # far faster than a full 128-row tile's
                    emit_outB_big(tail1[0][0], split=True)

                # ---- slow generic path for wide tail spans (NTL > 9) ----
                for offt, wt in slow_tails:
                    for pair in range(NP):
                        for hh in range(2):
                            h = 2 * pair + hh
                            pa = psa.tile([P, PSW], f32, tag="att", name="pa_w")
                            tw = [sc_wide(h, offt, wt, jt) for jt in range(NTL)]
                            for jt in range(NTL):
                                pv_wide(h, wt, jt, tw[jt], pa)
                            corr_wide(h, offt, wt, pa)
                            if hh == 0:
                                ap_t = app.tile([P, PSW], bf, tag="ap", name="ap_w")
                            norm_wide(h, wt, pa, ap_t)
                        transpose_pair(pair, offt, wt, ap_t)
                    for it in range(offt, offt + wt):
                        for ch in range(2):
                            emit_half(it, ch, 0)

                # B halves for slow-tail tiles (already emitted for the rest)
                for offt, wt in slow_tails:
                    for it in range(offt, offt + wt):
                        for ch in range(2):
                            emit_half(it, ch, 1)

    nc.compile()
    return nc


def _shard_inputs(x, w_qkv, w_out, b_out, mask):
    """Host-side live-token gather + per-core layout prep."""
    bf = ml_dtypes.bfloat16
    x = np.asarray(x, dtype=np.float32)
    w_qkv = np.asarray(w_qkv, dtype=np.float32)
    w_out = np.asarray(w_out, dtype=np.float32)
    mask = np.asarray(mask)

    NLs = [int(mask[b].sum()) for b in range(B)]
    NLP = int(np.ceil((max(NLs) + 1) / 8) * 8)
    NTL = (NLP + P - 1) // P

    w3 = w_qkv.reshape(DM, H, 3, DH)
    in_maps = []
    for c in range(NCORES):
        b, hg = c // HG, c % HG
        nl = NLs[b]
        live = np.nonzero(mask[b])[0]
        dead = np.nonzero(mask[b] == 0)[0]

        wq = w3[:, hg * HL : (hg + 1) * HL, 0, :].reshape(DM, FV) * SCALE
        wk = w3[:, hg * HL : (hg + 1) * HL, 1, :].reshape(DM, FV)
        # pair-major column layout: [q_p0 | k_p0 | q_p1 | k_p1 | ...]
        wqk_c = np.empty((DM, FQK), np.float32)
        for p in range(NP):
            wqk_c[:, p * 256 : p * 256 + 128] = wq[:, p * 128 : (p + 1) * 128]
            wqk_c[:, p * 256 + 128 : (p + 1) * 256] = wk[:, p * 128 : (p + 1) * 128]
        wv_c = np.ascontiguousarray(
            w3[:, hg * HL : (hg + 1) * HL, 2, :].reshape(DM, FV)
        )
        wout_c = np.ascontiguousarray(w_out[hg * FV : (hg + 1) * FV, :])

        xTl_c = np.zeros((DM, NLP), np.float32)
        xTl_c[:, :nl] = x[b].T[:, live]

        padrow_c = np.zeros((1, NLP), np.float32)
        padrow_c[0, nl:] = 1.0
        lind_c = np.zeros(NTL * P, np.float32)
        lind_c[:nl] = 1.0
        lind_c = np.ascontiguousarray(lind_c.reshape(NTL, P).T)

        # dvec: per head [sum_dead v_h | N_dead]
        xs = x[b][dead].sum(axis=0) if len(dead) else np.zeros(DM, np.float32)
        dv = (xs @ wv_c).reshape(HL, DH)
        dvec_c = np.empty((1, VROW), np.float32)
        for h in range(HL):
            dvec_c[0, h * VW : h * VW + DH] = dv[h]
            dvec_c[0, h * VW + DH] = float(len(dead))

        in_maps.append(
            {
                "xTl": xTl_c.astype(bf),
                "wqk": wqk_c.astype(bf),
                "wv": wv_c.astype(bf),
                "wout": wout_c.astype(bf),
                "padrow": padrow_c.astype(bf),
                "lind": lind_c.astype(np.float32),
                "dvec": dvec_c.astype(bf),
            }
        )
    return in_maps, NLP, NLs


def kernel(x, w_qkv, w_out, b_out, mask):
    from concourse.bass_utils import run_bass_kernel_spmd

    in_maps, NLP, NLs = _shard_inputs(x, w_qkv, w_out, b_out, mask)
    if NLP not in _CACHE:
        _CACHE[NLP] = _build_program(NLP)
    nc = _CACHE[NLP]

    res = run_bass_kernel_spmd(nc, in_maps, list(range(NCORES))).results

    mask = np.asarray(mask)
    b_out = np.asarray(b_out, dtype=np.float32)
    out = np.empty((B, N, DM), np.float32)
    for b in range(B):
        nl = NLs[b]
        live = np.nonzero(mask[b])[0]
        dead = np.nonzero(mask[b] == 0)[0]
        part = (
            res[HG * b]["outA"]
            + res[HG * b]["outB"].astype(np.float32)
            + res[HG * b + 1]["outA"]
            + res[HG * b + 1]["outB"].astype(np.float32)
        )
        out[b, live] = part[:nl]
        if len(dead):
            out[b, dead] = part[nl]
        out[b] += b_out[None, :]
    return out


# revision 100
# speedup vs baseline: 1.0085x; 1.0021x over previous
"""Multi-head attention (B=4, N=2048, DM=1024, H=16, DH=64) on 8 trn2 cores.

Sharding: core c -> (batch b = c//2, head-group hg = c%2 of 8 heads).

Live-token compaction: the pair mask only keeps (i,j) score pairs where
both tokens are live, and every dead query row of the reference output is
the SAME uniform average over all value tokens.  So the host gathers the
~NL live tokens of each batch into a compact [DM, NLP] x^T (NLP = padded
live count, multiple of 128, >= NL+1), the device runs attention on live
tokens only, and the host scatters rows back, filling dead rows with the
row produced by the first zero-padded query column.

Per-column semantics on device (q = x@Wq, k = x@Wk, scaled, no masking):
  - live i, live j: t = exp(q_i.k_j) -- the real softmax numerator.
  - pad i (x=0 -> q=0): t = 1 for all j, plus a rank-1 correction row
    (+padrow_i * dvec, dvec = [sum_dead v | N_dead] from the host) makes
    pv = [sum_all v | N], i.e. exactly the reference's uniform dead row.
  - pad j (x=0 -> k=v=0): t = exp(0) = 1 but vplus rows are zero (the
    denominator ones-column is L=live-indicator, not constant 1), so pads
    contribute nothing.

Device layout mirrors the dense kernel: feature-major q/k ([64,NLP] per
head), token-major v with an appended denominator column, scores
transposed [j,i] so PV needs no transpose, exp on ACT, a small [N,64]
transpose per head for the output projection.

Scheduling: heads run as a software pipeline -- window W(h) emits scores
+exp for head h, PV for head h-1, the batched last-i-tile ("tail") scores
for head h-1 and PV for head h-2, plus deadline-scheduled filler chunks
(later pairs' qk projections, then partial output chunks).  The output
projection is split into two half-contractions written to separate DRAM
tensors (outA = pairs 0-1 in f32 overlapped with the mid loop, outB =
pairs 2-3 in bf16 to halve the drain DMA); the host sums them.
"""

import sys

sys.path.insert(0, "/opt/trn_rl_repo")

import numpy as np
import ml_dtypes

B, N, DM, H, DH = 4, 2048, 1024, 16, 64
SCALE = DH**-0.5
NCORES = 8
HG = 2  # head groups (tensor-parallel factor)
HL = H // HG  # 8 heads per core
NP = HL // 2  # 4 head pairs
FQK = HL * 2 * DH  # 1024 qk features per core
FV = HL * DH  # 512 v features per core
P = 128
DMT = DM // P  # 8 dm tiles
VW = DH + 1  # 65: v columns + denominator column
VROW = HL * VW  # 520
HT = FV // P  # 4 head-dim tiles for the projection

_CACHE = {}


def _build_program(NLP):
    import concourse.mybir as mybir
    import concourse.tile as tile
    from concourse import bacc
    from concourse.masks import make_identity

    bf = mybir.dt.bfloat16
    f32 = mybir.dt.float32
    EXP = mybir.ActivationFunctionType.Exp
    COPY = mybir.ActivationFunctionType.Copy

    NTL = (NLP + P - 1) // P  # live token tiles (last may be partial)
    TW = NLP - (NTL - 1) * P  # width of the last tile
    # i-span structure: one wide main span (software-pipelined head loop),
    # remaining tiles handled in the pipelined tail path (w==1) or a slow
    # generic path (w>1, only for much larger masks).
    spans = []
    off = 0
    while NTL - off > 0:
        w = min(8, NTL - off)
        spans.append((off, w))
        off += w
    off0, w0 = spans[0]
    tail1 = [s for s in spans[1:] if s[1] == 1]
    slow_tails = [s for s in spans[1:] if s[1] > 1]

    def IW(it):
        return TW if it == NTL - 1 else P

    # qk_all block stride: a (P-TW)-col pad after each feature block so the
    # full-width kT reads of the partial last j-tile never touch another
    # block's (possibly unwritten) data
    BS = NLP + (P - TW) if TW < P else NLP
    # qk-projection column chunks (psum-bank sized)
    qk_chunks = [(c0, min(512, NLP - c0)) for c0 in range(0, NLP, 512)]

    nc = bacc.Bacc(
        "TRN2", target_bir_lowering=False, debug=False, num_devices=NCORES
    )
    xTl = nc.dram_tensor("xTl", [DM, NLP], bf, kind="ExternalInput")
    wqk = nc.dram_tensor("wqk", [DM, FQK], bf, kind="ExternalInput")
    wv = nc.dram_tensor("wv", [DM, FV], bf, kind="ExternalInput")
    wout = nc.dram_tensor("wout", [FV, DM], bf, kind="ExternalInput")
    padrow = nc.dram_tensor("padrow", [1, NLP], bf, kind="ExternalInput")
    lind = nc.dram_tensor("lind", [P, NTL], f32, kind="ExternalInput")
    dvec = nc.dram_tensor("dvec", [1, VROW], bf, kind="ExternalInput")
    outA = nc.dram_tensor("outA", [NLP, DM], f32, kind="ExternalOutput")
    outB = nc.dram_tensor("outB", [NLP, DM], bf, kind="ExternalOutput")

    with tile.TileContext(nc) as tc:
        with tc.tile_pool(name="const", bufs=1) as cp:
            xTl_sb = cp.tile([P, DMT * NLP], bf, tag="xTl")
            wqk_sb = cp.tile([P, DMT * FQK], bf, tag="wqk")
            wv_sb = cp.tile([P, DMT * FV], bf, tag="wv")
            wout_sb = cp.tile([P, HT * DM], bf, tag="wout")
            padrow_sb = cp.tile([1, NLP], bf, tag="padrow")
            lind_sb = cp.tile([P, NTL], f32, tag="lind")
            dvec_sb = cp.tile([1, VROW], bf, tag="dvec")
            ident = cp.tile([P, P], bf, tag="ident")
            zeros8 = cp.tile([P, HL], bf, tag="zeros8")
            vplus = cp.tile([P, NTL * VROW], bf, tag="vplus")
            qk_all = cp.tile([P, HL * BS], bf, tag="qkall")
            attT = cp.tile([P, HT * NLP], bf, tag="attT")

            # DMA order mirrors consumption: per-dm-tile x^T plus the
            # pair-0 qk weight columns first (feeds the first projection),
            # then v weights (needed by the head-1 window), small tensors,
            # then the remaining qk weight columns and w_out.
            for dmt in range(DMT):
                nc.sync.dma_start(
                    out=wqk_sb[:, dmt * FQK : dmt * FQK + 3 * P],
                    in_=wqk[dmt * P : (dmt + 1) * P, 0 : 3 * P],
                )
                nc.sync.dma_start(
                    out=xTl_sb[:, dmt * NLP : (dmt + 1) * NLP],
                    in_=xTl[dmt * P : (dmt + 1) * P, :],
                )
            nc.sync.dma_start(
                out=wv_sb[:, :].rearrange("p (d f) -> p d f", d=DMT, f=FV),
                in_=wv[:, :].rearrange("(d p) c -> p d c", p=P),
            )
            nc.sync.dma_start(out=lind_sb[:, :], in_=lind[:, :])
            nc.sync.dma_start(out=padrow_sb[:, :], in_=padrow[:, :])
            nc.sync.dma_start(out=dvec_sb[:, :], in_=dvec[:, :])
            for dmt in range(DMT):
                nc.sync.dma_start(
                    out=wqk_sb[:, dmt * FQK + 3 * P : (dmt + 1) * FQK],
                    in_=wqk[dmt * P : (dmt + 1) * P, 3 * P :],
                )
            for ht in range(HT):
                nc.sync.dma_start(
                    out=wout_sb[:, ht * DM : (ht + 1) * DM],
                    in_=wout[ht * P : (ht + 1) * P, :],
                )
            make_identity(nc, ident)
            nc.gpsimd.memset(zeros8[:, :], 0.0)
            # zero the pad margin after each feature block (spill target of
            # the full-width kT reads of the partial last j-tile)
            if TW < P:
                for f in range(HL):
                    nc.gpsimd.memset(qk_all[:, f * BS + NLP : (f + 1) * BS], 0.0)
            if TW < P:
                # rows of the partial last j-tile beyond the live+pad range
                # never get v written; zero the whole block up front (the v
                # eviction later overwrites rows [0:TW]) so spilled-garbage
                # exp rows contract against zeros
                nc.gpsimd.memset(vplus[:, (NTL - 1) * VROW : NTL * VROW], 0.0)

            vp4 = vplus.rearrange("p (t g c) -> p t g c", t=NTL, g=HL, c=VW)

            with (
                tc.tile_pool(name="psqk", bufs=2, space="PSUM") as pqk,
                tc.tile_pool(name="pss", bufs=2, space="PSUM") as pss,
                tc.tile_pool(name="psa", bufs=1, space="PSUM") as psa,
                tc.tile_pool(name="tpool", bufs=20) as tp,
                tc.tile_pool(name="ttpool", bufs=3) as ttp,
                tc.tile_pool(name="appool", bufs=2) as app,
                tc.tile_pool(name="spool", bufs=4) as sp,
            ):
                PSW = min(w0 * P, 1024)

                def emit_qk_chunk(f, c0, cw):
                    ps = pqk.tile([P, 512], f32, tag="qk", name="ps_qk")
                    for dmt in range(DMT):
                        nc.tensor.matmul(
                            ps[:, :cw],
                            wqk_sb[:, dmt * FQK + f * P : dmt * FQK + (f + 1) * P],
                            xTl_sb[:, dmt * NLP + c0 : dmt * NLP + c0 + cw],
                            start=(dmt == 0),
                            stop=(dmt == DMT - 1),
                        )
                    nc.vector.tensor_copy(
                        qk_all[:, f * BS + c0 : f * BS + c0 + cw], ps[:, :cw]
                    )

                def emit_v(tt):
                    W = IW(tt)
                    ps = pqk.tile([P, FV], f32, tag="qk", name="ps_v")
                    for dmt in range(DMT):
                        nc.tensor.matmul(
                            ps[0:W, :],
                            xTl_sb[:, dmt * NLP + tt * P : dmt * NLP + tt * P + W],
                            wv_sb[:, dmt * FV : (dmt + 1) * FV],
                            start=(dmt == 0),
                            stop=(dmt == DMT - 1),
                        )
                    nc.vector.tensor_copy(
                        vp4[0:W, tt, :, 0:DH],
                        ps[0:W].rearrange("p (g c) -> p g c", g=HL, c=DH),
                    )
                    # denominator column = live indicator (0 for pad rows)
                    nc.vector.tensor_scalar_add(
                        vp4[0:W, tt, :, DH],
                        zeros8[0:W, :],
                        lind_sb[0:W, tt : tt + 1],
                    )

                def sc_wide(h, off, w, jt):
                    pair, hh = h // 2, h % 2
                    p0 = hh * 64
                    qc = (2 * pair) * BS + off * P
                    kc = (2 * pair + 1) * BS
                    ps_s = pss.tile([P, PSW], f32, tag="s", name="ps_s")
                    for c0 in range(0, w * P, 512):
                        cw = min(512, w * P - c0)
                        nc.tensor.matmul(
                            ps_s[:, c0 : c0 + cw],
                            qk_all[p0 : p0 + 64, kc + jt * P : kc + (jt + 1) * P],
                            qk_all[p0 : p0 + 64, qc + c0 : qc + c0 + cw],
                            start=True,
                            stop=True,
                        )
                    t_sb = tp.tile([P, PSW], bf, tag="t", name="t_sb")
                    nc.scalar.activation(t_sb[:, : w * P], ps_s[:, : w * P], EXP)
                    return t_sb

                def pv_wide(h, w, jt, t_sb, pa):
                    vsl = vplus[:, jt * VROW + h * VW : jt * VROW + (h + 1) * VW]
                    for it in range(w):
                        nc.tensor.matmul(
                            pa[:, it * P : it * P + VW],
                            t_sb[:, it * P : (it + 1) * P],
                            vsl,
                            start=(jt == 0 and it % 4 == 0),
                            stop=False,
                        )

                def corr_wide(h, off, w, pa):
                    for it in range(w):
                        nc.tensor.matmul(
                            pa[:, it * P : it * P + VW],
                            padrow_sb[:, (off + it) * P : (off + it + 1) * P],
                            dvec_sb[:, h * VW : (h + 1) * VW],
                            start=False,
                            stop=(it % 4 == 3 or it == w - 1),
                        )

                def norm_wide(h, w, pa, ap):
                    p0 = (h % 2) * 64
                    r_sb = sp.tile([P, 8], f32, tag="r", name="r_sb")
                    pa3 = pa.rearrange("p (t c) -> p t c", t=PSW // P, c=P)
                    nc.vector.reciprocal(r_sb[:, :w], pa3[:, :w, DH])
                    for it in range(w):
                        nc.vector.tensor_scalar_mul(
                            ap[:, it * P + p0 : it * P + p0 + DH],
                            pa[:, it * P : it * P + DH],
                            r_sb[:, it : it + 1],
                        )

                def pv_slot(h, off, w, it, pa, t_list, ap):
                    # slot-major: finish output tile `it` for head h in one
                    # go (all-j PV + correction + normalize), so downstream
                    # per-tile work pipelines inside the window
                    p0 = (h % 2) * 64
                    vcol = h * VW
                    for jt in range(NTL):
                        nc.tensor.matmul(
                            pa[:, it * P : it * P + VW],
                            t_list[jt][:, it * P : (it + 1) * P],
                            vplus[:, jt * VROW + vcol : jt * VROW + vcol + VW],
                            start=(jt == 0 and it % 4 == 0),
                            stop=False,
                        )
                    nc.tensor.matmul(
                        pa[:, it * P : it * P + VW],
                        padrow_sb[:, (off + it) * P : (off + it + 1) * P],
                        dvec_sb[:, h * VW : (h + 1) * VW],
                        start=False,
                        stop=(it % 4 == 3 or it == w - 1),
                    )
                    r1 = sp.tile([P, 8], f32, tag="r", name="r1")
                    nc.vector.reciprocal(
                        r1[:, 0:1], pa[:, it * P + DH : it * P + DH + 1]
                    )
                    nc.vector.tensor_scalar_mul(
                        ap[:, it * P + p0 : it * P + p0 + DH],
                        pa[:, it * P : it * P + DH],
                        r1[:, 0:1],
                    )

                def transpose_it(pair, off, it, ap):
                    ps_tr = pqk.tile([P, P], bf, tag="qk", name="ps_tr")
                    nc.tensor.transpose(
                        ps_tr[:, :], ap[:, it * P : (it + 1) * P], ident
                    )
                    dst = attT[
                        :,
                        pair * NLP + (off + it) * P : pair * NLP
                        + (off + it + 1) * P,
                    ]
                    nc.vector.tensor_copy(dst, ps_tr[:, :])

                def transpose_pair(pair, off, w, ap):
                    # batch 4 transposes per 1-bank slot -> one eviction copy
                    it = 0
                    while it < w:
                        nb = min(4, w - it)
                        if IW(off + it + nb - 1) != P:
                            nb = 1
                        ps_tr = pqk.tile([P, 512], bf, tag="qk", name="ps_tr")
                        wtot = 0
                        for k in range(nb):
                            W = IW(off + it + k)
                            nc.tensor.transpose(
                                ps_tr[:, k * P : k * P + W],
                                ap[0:W, (it + k) * P : (it + k + 1) * P],
                                ident[0:W, 0:W],
                            )
                            wtot = k * P + W
                        nc.vector.tensor_copy(
                            attT[
                                :,
                                pair * NLP + (off + it) * P : pair * NLP
                                + (off + it) * P
                                + wtot,
                            ],
                            ps_tr[:, 0:wtot],
                        )
                        it += nb

                def s_tail(h, off):
                    # batched narrow-tail scores: the TW-wide last i-tile for
                    # all j-tiles, packed into as few psum banks / exp
                    # instructions as possible
                    pair, hh = h // 2, h % 2
                    p0 = hh * 64
                    qc = (2 * pair) * BS + off * P
                    kc = (2 * pair + 1) * BS
                    bpb = max(1, 512 // TW)  # batches per psum bank
                    t_t = ttp.tile([P, NTL * TW], bf, tag="tt", name="t_tail")
                    done = 0
                    while done < NTL:
                        nb = min(2 * bpb, NTL - done)  # one 2-bank slot
                        ps_s = pss.tile([P, PSW], f32, tag="s", name="ps_st")
                        for g in range(nb):
                            jt = done + g
                            pos = (g // bpb) * 512 + (g % bpb) * TW
                            nc.tensor.matmul(
                                ps_s[:, pos : pos + TW],
                                qk_all[p0 : p0 + 64, kc + jt * P : kc + jt * P + P],
                                qk_all[p0 : p0 + 64, qc : qc + TW],
                                start=True,
                                stop=True,
                            )
                        for bk in range((nb + bpb - 1) // bpb):
                            cnt = min(bpb, nb - bk * bpb)
                            nc.scalar.activation(
                                t_t[
                                    :,
                                    (done + bk * bpb) * TW : (done + bk * bpb + cnt)
                                    * TW,
                                ],
                                ps_s[:, bk * 512 : bk * 512 + cnt * TW],
                                EXP,
                            )
                        done += nb
                    return t_t

                def p_tail(h, off, t_t, ap):
                    # 65-col accumulator lives in a 1-bank "qk" slot so it
                    # never waits on the main PV accumulator (deadlock risk)
                    p0 = (h % 2) * 64
                    pa = pqk.tile([P, 512], f32, tag="qk", name="pa_t")
                    for jt in range(NTL):
                        nc.tensor.matmul(
                            pa[0:TW, 0:VW],
                            t_t[:, jt * TW : (jt + 1) * TW],
                            vplus[:, jt * VROW + h * VW : jt * VROW + (h + 1) * VW],
                            start=(jt == 0),
                            stop=False,
                        )
                    nc.tensor.matmul(
                        pa[0:TW, 0:VW],
                        padrow_sb[:, off * P : off * P + TW],
                        dvec_sb[:, h * VW : (h + 1) * VW],
                        start=False,
                        stop=True,
                    )
                    r_sb = sp.tile([P, 8], f32, tag="r", name="r_t")
                    nc.vector.reciprocal(r_sb[0:TW, 0:1], pa[0:TW, DH : DH + 1])
                    nc.vector.tensor_scalar_mul(
                        ap[0:TW, p0 : p0 + DH], pa[0:TW, 0:DH], r_sb[0:TW, 0:1]
                    )

                nout = [0]
                nbig = [0]

                def emit_half(it, ch, half):
                    # half 0: pairs 0-1 -> outA f32; half 1: pairs 2-3 -> outB bf16
                    W = IW(it)
                    ps_o = pqk.tile([P, 512], f32, tag="qk", name="ps_o")
                    for ht in (0, 1) if half == 0 else (2, 3):
                        nc.tensor.matmul(
                            ps_o[0:W, :],
                            attT[:, ht * NLP + it * P : ht * NLP + it * P + W],
                            wout_sb[:, ht * DM + ch * 512 : ht * DM + (ch + 1) * 512],
                            start=(ht % 2 == 0),
                            stop=(ht % 2 == 1),
                        )
                    dt = f32 if half == 0 else bf
                    o_sb = sp.tile([P, 512], dt, tag="obA" if half == 0 else "obB",
                                   name="o_sb")
                    # A-half evictions stay off ACT (it paces mid-loop exps);
                    # B-half runs in the drain where ACT is idle.
                    if half == 1:
                        nc.scalar.activation(o_sb[0:W, :], ps_o[0:W, :], COPY)
                    else:
                        nc.vector.tensor_copy(o_sb[0:W, :], ps_o[0:W, :])
                    nout[0] += 1
                    dst = outA if half == 0 else outB
                    nc.sync.dma_start(
                        out=dst[it * P : it * P + W, ch * 512 : (ch + 1) * 512],
                        in_=o_sb[0:W, :],
                    )

                # ---- filler queues ----
                # qk chunks for pairs 1..3: pair p before head 2p's scores.
                fast_start = len(qk_chunks) == 3
                fill_units = [
                    (f, c0, cw)
                    for pair in range(1, NP)
                    for f in (2 * pair, 2 * pair + 1)
                    for (c0, cw) in qk_chunks
                    if not (fast_start and f == 2 and c0 < qk_chunks[2][0])
                ]
                n_units = len(fill_units)
                cpp = 2 * len(qk_chunks)
                fill_pos = [0]
                cpair1 = cpp - (2 if fast_start else 0)

                def emit_fill_to(tgt):
                    k = fill_pos[0]
                    for u in fill_units[k : min(n_units, tgt)]:
                        emit_qk_chunk(*u)
                    fill_pos[0] = max(k, min(n_units, tgt))

                def needed_before(h):
                    p = max(0, h // 2)
                    return min(n_units, cpair1 if p == 1 else
                               cpair1 + (p - 1) * cpp if p > 1 else 0)

                # A half-chunks (pairs 0-1): ready once pair-1 main+tail
                # transposes are done (end of window 5); fill windows 6-7.
                a_units = [
                    (it, ch) for it in range(off0, off0 + w0) for ch in range(2)
                ]
                nA = len(a_units)
                a_pos = [0]

                def emit_a_to(tgt):
                    k = a_pos[0]
                    for u in a_units[k : min(nA, tgt)]:
                        emit_half(u[0], u[1], 0)
                    a_pos[0] = max(k, min(nA, tgt))

                # ---- window 0: pair-0 projections + head-0 scores,
                #      pair-1 qk chunks interleaved ----
                if len(qk_chunks) == 3:
                    # dmt-outer interleave across the f0/f1 chunks plus
                    # pair-1's first q chunks keeps the PE fed at DMA
                    # arrival pace (chunk-slots borrowed from the idle
                    # pss/psa pools + pqk)
                    sA = pss.tile([P, PSW], f32, tag="s", name="ps_q0")
                    sB = pss.tile([P, PSW], f32, tag="s", name="ps_q1")
                    qA = pqk.tile([P, 512], f32, tag="qk", name="ps_q2")
                    qB = pqk.tile([P, 512], f32, tag="qk", name="ps_q3")
                    aA = psa.tile([P, PSW], f32, tag="att", name="ps_q4")
                    units = [
                        (0, qk_chunks[0][0], qk_chunks[0][1], sA, 0),
                        (0, qk_chunks[1][0], qk_chunks[1][1], sA, 512),
                        (0, qk_chunks[2][0], qk_chunks[2][1], sB, 0),
                        (1, qk_chunks[0][0], qk_chunks[0][1], sB, 512),
                        (1, qk_chunks[1][0], qk_chunks[1][1], qA, 0),
                        (1, qk_chunks[2][0], qk_chunks[2][1], qB, 0),
                        (2, qk_chunks[0][0], qk_chunks[0][1], aA, 0),
                        (2, qk_chunks[1][0], qk_chunks[1][1], aA, 512),
                    ]
                    for dmt in range(DMT):
                        for f, c0, cw, ps, so in units:
                            nc.tensor.matmul(
                                ps[:, so : so + cw],
                                wqk_sb[
                                    :, dmt * FQK + f * P : dmt * FQK + (f + 1) * P
                                ],
                                xTl_sb[:, dmt * NLP + c0 : dmt * NLP + c0 + cw],
                                start=(dmt == 0),
                                stop=(dmt == DMT - 1),
                            )
                    for f, c0, cw, ps, so in units:
                        nc.vector.tensor_copy(
                            qk_all[:, f * BS + c0 : f * BS + c0 + cw],
                            ps[:, so : so + cw],
                        )
                else:
                    for f in (0, 1):
                        for c0, cw in qk_chunks:
                            emit_qk_chunk(f, c0, cw)
                t_store = {0: []}
                tt_store = {}
                for jt in range(NTL):
                    t_store[0].append(sc_wide(0, off0, w0, jt))
                if tail1:
                    # pair-0 tail scores cover the wv DMA wait; their spill
                    # reads (pair-1 q block) are written by the fast startup
                    tt_store[0] = s_tail(0, tail1[0][0])
                    tt_store[1] = s_tail(1, tail1[0][0])
                for tt in range(min(3, NTL)):
                    emit_v(tt)
                emit_fill_to(cpair1)

                # ---- windows 1..7: S(h) || PV(h-1) || tail(h-1 scores,
                #      h-2 PV) || fillers ----
                ap_cur = None
                apt_cur = None
                for h in range(1, HL):
                    emit_fill_to(needed_before(h))
                    fprev = fill_pos[0]
                    fth = n_units if h >= 5 else max(
                        needed_before(h + 1), (n_units * h * 2 + 8) // 9
                    )
                    aprev = a_pos[0]
                    ath = {5: 5, 6: 13}.get(h, 0 if h < 5 else nA)
                    t_store[h] = []
                    pa = psa.tile([P, PSW], f32, tag="att", name="pa")
                    for jt in range(NTL):
                        t_store[h].append(sc_wide(h, off0, w0, jt))
                        if h == 1 and jt >= 3:
                            emit_v(jt)
                        emit_a_to(aprev + ((ath - aprev) * (jt + 1)) // NTL)
                        pv_wide(h - 1, w0, jt, t_store[h - 1][jt], pa)
                        emit_fill_to(fprev + ((fth - fprev) * (jt + 1)) // NTL)
                        t_store[h - 1][jt] = None
                        if jt == 2 and h >= 2 and tail1 and (h - 2) in tt_store:
                            # lag-2 tail PV for head h-2
                            offt = tail1[0][0]
                            if (h - 2) % 2 == 0:
                                apt_cur = app.tile([P, P], bf, tag="apt", name="apt")
                            p_tail(h - 2, offt, tt_store.pop(h - 2), apt_cur)
                            if (h - 2) % 2 == 1:
                                transpose_pair((h - 2) // 2, offt, 1, apt_cur)
                    if tail1 and h >= 3:
                        tt_store[h - 1] = s_tail(h - 1, tail1[0][0])
                    if tail1 and h == HL - 1:
                        tt_store[h] = s_tail(h, tail1[0][0])
                    corr_wide(h - 1, off0, w0, pa)
                    if (h - 1) % 2 == 0:
                        ap_cur = app.tile([P, PSW], bf, tag="ap", name="ap")
                    norm_wide(h - 1, w0, pa, ap_cur)
                    if (h - 1) % 2 == 1:
                        transpose_pair((h - 1) // 2, off0, w0, ap_cur)

                # ---- epilogue: PV(7), tails 6-7, pipelined pair-3 finish ----
                def emit_outB_big(it, split=False):
                    # whole-row B chunk: pairs 2-3 for both DM halves in one
                    # 2-bank pss slot, one eviction, one outB DMA
                    W = IW(it)
                    ps_o = pss.tile([P, PSW], f32, tag="s", name="ps_b")
                    for ch in range(2):
                        for ht in (2, 3):
                            nc.tensor.matmul(
                                ps_o[0:W, ch * 512 : (ch + 1) * 512],
                                attT[:, ht * NLP + it * P : ht * NLP + it * P + W],
                                wout_sb[
                                    :, ht * DM + ch * 512 : ht * DM + (ch + 1) * 512
                                ],
                                start=(ht == 2),
                                stop=(ht == 3),
                            )
                    o_sb = sp.tile([P, 1024], bf, tag="obB", name="o_sbB")
                    nbig[0] += 1
                    if split:
                        # parallel eviction halves (ACT + DVE): shallow drain
                        nc.scalar.activation(
                            o_sb[0:W, 0:512], ps_o[0:W, 0:512], COPY
                        )
                        nc.vector.tensor_copy(
                            o_sb[0:W, 512:1024], ps_o[0:W, 512:1024]
                        )
                        nc.sync.dma_start(
                            out=outB[it * P : it * P + W, 0:512],
                            in_=o_sb[0:W, 0:512],
                        )
                        nc.sync.dma_start(
                            out=outB[it * P : it * P + W, 512:1024],
                            in_=o_sb[0:W, 512:1024],
                        )
                        return
                    if nbig[0] % 2 == 1:
                        nc.scalar.activation(o_sb[0:W, :], ps_o[0:W, :1024], COPY)
                    else:
                        nc.vector.tensor_copy(o_sb[0:W, :], ps_o[0:W, :1024])
                    nc.sync.dma_start(
                        out=outB[it * P : it * P + W, :], in_=o_sb[0:W, :]
                    )

                if tail1 and (HL - 1) not in tt_store:
                    tt_store[HL - 1] = s_tail(HL - 1, tail1[0][0])
                emit_fill_to(n_units)
                aprev = a_pos[0]
                pa = psa.tile([P, PSW], f32, tag="att", name="pa")
                for jt in range(NTL):
                    emit_a_to(aprev + ((nA - aprev) * (jt + 1)) // NTL)
                    pv_wide(HL - 1, w0, jt, t_store[HL - 1][jt], pa)
                    if jt == 2 and tail1 and (HL - 2) in tt_store:
                        offt = tail1[0][0]
                        apt_cur = app.tile([P, P], bf, tag="apt", name="apt")
                        p_tail(HL - 2, offt, tt_store.pop(HL - 2), apt_cur)
                    if jt == 4 and tail1 and (HL - 1) in tt_store:
                        offt = tail1[0][0]
                        p_tail(HL - 1, offt, tt_store.pop(HL - 1), apt_cur)
                        transpose_pair(NP - 1, offt, 1, apt_cur)
                    if jt == 0 and tail1:
                        # tail i-tile outA rows (pairs 0-1 only): PE work to
                        # cover the window-boundary normalize wait
                        offt = tail1[0][0]
                        emit_half(offt, 0, 0)
                        emit_half(offt, 1, 0)

                corr_wide(HL - 1, off0, w0, pa)
                emit_a_to(nA)
                # pipelined pair-3 finish: normalize slot -> transpose ->
                # previous tile's whole-row B chunk (hides eviction latency)
                p0e = ((HL - 1) % 2) * 64
                r_sb = sp.tile([P, 8], f32, tag="r", name="r_e")
                pa3 = pa.rearrange("p (t c) -> p t c", t=PSW // P, c=P)
                nc.vector.reciprocal(r_sb[:, :w0], pa3[:, :w0, DH])
                for it in range(w0):
                    nc.vector.tensor_scalar_mul(
                        ap_cur[:, it * P + p0e : it * P + p0e + DH],
                        pa[:, it * P : it * P + DH],
                        r_sb[:, it : it + 1],
                    )
                    ps_tr = pqk.tile([P, P], bf, tag="qk", name="ps_tr")
                    nc.tensor.transpose(
                        ps_tr[:, :], ap_cur[:, it * P : (it + 1) * P], ident
                    )
                    nc.vector.tensor_copy(
                        attT[
                            :,
                            (NP - 1) * NLP + (off0 + it) * P : (NP - 1) * NLP
                            + (off0 + it + 1) * P,
                        ],
                        ps_tr[:, :],
                    )
                    if it >= 1:
                        emit_outB_big(off0 + it - 1, split=(it == w0 - 1))
                emit_outB_big(off0 + w0 - 1, split=True)
                if tail1:
                    # tail i-tile outB last: its 24-row eviction/DMAs drain
                    # far faster than a full 128-row tile's
                    emit_outB_big(tail1[0][0], split=True)

                # ---- slow generic path for wide tail spans (NTL > 9) ----
                for offt, wt in slow_tails:
                    for pair in range(NP):
                        for hh in range(2):
                            h = 2 * pair + hh
                            pa = psa.tile([P, PSW], f32, tag="att", name="pa_w")
                            tw = [sc_wide(h, offt, wt, jt) for jt in range(NTL)]
                            for jt in range(NTL):
                                pv_wide(h, wt, jt, tw[jt], pa)
                            corr_wide(h, offt, wt, pa)
                            if hh == 0:
                                ap_t = app.tile([P, PSW], bf, tag="ap", name="ap_w")
                            norm_wide(h, wt, pa, ap_t)
                        transpose_pair(pair, offt, wt, ap_t)
                    for it in range(offt, offt + wt):
                        for ch in range(2):
                            emit_half(it, ch, 0)

                # B halves for slow-tail tiles (already emitted for the rest)
                for offt, wt in slow_tails:
                    for it in range(offt, offt + wt):
                        for ch in range(2):
                            emit_half(it, ch, 1)

    nc.compile()
    return nc


def _shard_inputs(x, w_qkv, w_out, b_out, mask):
    """Host-side live-token gather + per-core layout prep."""
    bf = ml_dtypes.bfloat16
    x = np.asarray(x, dtype=np.float32)
    w_qkv = np.asarray(w_qkv, dtype=np.float32)
    w_out = np.asarray(w_out, dtype=np.float32)
    mask = np.asarray(mask)

    NLs = [int(mask[b].sum()) for b in range(B)]
    NLP = int(np.ceil((max(NLs) + 1) / 8) * 8)
    NTL = (NLP + P - 1) // P

    w3 = w_qkv.reshape(DM, H, 3, DH)
    in_maps = []
    for c in range(NCORES):
        b, hg = c // HG, c % HG
        nl = NLs[b]
        live = np.nonzero(mask[b])[0]
        dead = np.nonzero(mask[b] == 0)[0]

        wq = w3[:, hg * HL : (hg + 1) * HL, 0, :].reshape(DM, FV) * SCALE
        wk = w3[:, hg * HL : (hg + 1) * HL, 1, :].reshape(DM, FV)
        # pair-major column layout: [q_p0 | k_p0 | q_p1 | k_p1 | ...]
        wqk_c = np.empty((DM, FQK), np.float32)
        for p in range(NP):
            wqk_c[:, p * 256 : p * 256 + 128] = wq[:, p * 128 : (p + 1) * 128]
            wqk_c[:, p * 256 + 128 : (p + 1) * 256] = wk[:, p * 128 : (p + 1) * 128]
        wv_c = np.ascontiguousarray(
            w3[:, hg * HL : (hg + 1) * HL, 2, :].reshape(DM, FV)
        )
        wout_c = np.ascontiguousarray(w_out[hg * FV : (hg + 1) * FV, :])

        xTl_c = np.zeros((DM, NLP), np.float32)
        xTl_c[:, :nl] = x[b].T[:, live]

        padrow_c = np.zeros((1, NLP), np.float32)
        padrow_c[0, nl:] = 1.0
        lind_c = np.zeros(NTL * P, np.float32)
        lind_c[:nl] = 1.0
        lind_c = np.ascontiguousarray(lind_c.reshape(NTL, P).T)

        # dvec: per head [sum_dead v_h | N_dead]
        xs = x[b][dead].sum(axis=0) if len(dead) else np.zeros(DM, np.float32)
        dv = (xs @ wv_c).reshape(HL, DH)
        dvec_c = np.empty((1, VROW), np.float32)
        for h in range(HL):
            dvec_c[0, h * VW : h * VW + DH] = dv[h]
            dvec_c[0, h * VW + DH] = float(len(dead))

        in_maps.append(
            {
                "xTl": xTl_c.astype(bf),
                "wqk": wqk_c.astype(bf),
                "wv": wv_c.astype(bf),
                "wout": wout_c.astype(bf),
                "padrow": padrow_c.astype(bf),
                "lind": lind_c.astype(np.float32),
                "dvec": dvec_c.astype(bf),
            }
        )
    return in_maps, NLP, NLs


def kernel(x, w_qkv, w_out, b_out, mask):
    from concourse.bass_utils import run_bass_kernel_spmd

    in_maps, NLP, NLs = _shard_inputs(x, w_qkv, w_out, b_out, mask)
    if NLP not in _CACHE:
        _CACHE[NLP] = _build_program(NLP)
    nc = _CACHE[NLP]

    res = run_bass_kernel_spmd(nc, in_maps, list(range(NCORES))).results

    mask = np.asarray(mask)
    b_out = np.asarray(b_out, dtype=np.float32)
    out = np.empty((B, N, DM), np.float32)
    for b in range(B):
        nl = NLs[b]
        live = np.nonzero(mask[b])[0]
        dead = np.nonzero(mask[b] == 0)[0]
        part = (
            res[HG * b]["outA"]
            + res[HG * b]["outB"].astype(np.float32)
            + res[HG * b + 1]["outA"]
            + res[HG * b + 1]["outB"].astype(np.float32)
        )
        out[b, live] = part[:nl]
        if len(dead):
            out[b, dead] = part[nl]
        out[b] += b_out[None, :]
    return out


# revision 112
# speedup vs baseline: 1.0091x; 1.0006x over previous
"""Multi-head attention (B=4, N=2048, DM=1024, H=16, DH=64) on 8 trn2 cores.

Sharding: core c -> (batch b = c//2, head-group hg = c%2 of 8 heads).

Live-token compaction: the pair mask only keeps (i,j) score pairs where
both tokens are live, and every dead query row of the reference output is
the SAME uniform average over all value tokens.  So the host gathers the
~NL live tokens of each batch into a compact [DM, NLP] x^T (NLP = padded
live count, multiple of 128, >= NL+1), the device runs attention on live
tokens only, and the host scatters rows back, filling dead rows with the
row produced by the first zero-padded query column.

Per-column semantics on device (q = x@Wq, k = x@Wk, scaled, no masking):
  - live i, live j: t = exp(q_i.k_j) -- the real softmax numerator.
  - pad i (x=0 -> q=0): t = 1 for all j, plus a rank-1 correction row
    (+padrow_i * dvec, dvec = [sum_dead v | N_dead] from the host) makes
    pv = [sum_all v | N], i.e. exactly the reference's uniform dead row.
  - pad j (x=0 -> k=v=0): t = exp(0) = 1 but vplus rows are zero (the
    denominator ones-column is L=live-indicator, not constant 1), so pads
    contribute nothing.

Device layout mirrors the dense kernel: feature-major q/k ([64,NLP] per
head), token-major v with an appended denominator column, scores
transposed [j,i] so PV needs no transpose, exp on ACT, a small [N,64]
transpose per head for the output projection.

Scheduling: heads run as a software pipeline -- window W(h) emits scores
+exp for head h, PV for head h-1, the batched last-i-tile ("tail") scores
for head h-1 and PV for head h-2, plus deadline-scheduled filler chunks
(later pairs' qk projections, then partial output chunks).  The output
projection is split into two half-contractions written to separate DRAM
tensors (outA = pairs 0-1 in f32 overlapped with the mid loop, outB =
pairs 2-3 in bf16 to halve the drain DMA); the host sums them.
"""

import sys

sys.path.insert(0, "/opt/trn_rl_repo")

import numpy as np
import ml_dtypes

B, N, DM, H, DH = 4, 2048, 1024, 16, 64
SCALE = DH**-0.5
NCORES = 8
HG = 2  # head groups (tensor-parallel factor)
HL = H // HG  # 8 heads per core
NP = HL // 2  # 4 head pairs
FQK = HL * 2 * DH  # 1024 qk features per core
FV = HL * DH  # 512 v features per core
P = 128
DMT = DM // P  # 8 dm tiles
VW = DH + 1  # 65: v columns + denominator column
VROW = HL * VW  # 520
HT = FV // P  # 4 head-dim tiles for the projection

_CACHE = {}


def _build_program(NLP):
    import concourse.mybir as mybir
    import concourse.tile as tile
    from concourse import bacc
    from concourse.masks import make_identity

    bf = mybir.dt.bfloat16
    f32 = mybir.dt.float32
    EXP = mybir.ActivationFunctionType.Exp
    COPY = mybir.ActivationFunctionType.Copy

    NTL = (NLP + P - 1) // P  # live token tiles (last may be partial)
    TW = NLP - (NTL - 1) * P  # width of the last tile
    # i-span structure: one wide main span (software-pipelined head loop),
    # remaining tiles handled in the pipelined tail path (w==1) or a slow
    # generic path (w>1, only for much larger masks).
    spans = []
    off = 0
    while NTL - off > 0:
        w = min(8, NTL - off)
        spans.append((off, w))
        off += w
    off0, w0 = spans[0]
    tail1 = [s for s in spans[1:] if s[1] == 1]
    slow_tails = [s for s in spans[1:] if s[1] > 1]

    def IW(it):
        return TW if it == NTL - 1 else P

    # qk_all block stride: a (P-TW)-col pad after each feature block so the
    # full-width kT reads of the partial last j-tile never touch another
    # block's (possibly unwritten) data
    BS = NLP + (P - TW) if TW < P else NLP
    # qk-projection column chunks (psum-bank sized)
    qk_chunks = [(c0, min(512, NLP - c0)) for c0 in range(0, NLP, 512)]

    nc = bacc.Bacc(
        "TRN2", target_bir_lowering=False, debug=False, num_devices=NCORES
    )
    xTl = nc.dram_tensor("xTl", [DM, NLP], bf, kind="ExternalInput")
    wqk = nc.dram_tensor("wqk", [DM, FQK], bf, kind="ExternalInput")
    wv = nc.dram_tensor("wv", [DM, FV], bf, kind="ExternalInput")
    wout = nc.dram_tensor("wout", [FV, DM], bf, kind="ExternalInput")
    padrow = nc.dram_tensor("padrow", [1, NLP], bf, kind="ExternalInput")
    lind = nc.dram_tensor("lind", [P, NTL], f32, kind="ExternalInput")
    dvec = nc.dram_tensor("dvec", [1, VROW], bf, kind="ExternalInput")
    outA = nc.dram_tensor("outA", [NLP, DM], f32, kind="ExternalOutput")
    outB = nc.dram_tensor("outB", [NLP, DM], bf, kind="ExternalOutput")

    with tile.TileContext(nc) as tc:
        with tc.tile_pool(name="const", bufs=1) as cp:
            xTl_sb = cp.tile([P, DMT * NLP], bf, tag="xTl")
            wqk_sb = cp.tile([P, DMT * FQK], bf, tag="wqk")
            wv_sb = cp.tile([P, DMT * FV], bf, tag="wv")
            wout_sb = cp.tile([P, HT * DM], bf, tag="wout")
            padrow_sb = cp.tile([1, NLP], bf, tag="padrow")
            lind_sb = cp.tile([P, NTL], f32, tag="lind")
            dvec_sb = cp.tile([1, VROW], bf, tag="dvec")
            ident = cp.tile([P, P], bf, tag="ident")
            zeros8 = cp.tile([P, HL], bf, tag="zeros8")
            vplus = cp.tile([P, NTL * VROW], bf, tag="vplus")
            qk_all = cp.tile([P, HL * BS], bf, tag="qkall")
            attT = cp.tile([P, HT * NLP], bf, tag="attT")

            # DMA order mirrors consumption: per-dm-tile x^T plus the
            # pair-0 qk weight columns first (feeds the first projection),
            # then v weights (needed by the head-1 window), small tensors,
            # then the remaining qk weight columns and w_out.
            for dmt in range(DMT):
                nc.sync.dma_start(
                    out=wqk_sb[:, dmt * FQK : dmt * FQK + 3 * P],
                    in_=wqk[dmt * P : (dmt + 1) * P, 0 : 3 * P],
                )
                nc.sync.dma_start(
                    out=xTl_sb[:, dmt * NLP : (dmt + 1) * NLP],
                    in_=xTl[dmt * P : (dmt + 1) * P, :],
                )
            nc.sync.dma_start(
                out=wv_sb[:, :].rearrange("p (d f) -> p d f", d=DMT, f=FV),
                in_=wv[:, :].rearrange("(d p) c -> p d c", p=P),
            )
            nc.sync.dma_start(out=lind_sb[:, :], in_=lind[:, :])
            nc.sync.dma_start(out=padrow_sb[:, :], in_=padrow[:, :])
            nc.sync.dma_start(out=dvec_sb[:, :], in_=dvec[:, :])
            for dmt in range(DMT):
                nc.sync.dma_start(
                    out=wqk_sb[:, dmt * FQK + 3 * P : (dmt + 1) * FQK],
                    in_=wqk[dmt * P : (dmt + 1) * P, 3 * P :],
                )
            for ht in range(HT):
                nc.sync.dma_start(
                    out=wout_sb[:, ht * DM : (ht + 1) * DM],
                    in_=wout[ht * P : (ht + 1) * P, :],
                )
            make_identity(nc, ident)
            nc.gpsimd.memset(zeros8[:, :], 0.0)
            # zero the pad margin after each feature block (spill target of
            # the full-width kT reads of the partial last j-tile)
            if TW < P:
                for f in range(HL):
                    nc.gpsimd.memset(qk_all[:, f * BS + NLP : (f + 1) * BS], 0.0)
            if TW < P:
                # rows of the partial last j-tile beyond the live+pad range
                # never get v written; zero the whole block up front (the v
                # eviction later overwrites rows [0:TW]) so spilled-garbage
                # exp rows contract against zeros
                nc.gpsimd.memset(vplus[:, (NTL - 1) * VROW : NTL * VROW], 0.0)

            vp4 = vplus.rearrange("p (t g c) -> p t g c", t=NTL, g=HL, c=VW)

            with (
                tc.tile_pool(name="psqk", bufs=2, space="PSUM") as pqk,
                tc.tile_pool(name="pss", bufs=2, space="PSUM") as pss,
                tc.tile_pool(name="psa", bufs=1, space="PSUM") as psa,
                tc.tile_pool(name="tpool", bufs=20) as tp,
                tc.tile_pool(name="ttpool", bufs=3) as ttp,
                tc.tile_pool(name="appool", bufs=2) as app,
                tc.tile_pool(name="spool", bufs=6) as sp,
            ):
                PSW = min(w0 * P, 1024)

                def emit_qk_chunk(f, c0, cw):
                    ps = pqk.tile([P, 512], f32, tag="qk", name="ps_qk")
                    for dmt in range(DMT):
                        nc.tensor.matmul(
                            ps[:, :cw],
                            wqk_sb[:, dmt * FQK + f * P : dmt * FQK + (f + 1) * P],
                            xTl_sb[:, dmt * NLP + c0 : dmt * NLP + c0 + cw],
                            start=(dmt == 0),
                            stop=(dmt == DMT - 1),
                        )
                    nc.vector.tensor_copy(
                        qk_all[:, f * BS + c0 : f * BS + c0 + cw], ps[:, :cw]
                    )

                def emit_v(tt):
                    W = IW(tt)
                    ps = pqk.tile([P, FV], f32, tag="qk", name="ps_v")
                    for dmt in range(DMT):
                        nc.tensor.matmul(
                            ps[0:W, :],
                            xTl_sb[:, dmt * NLP + tt * P : dmt * NLP + tt * P + W],
                            wv_sb[:, dmt * FV : (dmt + 1) * FV],
                            start=(dmt == 0),
                            stop=(dmt == DMT - 1),
                        )
                    nc.vector.tensor_copy(
                        vp4[0:W, tt, :, 0:DH],
                        ps[0:W].rearrange("p (g c) -> p g c", g=HL, c=DH),
                    )
                    # denominator column = live indicator (0 for pad rows)
                    nc.vector.tensor_scalar_add(
                        vp4[0:W, tt, :, DH],
                        zeros8[0:W, :],
                        lind_sb[0:W, tt : tt + 1],
                    )

                def sc_wide(h, off, w, jt):
                    pair, hh = h // 2, h % 2
                    p0 = hh * 64
                    qc = (2 * pair) * BS + off * P
                    kc = (2 * pair + 1) * BS
                    ps_s = pss.tile([P, PSW], f32, tag="s", name="ps_s")
                    for c0 in range(0, w * P, 512):
                        cw = min(512, w * P - c0)
                        nc.tensor.matmul(
                            ps_s[:, c0 : c0 + cw],
                            qk_all[p0 : p0 + 64, kc + jt * P : kc + (jt + 1) * P],
                            qk_all[p0 : p0 + 64, qc + c0 : qc + c0 + cw],
                            start=True,
                            stop=True,
                        )
                    t_sb = tp.tile([P, PSW], bf, tag="t", name="t_sb")
                    nc.scalar.activation(t_sb[:, : w * P], ps_s[:, : w * P], EXP)
                    return t_sb

                def pv_wide(h, w, jt, t_sb, pa):
                    vsl = vplus[:, jt * VROW + h * VW : jt * VROW + (h + 1) * VW]
                    for it in range(w):
                        nc.tensor.matmul(
                            pa[:, it * P : it * P + VW],
                            t_sb[:, it * P : (it + 1) * P],
                            vsl,
                            start=(jt == 0 and it % 4 == 0),
                            stop=False,
                        )

                def corr_wide(h, off, w, pa):
                    for it in range(w):
                        nc.tensor.matmul(
                            pa[:, it * P : it * P + VW],
                            padrow_sb[:, (off + it) * P : (off + it + 1) * P],
                            dvec_sb[:, h * VW : (h + 1) * VW],
                            start=False,
                            stop=(it % 4 == 3 or it == w - 1),
                        )

                def norm_wide(h, w, pa, ap):
                    p0 = (h % 2) * 64
                    r_sb = sp.tile([P, 8], f32, tag="r", name="r_sb")
                    pa3 = pa.rearrange("p (t c) -> p t c", t=PSW // P, c=P)
                    nc.vector.reciprocal(r_sb[:, :w], pa3[:, :w, DH])
                    for it in range(w):
                        nc.vector.tensor_scalar_mul(
                            ap[:, it * P + p0 : it * P + p0 + DH],
                            pa[:, it * P : it * P + DH],
                            r_sb[:, it : it + 1],
                        )

                def pv_slot(h, off, w, it, pa, t_list, ap):
                    # slot-major: finish output tile `it` for head h in one
                    # go (all-j PV + correction + normalize), so downstream
                    # per-tile work pipelines inside the window
                    p0 = (h % 2) * 64
                    vcol = h * VW
                    for jt in range(NTL):
                        nc.tensor.matmul(
                            pa[:, it * P : it * P + VW],
                            t_list[jt][:, it * P : (it + 1) * P],
                            vplus[:, jt * VROW + vcol : jt * VROW + vcol + VW],
                            start=(jt == 0 and it % 4 == 0),
                            stop=False,
                        )
                    nc.tensor.matmul(
                        pa[:, it * P : it * P + VW],
                        padrow_sb[:, (off + it) * P : (off + it + 1) * P],
                        dvec_sb[:, h * VW : (h + 1) * VW],
                        start=False,
                        stop=(it % 4 == 3 or it == w - 1),
                    )
                    r1 = sp.tile([P, 8], f32, tag="r", name="r1")
                    nc.vector.reciprocal(
                        r1[:, 0:1], pa[:, it * P + DH : it * P + DH + 1]
                    )
                    nc.vector.tensor_scalar_mul(
                        ap[:, it * P + p0 : it * P + p0 + DH],
                        pa[:, it * P : it * P + DH],
                        r1[:, 0:1],
                    )

                def transpose_it(pair, off, it, ap):
                    ps_tr = pqk.tile([P, P], bf, tag="qk", name="ps_tr")
                    nc.tensor.transpose(
                        ps_tr[:, :], ap[:, it * P : (it + 1) * P], ident
                    )
                    dst = attT[
                        :,
                        pair * NLP + (off + it) * P : pair * NLP
                        + (off + it + 1) * P,
                    ]
                    nc.vector.tensor_copy(dst, ps_tr[:, :])

                def transpose_pair(pair, off, w, ap):
                    # batch 4 transposes per 1-bank slot -> one eviction copy
                    it = 0
                    while it < w:
                        nb = min(4, w - it)
                        if IW(off + it + nb - 1) != P:
                            nb = 1
                        ps_tr = pqk.tile([P, 512], bf, tag="qk", name="ps_tr")
                        wtot = 0
                        for k in range(nb):
                            W = IW(off + it + k)
                            nc.tensor.transpose(
                                ps_tr[:, k * P : k * P + W],
                                ap[0:W, (it + k) * P : (it + k + 1) * P],
                                ident[0:W, 0:W],
                            )
                            wtot = k * P + W
                        nc.vector.tensor_copy(
                            attT[
                                :,
                                pair * NLP + (off + it) * P : pair * NLP
                                + (off + it) * P
                                + wtot,
                            ],
                            ps_tr[:, 0:wtot],
                        )
                        it += nb

                def s_tail(h, off):
                    # batched narrow-tail scores: the TW-wide last i-tile for
                    # all j-tiles, packed into as few psum banks / exp
                    # instructions as possible
                    pair, hh = h // 2, h % 2
                    p0 = hh * 64
                    qc = (2 * pair) * BS + off * P
                    kc = (2 * pair + 1) * BS
                    bpb = max(1, 512 // TW)  # batches per psum bank
                    t_t = ttp.tile([P, NTL * TW], bf, tag="tt", name="t_tail")
                    done = 0
                    while done < NTL:
                        nb = min(2 * bpb, NTL - done)  # one 2-bank slot
                        ps_s = pss.tile([P, PSW], f32, tag="s", name="ps_st")
                        for g in range(nb):
                            jt = done + g
                            pos = (g // bpb) * 512 + (g % bpb) * TW
                            nc.tensor.matmul(
                                ps_s[:, pos : pos + TW],
                                qk_all[p0 : p0 + 64, kc + jt * P : kc + jt * P + P],
                                qk_all[p0 : p0 + 64, qc : qc + TW],
                                start=True,
                                stop=True,
                            )
                        for bk in range((nb + bpb - 1) // bpb):
                            cnt = min(bpb, nb - bk * bpb)
                            nc.scalar.activation(
                                t_t[
                                    :,
                                    (done + bk * bpb) * TW : (done + bk * bpb + cnt)
                                    * TW,
                                ],
                                ps_s[:, bk * 512 : bk * 512 + cnt * TW],
                                EXP,
                            )
                        done += nb
                    return t_t

                def p_tail(h, off, t_t, ap):
                    # 65-col accumulator lives in a 1-bank "qk" slot so it
                    # never waits on the main PV accumulator (deadlock risk)
                    p0 = (h % 2) * 64
                    pa = pqk.tile([P, 512], f32, tag="qk", name="pa_t")
                    for jt in range(NTL):
                        nc.tensor.matmul(
                            pa[0:TW, 0:VW],
                            t_t[:, jt * TW : (jt + 1) * TW],
                            vplus[:, jt * VROW + h * VW : jt * VROW + (h + 1) * VW],
                            start=(jt == 0),
                            stop=False,
                        )
                    nc.tensor.matmul(
                        pa[0:TW, 0:VW],
                        padrow_sb[:, off * P : off * P + TW],
                        dvec_sb[:, h * VW : (h + 1) * VW],
                        start=False,
                        stop=True,
                    )
                    r_sb = sp.tile([P, 8], f32, tag="r", name="r_t")
                    nc.vector.reciprocal(r_sb[0:TW, 0:1], pa[0:TW, DH : DH + 1])
                    nc.vector.tensor_scalar_mul(
                        ap[0:TW, p0 : p0 + DH], pa[0:TW, 0:DH], r_sb[0:TW, 0:1]
                    )

                nout = [0]
                nbig = [0]

                def emit_half(it, ch, half):
                    # half 0: pairs 0-1 -> outA f32; half 1: pairs 2-3 -> outB bf16
                    W = IW(it)
                    ps_o = pqk.tile([P, 512], f32, tag="qk", name="ps_o")
                    for ht in (0, 1) if half == 0 else (2, 3):
                        nc.tensor.matmul(
                            ps_o[0:W, :],
                            attT[:, ht * NLP + it * P : ht * NLP + it * P + W],
                            wout_sb[:, ht * DM + ch * 512 : ht * DM + (ch + 1) * 512],
                            start=(ht % 2 == 0),
                            stop=(ht % 2 == 1),
                        )
                    dt = f32 if half == 0 else bf
                    o_sb = sp.tile([P, 512], dt, tag="obA" if half == 0 else "obB",
                                   name="o_sb")
                    # A-half evictions stay off ACT (it paces mid-loop exps);
                    # B-half runs in the drain where ACT is idle.
                    if half == 1:
                        nc.scalar.activation(o_sb[0:W, :], ps_o[0:W, :], COPY)
                    else:
                        nc.vector.tensor_copy(o_sb[0:W, :], ps_o[0:W, :])
                    nout[0] += 1
                    dst = outA if half == 0 else outB
                    nc.sync.dma_start(
                        out=dst[it * P : it * P + W, ch * 512 : (ch + 1) * 512],
                        in_=o_sb[0:W, :],
                    )

                # ---- filler queues ----
                # qk chunks for pairs 1..3: pair p before head 2p's scores.
                fast_start = len(qk_chunks) == 3
                fill_units = [
                    (f, c0, cw)
                    for pair in range(1, NP)
                    for f in (2 * pair, 2 * pair + 1)
                    for (c0, cw) in qk_chunks
                    if not (fast_start and f == 2 and c0 < qk_chunks[2][0])
                ]
                n_units = len(fill_units)
                cpp = 2 * len(qk_chunks)
                fill_pos = [0]
                cpair1 = cpp - (2 if fast_start else 0)

                def emit_fill_to(tgt):
                    k = fill_pos[0]
                    for u in fill_units[k : min(n_units, tgt)]:
                        emit_qk_chunk(*u)
                    fill_pos[0] = max(k, min(n_units, tgt))

                def needed_before(h):
                    p = max(0, h // 2)
                    return min(n_units, cpair1 if p == 1 else
                               cpair1 + (p - 1) * cpp if p > 1 else 0)

                # A half-chunks (pairs 0-1): ready once pair-1 main+tail
                # transposes are done (end of window 5); fill windows 6-7.
                a_units = [
                    (it, ch) for it in range(off0, off0 + w0) for ch in range(2)
                ]
                nA = len(a_units)
                a_pos = [0]

                def emit_a_to(tgt):
                    k = a_pos[0]
                    for u in a_units[k : min(nA, tgt)]:
                        emit_half(u[0], u[1], 0)
                    a_pos[0] = max(k, min(nA, tgt))

                # ---- window 0: pair-0 projections + head-0 scores,
                #      pair-1 qk chunks interleaved ----
                if len(qk_chunks) == 3:
                    # dmt-outer interleave across the f0/f1 chunks plus
                    # pair-1's first q chunks keeps the PE fed at DMA
                    # arrival pace (chunk-slots borrowed from the idle
                    # pss/psa pools + pqk)
                    sA = pss.tile([P, PSW], f32, tag="s", name="ps_q0")
                    sB = pss.tile([P, PSW], f32, tag="s", name="ps_q1")
                    qA = pqk.tile([P, 512], f32, tag="qk", name="ps_q2")
                    qB = pqk.tile([P, 512], f32, tag="qk", name="ps_q3")
                    aA = psa.tile([P, PSW], f32, tag="att", name="ps_q4")
                    units = [
                        (0, qk_chunks[0][0], qk_chunks[0][1], sA, 0),
                        (0, qk_chunks[1][0], qk_chunks[1][1], sA, 512),
                        (0, qk_chunks[2][0], qk_chunks[2][1], sB, 0),
                        (1, qk_chunks[0][0], qk_chunks[0][1], sB, 512),
                        (1, qk_chunks[1][0], qk_chunks[1][1], qA, 0),
                        (1, qk_chunks[2][0], qk_chunks[2][1], qB, 0),
                        (2, qk_chunks[0][0], qk_chunks[0][1], aA, 0),
                        (2, qk_chunks[1][0], qk_chunks[1][1], aA, 512),
                    ]
                    for dmt in range(DMT):
                        for f, c0, cw, ps, so in units:
                            nc.tensor.matmul(
                                ps[:, so : so + cw],
                                wqk_sb[
                                    :, dmt * FQK + f * P : dmt * FQK + (f + 1) * P
                                ],
                                xTl_sb[:, dmt * NLP + c0 : dmt * NLP + c0 + cw],
                                start=(dmt == 0),
                                stop=(dmt == DMT - 1),
                            )
                    for f, c0, cw, ps, so in units:
                        nc.vector.tensor_copy(
                            qk_all[:, f * BS + c0 : f * BS + c0 + cw],
                            ps[:, so : so + cw],
                        )
                else:
                    for f in (0, 1):
                        for c0, cw in qk_chunks:
                            emit_qk_chunk(f, c0, cw)
                t_store = {0: []}
                tt_store = {}
                for jt in range(NTL):
                    t_store[0].append(sc_wide(0, off0, w0, jt))
                if tail1:
                    # pair-0 tail scores cover the wv DMA wait; their spill
                    # reads (pair-1 q block) are written by the fast startup
                    tt_store[0] = s_tail(0, tail1[0][0])
                    tt_store[1] = s_tail(1, tail1[0][0])
                for tt in range(min(3, NTL)):
                    emit_v(tt)
                emit_fill_to(cpair1)

                # ---- windows 1..7: S(h) || PV(h-1) || tail(h-1 scores,
                #      h-2 PV) || fillers ----
                ap_cur = None
                apt_cur = None
                for h in range(1, HL):
                    emit_fill_to(needed_before(h))
                    fprev = fill_pos[0]
                    fth = n_units if h >= 5 else max(
                        needed_before(h + 1), (n_units * h * 2 + 8) // 9
                    )
                    aprev = a_pos[0]
                    ath = {5: 5, 6: 13}.get(h, 0 if h < 5 else nA)
                    t_store[h] = []
                    pa = psa.tile([P, PSW], f32, tag="att", name="pa")
                    for jt in range(NTL):
                        t_store[h].append(sc_wide(h, off0, w0, jt))
                        if h == 1 and jt >= 3:
                            emit_v(jt)
                        emit_a_to(aprev + ((ath - aprev) * (jt + 1)) // NTL)
                        pv_wide(h - 1, w0, jt, t_store[h - 1][jt], pa)
                        emit_fill_to(fprev + ((fth - fprev) * (jt + 1)) // NTL)
                        t_store[h - 1][jt] = None
                        if jt == 2 and h >= 2 and tail1 and (h - 2) in tt_store:
                            # lag-2 tail PV for head h-2
                            offt = tail1[0][0]
                            if (h - 2) % 2 == 0:
                                apt_cur = app.tile([P, P], bf, tag="apt", name="apt")
                            p_tail(h - 2, offt, tt_store.pop(h - 2), apt_cur)
                            if (h - 2) % 2 == 1:
                                transpose_pair((h - 2) // 2, offt, 1, apt_cur)
                    if tail1 and h >= 3:
                        tt_store[h - 1] = s_tail(h - 1, tail1[0][0])
                    if tail1 and h == HL - 1:
                        tt_store[h] = s_tail(h, tail1[0][0])
                    corr_wide(h - 1, off0, w0, pa)
                    if (h - 1) % 2 == 0:
                        ap_cur = app.tile([P, PSW], bf, tag="ap", name="ap")
                    norm_wide(h - 1, w0, pa, ap_cur)
                    if (h - 1) % 2 == 1:
                        transpose_pair((h - 1) // 2, off0, w0, ap_cur)

                # ---- epilogue: PV(7), tails 6-7, pipelined pair-3 finish ----
                def emit_outB_big(it, split=False):
                    # whole-row B chunk: pairs 2-3 for both DM halves in one
                    # 2-bank pss slot, one eviction, one outB DMA
                    W = IW(it)
                    ps_o = pss.tile([P, PSW], f32, tag="s", name="ps_b")
                    for ch in range(2):
                        for ht in (2, 3):
                            nc.tensor.matmul(
                                ps_o[0:W, ch * 512 : (ch + 1) * 512],
                                attT[:, ht * NLP + it * P : ht * NLP + it * P + W],
                                wout_sb[
                                    :, ht * DM + ch * 512 : ht * DM + (ch + 1) * 512
                                ],
                                start=(ht == 2),
                                stop=(ht == 3),
                            )
                    o_sb = sp.tile([P, 1024], bf, tag="obB", name="o_sbB")
                    nbig[0] += 1
                    if split:
                        # parallel eviction halves (ACT + DVE): shallow drain
                        nc.scalar.activation(
                            o_sb[0:W, 0:512], ps_o[0:W, 0:512], COPY
                        )
                        nc.vector.tensor_copy(
                            o_sb[0:W, 512:1024], ps_o[0:W, 512:1024]
                        )
                        nc.sync.dma_start(
                            out=outB[it * P : it * P + W, 0:512],
                            in_=o_sb[0:W, 0:512],
                        )
                        nc.sync.dma_start(
                            out=outB[it * P : it * P + W, 512:1024],
                            in_=o_sb[0:W, 512:1024],
                        )
                        return
                    if nbig[0] % 2 == 1:
                        nc.scalar.activation(o_sb[0:W, :], ps_o[0:W, :1024], COPY)
                    else:
                        nc.vector.tensor_copy(o_sb[0:W, :], ps_o[0:W, :1024])
                    nc.sync.dma_start(
                        out=outB[it * P : it * P + W, :], in_=o_sb[0:W, :]
                    )

                if tail1 and (HL - 1) not in tt_store:
                    tt_store[HL - 1] = s_tail(HL - 1, tail1[0][0])
                emit_fill_to(n_units)
                aprev = a_pos[0]
                pa = psa.tile([P, PSW], f32, tag="att", name="pa")
                for jt in range(NTL):
                    emit_a_to(aprev + ((nA - aprev) * (jt + 1)) // NTL)
                    pv_wide(HL - 1, w0, jt, t_store[HL - 1][jt], pa)
                    if jt == 2 and tail1 and (HL - 2) in tt_store:
                        offt = tail1[0][0]
                        apt_cur = app.tile([P, P], bf, tag="apt", name="apt")
                        p_tail(HL - 2, offt, tt_store.pop(HL - 2), apt_cur)
                    if jt == 4 and tail1 and (HL - 1) in tt_store:
                        offt = tail1[0][0]
                        p_tail(HL - 1, offt, tt_store.pop(HL - 1), apt_cur)
                        transpose_pair(NP - 1, offt, 1, apt_cur)
                    if jt == 0 and tail1:
                        # tail i-tile outA rows (pairs 0-1 only): PE work to
                        # cover the window-boundary normalize wait
                        offt = tail1[0][0]
                        emit_half(offt, 0, 0)
                        emit_half(offt, 1, 0)

                corr_wide(HL - 1, off0, w0, pa)
                emit_a_to(nA)
                # pipelined pair-3 finish: normalize slot -> transpose ->
                # previous tile's whole-row B chunk (hides eviction latency)
                p0e = ((HL - 1) % 2) * 64
                r_sb = sp.tile([P, 8], f32, tag="r", name="r_e")
                pa3 = pa.rearrange("p (t c) -> p t c", t=PSW // P, c=P)
                nc.vector.reciprocal(r_sb[:, :w0], pa3[:, :w0, DH])
                for it in range(w0):
                    nc.vector.tensor_scalar_mul(
                        ap_cur[:, it * P + p0e : it * P + p0e + DH],
                        pa[:, it * P : it * P + DH],
                        r_sb[:, it : it + 1],
                    )
                    ps_tr = pqk.tile([P, P], bf, tag="qk", name="ps_tr")
                    nc.tensor.transpose(
                        ps_tr[:, :], ap_cur[:, it * P : (it + 1) * P], ident
                    )
                    nc.vector.tensor_copy(
                        attT[
                            :,
                            (NP - 1) * NLP + (off0 + it) * P : (NP - 1) * NLP
                            + (off0 + it + 1) * P,
                        ],
                        ps_tr[:, :],
                    )
                    if it >= 1:
                        emit_outB_big(off0 + it - 1, split=(it == w0 - 1))
                emit_outB_big(off0 + w0 - 1, split=True)
                if tail1:
                    # tail i-tile outB last: its 24-row eviction/DMAs drain
                    # far faster than a full 128-row tile's
                    emit_outB_big(tail1[0][0], split=True)

                # ---- slow generic path for wide tail spans (NTL > 9) ----
                for offt, wt in slow_tails:
                    for pair in range(NP):
                        for hh in range(2):
                            h = 2 * pair + hh
                            pa = psa.tile([P, PSW], f32, tag="att", name="pa_w")
                            tw = [sc_wide(h, offt, wt, jt) for jt in range(NTL)]
                            for jt in range(NTL):
                                pv_wide(h, wt, jt, tw[jt], pa)
                            corr_wide(h, offt, wt, pa)
                            if hh == 0:
                                ap_t = app.tile([P, PSW], bf, tag="ap", name="ap_w")
                            norm_wide(h, wt, pa, ap_t)
                        transpose_pair(pair, offt, wt, ap_t)
                    for it in range(offt, offt + wt):
                        for ch in range(2):
                            emit_half(it, ch, 0)

                # B halves for slow-tail tiles (already emitted for the rest)
                for offt, wt in slow_tails:
                    for it in range(offt, offt + wt):
                        for ch in range(2):
                            emit_half(it, ch, 1)

    nc.compile()
    return nc


def _shard_inputs(x, w_qkv, w_out, b_out, mask):
    """Host-side live-token gather + per-core layout prep."""
    bf = ml_dtypes.bfloat16
    x = np.asarray(x, dtype=np.float32)
    w_qkv = np.asarray(w_qkv, dtype=np.float32)
    w_out = np.asarray(w_out, dtype=np.float32)
    mask = np.asarray(mask)

    NLs = [int(mask[b].sum()) for b in range(B)]
    NLP = int(np.ceil((max(NLs) + 1) / 8) * 8)
    NTL = (NLP + P - 1) // P

    w3 = w_qkv.reshape(DM, H, 3, DH)
    in_maps = []
    for c in range(NCORES):
        b, hg = c // HG, c % HG
        nl = NLs[b]
        live = np.nonzero(mask[b])[0]
        dead = np.nonzero(mask[b] == 0)[0]

        wq = w3[:, hg * HL : (hg + 1) * HL, 0, :].reshape(DM, FV) * SCALE
        wk = w3[:, hg * HL : (hg + 1) * HL, 1, :].reshape(DM, FV)
        # pair-major column layout: [q_p0 | k_p0 | q_p1 | k_p1 | ...]
        wqk_c = np.empty((DM, FQK), np.float32)
        for p in range(NP):
            wqk_c[:, p * 256 : p * 256 + 128] = wq[:, p * 128 : (p + 1) * 128]
            wqk_c[:, p * 256 + 128 : (p + 1) * 256] = wk[:, p * 128 : (p + 1) * 128]
        wv_c = np.ascontiguousarray(
            w3[:, hg * HL : (hg + 1) * HL, 2, :].reshape(DM, FV)
        )
        wout_c = np.ascontiguousarray(w_out[hg * FV : (hg + 1) * FV, :])

        xTl_c = np.zeros((DM, NLP), np.float32)
        xTl_c[:, :nl] = x[b].T[:, live]

        padrow_c = np.zeros((1, NLP), np.float32)
        padrow_c[0, nl:] = 1.0
        lind_c = np.zeros(NTL * P, np.float32)
        lind_c[:nl] = 1.0
        lind_c = np.ascontiguousarray(lind_c.reshape(NTL, P).T)

        # dvec: per head [sum_dead v_h | N_dead]
        xs = x[b][dead].sum(axis=0) if len(dead) else np.zeros(DM, np.float32)
        dv = (xs @ wv_c).reshape(HL, DH)
        dvec_c = np.empty((1, VROW), np.float32)
        for h in range(HL):
            dvec_c[0, h * VW : h * VW + DH] = dv[h]
            dvec_c[0, h * VW + DH] = float(len(dead))

        in_maps.append(
            {
                "xTl": xTl_c.astype(bf),
                "wqk": wqk_c.astype(bf),
                "wv": wv_c.astype(bf),
                "wout": wout_c.astype(bf),
                "padrow": padrow_c.astype(bf),
                "lind": lind_c.astype(np.float32),
                "dvec": dvec_c.astype(bf),
            }
        )
    return in_maps, NLP, NLs


def kernel(x, w_qkv, w_out, b_out, mask):
    from concourse.bass_utils import run_bass_kernel_spmd

    in_maps, NLP, NLs = _shard_inputs(x, w_qkv, w_out, b_out, mask)
    if NLP not in _CACHE:
        _CACHE[NLP] = _build_program(NLP)
    nc = _CACHE[NLP]

    res = run_bass_kernel_spmd(nc, in_maps, list(range(NCORES))).results

    mask = np.asarray(mask)
    b_out = np.asarray(b_out, dtype=np.float32)
    out = np.empty((B, N, DM), np.float32)
    for b in range(B):
        nl = NLs[b]
        live = np.nonzero(mask[b])[0]
        dead = np.nonzero(mask[b] == 0)[0]
        part = (
            res[HG * b]["outA"]
            + res[HG * b]["outB"].astype(np.float32)
            + res[HG * b + 1]["outA"]
            + res[HG * b + 1]["outB"].astype(np.float32)
        )
        out[b, live] = part[:nl]
        if len(dead):
            out[b, dead] = part[nl]
        out[b] += b_out[None, :]
    return out


# revision 113
# speedup vs baseline: 1.0095x; 1.0003x over previous
"""Multi-head attention (B=4, N=2048, DM=1024, H=16, DH=64) on 8 trn2 cores.

Sharding: core c -> (batch b = c//2, head-group hg = c%2 of 8 heads).

Live-token compaction: the pair mask only keeps (i,j) score pairs where
both tokens are live, and every dead query row of the reference output is
the SAME uniform average over all value tokens.  So the host gathers the
~NL live tokens of each batch into a compact [DM, NLP] x^T (NLP = padded
live count, multiple of 128, >= NL+1), the device runs attention on live
tokens only, and the host scatters rows back, filling dead rows with the
row produced by the first zero-padded query column.

Per-column semantics on device (q = x@Wq, k = x@Wk, scaled, no masking):
  - live i, live j: t = exp(q_i.k_j) -- the real softmax numerator.
  - pad i (x=0 -> q=0): t = 1 for all j, plus a rank-1 correction row
    (+padrow_i * dvec, dvec = [sum_dead v | N_dead] from the host) makes
    pv = [sum_all v | N], i.e. exactly the reference's uniform dead row.
  - pad j (x=0 -> k=v=0): t = exp(0) = 1 but vplus rows are zero (the
    denominator ones-column is L=live-indicator, not constant 1), so pads
    contribute nothing.

Device layout mirrors the dense kernel: feature-major q/k ([64,NLP] per
head), token-major v with an appended denominator column, scores
transposed [j,i] so PV needs no transpose, exp on ACT, a small [N,64]
transpose per head for the output projection.

Scheduling: heads run as a software pipeline -- window W(h) emits scores
+exp for head h, PV for head h-1, the batched last-i-tile ("tail") scores
for head h-1 and PV for head h-2, plus deadline-scheduled filler chunks
(later pairs' qk projections, then partial output chunks).  The output
projection is split into two half-contractions written to separate DRAM
tensors (outA = pairs 0-1 in f32 overlapped with the mid loop, outB =
pairs 2-3 in bf16 to halve the drain DMA); the host sums them.
"""

import sys

sys.path.insert(0, "/opt/trn_rl_repo")

import numpy as np
import ml_dtypes

B, N, DM, H, DH = 4, 2048, 1024, 16, 64
SCALE = DH**-0.5
NCORES = 8
HG = 2  # head groups (tensor-parallel factor)
HL = H // HG  # 8 heads per core
NP = HL // 2  # 4 head pairs
FQK = HL * 2 * DH  # 1024 qk features per core
FV = HL * DH  # 512 v features per core
P = 128
DMT = DM // P  # 8 dm tiles
VW = DH + 1  # 65: v columns + denominator column
VROW = HL * VW  # 520
HT = FV // P  # 4 head-dim tiles for the projection

_CACHE = {}


def _build_program(NLP):
    import concourse.mybir as mybir
    import concourse.tile as tile
    from concourse import bacc
    from concourse.masks import make_identity

    bf = mybir.dt.bfloat16
    f32 = mybir.dt.float32
    EXP = mybir.ActivationFunctionType.Exp
    COPY = mybir.ActivationFunctionType.Copy

    NTL = (NLP + P - 1) // P  # live token tiles (last may be partial)
    TW = NLP - (NTL - 1) * P  # width of the last tile
    # i-span structure: one wide main span (software-pipelined head loop),
    # remaining tiles handled in the pipelined tail path (w==1) or a slow
    # generic path (w>1, only for much larger masks).
    spans = []
    off = 0
    while NTL - off > 0:
        w = min(8, NTL - off)
        spans.append((off, w))
        off += w
    off0, w0 = spans[0]
    tail1 = [s for s in spans[1:] if s[1] == 1]
    slow_tails = [s for s in spans[1:] if s[1] > 1]

    def IW(it):
        return TW if it == NTL - 1 else P

    # qk_all block stride: a (P-TW)-col pad after each feature block so the
    # full-width kT reads of the partial last j-tile never touch another
    # block's (possibly unwritten) data
    BS = NLP + (P - TW) if TW < P else NLP
    # qk-projection column chunks (psum-bank sized)
    qk_chunks = [(c0, min(512, NLP - c0)) for c0 in range(0, NLP, 512)]

    nc = bacc.Bacc(
        "TRN2", target_bir_lowering=False, debug=False, num_devices=NCORES
    )
    xTl = nc.dram_tensor("xTl", [DM, NLP], bf, kind="ExternalInput")
    wqk = nc.dram_tensor("wqk", [DM, FQK], bf, kind="ExternalInput")
    wv = nc.dram_tensor("wv", [DM, FV], bf, kind="ExternalInput")
    wout = nc.dram_tensor("wout", [FV, DM], bf, kind="ExternalInput")
    padrow = nc.dram_tensor("padrow", [1, NLP], bf, kind="ExternalInput")
    lind = nc.dram_tensor("lind", [P, NTL], f32, kind="ExternalInput")
    dvec = nc.dram_tensor("dvec", [1, VROW], bf, kind="ExternalInput")
    outA = nc.dram_tensor("outA", [NLP, DM], f32, kind="ExternalOutput")
    outB = nc.dram_tensor("outB", [NLP, DM], bf, kind="ExternalOutput")

    with tile.TileContext(nc) as tc:
        with tc.tile_pool(name="const", bufs=1) as cp:
            xTl_sb = cp.tile([P, DMT * NLP], bf, tag="xTl")
            wqk_sb = cp.tile([P, DMT * FQK], bf, tag="wqk")
            wv_sb = cp.tile([P, DMT * FV], bf, tag="wv")
            wout_sb = cp.tile([P, HT * DM], bf, tag="wout")
            padrow_sb = cp.tile([1, NLP], bf, tag="padrow")
            lind_sb = cp.tile([P, NTL], f32, tag="lind")
            dvec_sb = cp.tile([1, VROW], bf, tag="dvec")
            ident = cp.tile([P, P], bf, tag="ident")
            zeros8 = cp.tile([P, HL], bf, tag="zeros8")
            vplus = cp.tile([P, NTL * VROW], bf, tag="vplus")
            qk_all = cp.tile([P, HL * BS], bf, tag="qkall")
            attT = cp.tile([P, HT * NLP], bf, tag="attT")

            # DMA order mirrors consumption: per-dm-tile x^T plus the
            # pair-0 qk weight columns first (feeds the first projection),
            # then v weights (needed by the head-1 window), small tensors,
            # then the remaining qk weight columns and w_out.
            for dmt in range(DMT):
                nc.sync.dma_start(
                    out=wqk_sb[:, dmt * FQK : dmt * FQK + 3 * P],
                    in_=wqk[dmt * P : (dmt + 1) * P, 0 : 3 * P],
                )
                nc.sync.dma_start(
                    out=xTl_sb[:, dmt * NLP : (dmt + 1) * NLP],
                    in_=xTl[dmt * P : (dmt + 1) * P, :],
                )
            nc.sync.dma_start(
                out=wv_sb[:, :].rearrange("p (d f) -> p d f", d=DMT, f=FV),
                in_=wv[:, :].rearrange("(d p) c -> p d c", p=P),
            )
            nc.sync.dma_start(out=lind_sb[:, :], in_=lind[:, :])
            nc.sync.dma_start(out=padrow_sb[:, :], in_=padrow[:, :])
            nc.sync.dma_start(out=dvec_sb[:, :], in_=dvec[:, :])
            for dmt in range(DMT):
                nc.sync.dma_start(
                    out=wqk_sb[:, dmt * FQK + 3 * P : (dmt + 1) * FQK],
                    in_=wqk[dmt * P : (dmt + 1) * P, 3 * P :],
                )
            for ht in range(HT):
                nc.sync.dma_start(
                    out=wout_sb[:, ht * DM : (ht + 1) * DM],
                    in_=wout[ht * P : (ht + 1) * P, :],
                )
            make_identity(nc, ident)
            nc.gpsimd.memset(zeros8[:, :], 0.0)
            # zero the pad margin after each feature block (spill target of
            # the full-width kT reads of the partial last j-tile)
            if TW < P:
                for f in range(HL):
                    nc.gpsimd.memset(qk_all[:, f * BS + NLP : (f + 1) * BS], 0.0)
            if TW < P:
                # rows of the partial last j-tile beyond the live+pad range
                # never get v written; zero the whole block up front (the v
                # eviction later overwrites rows [0:TW]) so spilled-garbage
                # exp rows contract against zeros
                nc.gpsimd.memset(vplus[:, (NTL - 1) * VROW : NTL * VROW], 0.0)

            vp4 = vplus.rearrange("p (t g c) -> p t g c", t=NTL, g=HL, c=VW)

            with (
                tc.tile_pool(name="psqk", bufs=2, space="PSUM") as pqk,
                tc.tile_pool(name="pss", bufs=2, space="PSUM") as pss,
                tc.tile_pool(name="psa", bufs=1, space="PSUM") as psa,
                tc.tile_pool(name="tpool", bufs=20) as tp,
                tc.tile_pool(name="ttpool", bufs=3) as ttp,
                tc.tile_pool(name="appool", bufs=2) as app,
                tc.tile_pool(name="spool", bufs=8) as sp,
            ):
                PSW = min(w0 * P, 1024)

                def emit_qk_chunk(f, c0, cw):
                    ps = pqk.tile([P, 512], f32, tag="qk", name="ps_qk")
                    for dmt in range(DMT):
                        nc.tensor.matmul(
                            ps[:, :cw],
                            wqk_sb[:, dmt * FQK + f * P : dmt * FQK + (f + 1) * P],
                            xTl_sb[:, dmt * NLP + c0 : dmt * NLP + c0 + cw],
                            start=(dmt == 0),
                            stop=(dmt == DMT - 1),
                        )
                    nc.vector.tensor_copy(
                        qk_all[:, f * BS + c0 : f * BS + c0 + cw], ps[:, :cw]
                    )

                def emit_v(tt):
                    W = IW(tt)
                    ps = pqk.tile([P, FV], f32, tag="qk", name="ps_v")
                    for dmt in range(DMT):
                        nc.tensor.matmul(
                            ps[0:W, :],
                            xTl_sb[:, dmt * NLP + tt * P : dmt * NLP + tt * P + W],
                            wv_sb[:, dmt * FV : (dmt + 1) * FV],
                            start=(dmt == 0),
                            stop=(dmt == DMT - 1),
                        )
                    nc.vector.tensor_copy(
                        vp4[0:W, tt, :, 0:DH],
                        ps[0:W].rearrange("p (g c) -> p g c", g=HL, c=DH),
                    )
                    # denominator column = live indicator (0 for pad rows)
                    nc.vector.tensor_scalar_add(
                        vp4[0:W, tt, :, DH],
                        zeros8[0:W, :],
                        lind_sb[0:W, tt : tt + 1],
                    )

                def sc_wide(h, off, w, jt):
                    pair, hh = h // 2, h % 2
                    p0 = hh * 64
                    qc = (2 * pair) * BS + off * P
                    kc = (2 * pair + 1) * BS
                    ps_s = pss.tile([P, PSW], f32, tag="s", name="ps_s")
                    for c0 in range(0, w * P, 512):
                        cw = min(512, w * P - c0)
                        nc.tensor.matmul(
                            ps_s[:, c0 : c0 + cw],
                            qk_all[p0 : p0 + 64, kc + jt * P : kc + (jt + 1) * P],
                            qk_all[p0 : p0 + 64, qc + c0 : qc + c0 + cw],
                            start=True,
                            stop=True,
                        )
                    t_sb = tp.tile([P, PSW], bf, tag="t", name="t_sb")
                    nc.scalar.activation(t_sb[:, : w * P], ps_s[:, : w * P], EXP)
                    return t_sb

                def pv_wide(h, w, jt, t_sb, pa):
                    vsl = vplus[:, jt * VROW + h * VW : jt * VROW + (h + 1) * VW]
                    for it in range(w):
                        nc.tensor.matmul(
                            pa[:, it * P : it * P + VW],
                            t_sb[:, it * P : (it + 1) * P],
                            vsl,
                            start=(jt == 0 and it % 4 == 0),
                            stop=False,
                        )

                def corr_wide(h, off, w, pa):
                    for it in range(w):
                        nc.tensor.matmul(
                            pa[:, it * P : it * P + VW],
                            padrow_sb[:, (off + it) * P : (off + it + 1) * P],
                            dvec_sb[:, h * VW : (h + 1) * VW],
                            start=False,
                            stop=(it % 4 == 3 or it == w - 1),
                        )

                def norm_wide(h, w, pa, ap):
                    p0 = (h % 2) * 64
                    r_sb = sp.tile([P, 8], f32, tag="r", name="r_sb")
                    pa3 = pa.rearrange("p (t c) -> p t c", t=PSW // P, c=P)
                    nc.vector.reciprocal(r_sb[:, :w], pa3[:, :w, DH])
                    for it in range(w):
                        nc.vector.tensor_scalar_mul(
                            ap[:, it * P + p0 : it * P + p0 + DH],
                            pa[:, it * P : it * P + DH],
                            r_sb[:, it : it + 1],
                        )

                def pv_slot(h, off, w, it, pa, t_list, ap):
                    # slot-major: finish output tile `it` for head h in one
                    # go (all-j PV + correction + normalize), so downstream
                    # per-tile work pipelines inside the window
                    p0 = (h % 2) * 64
                    vcol = h * VW
                    for jt in range(NTL):
                        nc.tensor.matmul(
                            pa[:, it * P : it * P + VW],
                            t_list[jt][:, it * P : (it + 1) * P],
                            vplus[:, jt * VROW + vcol : jt * VROW + vcol + VW],
                            start=(jt == 0 and it % 4 == 0),
                            stop=False,
                        )
                    nc.tensor.matmul(
                        pa[:, it * P : it * P + VW],
                        padrow_sb[:, (off + it) * P : (off + it + 1) * P],
                        dvec_sb[:, h * VW : (h + 1) * VW],
                        start=False,
                        stop=(it % 4 == 3 or it == w - 1),
                    )
                    r1 = sp.tile([P, 8], f32, tag="r", name="r1")
                    nc.vector.reciprocal(
                        r1[:, 0:1], pa[:, it * P + DH : it * P + DH + 1]
                    )
                    nc.vector.tensor_scalar_mul(
                        ap[:, it * P + p0 : it * P + p0 + DH],
                        pa[:, it * P : it * P + DH],
                        r1[:, 0:1],
                    )

                def transpose_it(pair, off, it, ap):
                    ps_tr = pqk.tile([P, P], bf, tag="qk", name="ps_tr")
                    nc.tensor.transpose(
                        ps_tr[:, :], ap[:, it * P : (it + 1) * P], ident
                    )
                    dst = attT[
                        :,
                        pair * NLP + (off + it) * P : pair * NLP
                        + (off + it + 1) * P,
                    ]
                    nc.vector.tensor_copy(dst, ps_tr[:, :])

                def transpose_pair(pair, off, w, ap):
                    # batch 4 transposes per 1-bank slot -> one eviction copy
                    it = 0
                    while it < w:
                        nb = min(4, w - it)
                        if IW(off + it + nb - 1) != P:
                            nb = 1
                        ps_tr = pqk.tile([P, 512], bf, tag="qk", name="ps_tr")
                        wtot = 0
                        for k in range(nb):
                            W = IW(off + it + k)
                            nc.tensor.transpose(
                                ps_tr[:, k * P : k * P + W],
                                ap[0:W, (it + k) * P : (it + k + 1) * P],
                                ident[0:W, 0:W],
                            )
                            wtot = k * P + W
                        nc.vector.tensor_copy(
                            attT[
                                :,
                                pair * NLP + (off + it) * P : pair * NLP
                                + (off + it) * P
                                + wtot,
                            ],
                            ps_tr[:, 0:wtot],
                        )
                        it += nb

                def s_tail(h, off):
                    # batched narrow-tail scores: the TW-wide last i-tile for
                    # all j-tiles, packed into as few psum banks / exp
                    # instructions as possible
                    pair, hh = h // 2, h % 2
                    p0 = hh * 64
                    qc = (2 * pair) * BS + off * P
                    kc = (2 * pair + 1) * BS
                    bpb = max(1, 512 // TW)  # batches per psum bank
                    t_t = ttp.tile([P, NTL * TW], bf, tag="tt", name="t_tail")
                    done = 0
                    while done < NTL:
                        nb = min(2 * bpb, NTL - done)  # one 2-bank slot
                        ps_s = pss.tile([P, PSW], f32, tag="s", name="ps_st")
                        for g in range(nb):
                            jt = done + g
                            pos = (g // bpb) * 512 + (g % bpb) * TW
                            nc.tensor.matmul(
                                ps_s[:, pos : pos + TW],
                                qk_all[p0 : p0 + 64, kc + jt * P : kc + jt * P + P],
                                qk_all[p0 : p0 + 64, qc : qc + TW],
                                start=True,
                                stop=True,
                            )
                        for bk in range((nb + bpb - 1) // bpb):
                            cnt = min(bpb, nb - bk * bpb)
                            nc.scalar.activation(
                                t_t[
                                    :,
                                    (done + bk * bpb) * TW : (done + bk * bpb + cnt)
                                    * TW,
                                ],
                                ps_s[:, bk * 512 : bk * 512 + cnt * TW],
                                EXP,
                            )
                        done += nb
                    return t_t

                def p_tail(h, off, t_t, ap):
                    # 65-col accumulator lives in a 1-bank "qk" slot so it
                    # never waits on the main PV accumulator (deadlock risk)
                    p0 = (h % 2) * 64
                    pa = pqk.tile([P, 512], f32, tag="qk", name="pa_t")
                    for jt in range(NTL):
                        nc.tensor.matmul(
                            pa[0:TW, 0:VW],
                            t_t[:, jt * TW : (jt + 1) * TW],
                            vplus[:, jt * VROW + h * VW : jt * VROW + (h + 1) * VW],
                            start=(jt == 0),
                            stop=False,
                        )
                    nc.tensor.matmul(
                        pa[0:TW, 0:VW],
                        padrow_sb[:, off * P : off * P + TW],
                        dvec_sb[:, h * VW : (h + 1) * VW],
                        start=False,
                        stop=True,
                    )
                    r_sb = sp.tile([P, 8], f32, tag="r", name="r_t")
                    nc.vector.reciprocal(r_sb[0:TW, 0:1], pa[0:TW, DH : DH + 1])
                    nc.vector.tensor_scalar_mul(
                        ap[0:TW, p0 : p0 + DH], pa[0:TW, 0:DH], r_sb[0:TW, 0:1]
                    )

                nout = [0]
                nbig = [0]

                def emit_half(it, ch, half):
                    # half 0: pairs 0-1 -> outA f32; half 1: pairs 2-3 -> outB bf16
                    W = IW(it)
                    ps_o = pqk.tile([P, 512], f32, tag="qk", name="ps_o")
                    for ht in (0, 1) if half == 0 else (2, 3):
                        nc.tensor.matmul(
                            ps_o[0:W, :],
                            attT[:, ht * NLP + it * P : ht * NLP + it * P + W],
                            wout_sb[:, ht * DM + ch * 512 : ht * DM + (ch + 1) * 512],
                            start=(ht % 2 == 0),
                            stop=(ht % 2 == 1),
                        )
                    dt = f32 if half == 0 else bf
                    o_sb = sp.tile([P, 512], dt, tag="obA" if half == 0 else "obB",
                                   name="o_sb")
                    # A-half evictions stay off ACT (it paces mid-loop exps);
                    # B-half runs in the drain where ACT is idle.
                    if half == 1:
                        nc.scalar.activation(o_sb[0:W, :], ps_o[0:W, :], COPY)
                    else:
                        nc.vector.tensor_copy(o_sb[0:W, :], ps_o[0:W, :])
                    nout[0] += 1
                    dst = outA if half == 0 else outB
                    nc.sync.dma_start(
                        out=dst[it * P : it * P + W, ch * 512 : (ch + 1) * 512],
                        in_=o_sb[0:W, :],
                    )

                # ---- filler queues ----
                # qk chunks for pairs 1..3: pair p before head 2p's scores.
                fast_start = len(qk_chunks) == 3
                fill_units = [
                    (f, c0, cw)
                    for pair in range(1, NP)
                    for f in (2 * pair, 2 * pair + 1)
                    for (c0, cw) in qk_chunks
                    if not (fast_start and f == 2 and c0 < qk_chunks[2][0])
                ]
                n_units = len(fill_units)
                cpp = 2 * len(qk_chunks)
                fill_pos = [0]
                cpair1 = cpp - (2 if fast_start else 0)

                def emit_fill_to(tgt):
                    k = fill_pos[0]
                    for u in fill_units[k : min(n_units, tgt)]:
                        emit_qk_chunk(*u)
                    fill_pos[0] = max(k, min(n_units, tgt))

                def needed_before(h):
                    p = max(0, h // 2)
                    return min(n_units, cpair1 if p == 1 else
                               cpair1 + (p - 1) * cpp if p > 1 else 0)

                # A half-chunks (pairs 0-1): ready once pair-1 main+tail
                # transposes are done (end of window 5); fill windows 6-7.
                a_units = [
                    (it, ch) for it in range(off0, off0 + w0) for ch in range(2)
                ]
                nA = len(a_units)
                a_pos = [0]

                def emit_a_to(tgt):
                    k = a_pos[0]
                    for u in a_units[k : min(nA, tgt)]:
                        emit_half(u[0], u[1], 0)
                    a_pos[0] = max(k, min(nA, tgt))

                # ---- window 0: pair-0 projections + head-0 scores,
                #      pair-1 qk chunks interleaved ----
                if len(qk_chunks) == 3:
                    # dmt-outer interleave across the f0/f1 chunks plus
                    # pair-1's first q chunks keeps the PE fed at DMA
                    # arrival pace (chunk-slots borrowed from the idle
                    # pss/psa pools + pqk)
                    sA = pss.tile([P, PSW], f32, tag="s", name="ps_q0")
                    sB = pss.tile([P, PSW], f32, tag="s", name="ps_q1")
                    qA = pqk.tile([P, 512], f32, tag="qk", name="ps_q2")
                    qB = pqk.tile([P, 512], f32, tag="qk", name="ps_q3")
                    aA = psa.tile([P, PSW], f32, tag="att", name="ps_q4")
                    units = [
                        (0, qk_chunks[0][0], qk_chunks[0][1], sA, 0),
                        (0, qk_chunks[1][0], qk_chunks[1][1], sA, 512),
                        (0, qk_chunks[2][0], qk_chunks[2][1], sB, 0),
                        (1, qk_chunks[0][0], qk_chunks[0][1], sB, 512),
                        (1, qk_chunks[1][0], qk_chunks[1][1], qA, 0),
                        (1, qk_chunks[2][0], qk_chunks[2][1], qB, 0),
                        (2, qk_chunks[0][0], qk_chunks[0][1], aA, 0),
                        (2, qk_chunks[1][0], qk_chunks[1][1], aA, 512),
                    ]
                    for dmt in range(DMT):
                        for f, c0, cw, ps, so in units:
                            nc.tensor.matmul(
                                ps[:, so : so + cw],
                                wqk_sb[
                                    :, dmt * FQK + f * P : dmt * FQK + (f + 1) * P
                                ],
                                xTl_sb[:, dmt * NLP + c0 : dmt * NLP + c0 + cw],
                                start=(dmt == 0),
                                stop=(dmt == DMT - 1),
                            )
                    for f, c0, cw, ps, so in units:
                        nc.vector.tensor_copy(
                            qk_all[:, f * BS + c0 : f * BS + c0 + cw],
                            ps[:, so : so + cw],
                        )
                else:
                    for f in (0, 1):
                        for c0, cw in qk_chunks:
                            emit_qk_chunk(f, c0, cw)
                t_store = {0: []}
                tt_store = {}
                for jt in range(NTL):
                    t_store[0].append(sc_wide(0, off0, w0, jt))
                if tail1:
                    # pair-0 tail scores cover the wv DMA wait; their spill
                    # reads (pair-1 q block) are written by the fast startup
                    tt_store[0] = s_tail(0, tail1[0][0])
                    tt_store[1] = s_tail(1, tail1[0][0])
                for tt in range(min(3, NTL)):
                    emit_v(tt)
                emit_fill_to(cpair1)

                # ---- windows 1..7: S(h) || PV(h-1) || tail(h-1 scores,
                #      h-2 PV) || fillers ----
                ap_cur = None
                apt_cur = None
                for h in range(1, HL):
                    emit_fill_to(needed_before(h))
                    fprev = fill_pos[0]
                    fth = n_units if h >= 5 else max(
                        needed_before(h + 1), (n_units * h * 2 + 8) // 9
                    )
                    aprev = a_pos[0]
                    ath = {5: 5, 6: 13}.get(h, 0 if h < 5 else nA)
                    t_store[h] = []
                    pa = psa.tile([P, PSW], f32, tag="att", name="pa")
                    for jt in range(NTL):
                        t_store[h].append(sc_wide(h, off0, w0, jt))
                        if h == 1 and jt >= 3:
                            emit_v(jt)
                        emit_a_to(aprev + ((ath - aprev) * (jt + 1)) // NTL)
                        pv_wide(h - 1, w0, jt, t_store[h - 1][jt], pa)
                        emit_fill_to(fprev + ((fth - fprev) * (jt + 1)) // NTL)
                        t_store[h - 1][jt] = None
                        if jt == 2 and h >= 2 and tail1 and (h - 2) in tt_store:
                            # lag-2 tail PV for head h-2
                            offt = tail1[0][0]
                            if (h - 2) % 2 == 0:
                                apt_cur = app.tile([P, P], bf, tag="apt", name="apt")
                            p_tail(h - 2, offt, tt_store.pop(h - 2), apt_cur)
                            if (h - 2) % 2 == 1:
                                transpose_pair((h - 2) // 2, offt, 1, apt_cur)
                    if tail1 and h >= 3:
                        tt_store[h - 1] = s_tail(h - 1, tail1[0][0])
                    if tail1 and h == HL - 1:
                        tt_store[h] = s_tail(h, tail1[0][0])
                    corr_wide(h - 1, off0, w0, pa)
                    if (h - 1) % 2 == 0:
                        ap_cur = app.tile([P, PSW], bf, tag="ap", name="ap")
                    norm_wide(h - 1, w0, pa, ap_cur)
                    if (h - 1) % 2 == 1:
                        transpose_pair((h - 1) // 2, off0, w0, ap_cur)

                # ---- epilogue: PV(7), tails 6-7, pipelined pair-3 finish ----
                def emit_outB_big(it, split=False):
                    # whole-row B chunk: pairs 2-3 for both DM halves in one
                    # 2-bank pss slot, one eviction, one outB DMA
                    W = IW(it)
                    ps_o = pss.tile([P, PSW], f32, tag="s", name="ps_b")
                    for ch in range(2):
                        for ht in (2, 3):
                            nc.tensor.matmul(
                                ps_o[0:W, ch * 512 : (ch + 1) * 512],
                                attT[:, ht * NLP + it * P : ht * NLP + it * P + W],
                                wout_sb[
                                    :, ht * DM + ch * 512 : ht * DM + (ch + 1) * 512
                                ],
                                start=(ht == 2),
                                stop=(ht == 3),
                            )
                    o_sb = sp.tile([P, 1024], bf, tag="obB", name="o_sbB")
                    nbig[0] += 1
                    if split:
                        # parallel eviction halves (ACT + DVE): shallow drain
                        nc.scalar.activation(
                            o_sb[0:W, 0:512], ps_o[0:W, 0:512], COPY
                        )
                        nc.vector.tensor_copy(
                            o_sb[0:W, 512:1024], ps_o[0:W, 512:1024]
                        )
                        nc.sync.dma_start(
                            out=outB[it * P : it * P + W, 0:512],
                            in_=o_sb[0:W, 0:512],
                        )
                        nc.sync.dma_start(
                            out=outB[it * P : it * P + W, 512:1024],
                            in_=o_sb[0:W, 512:1024],
                        )
                        return
                    if nbig[0] % 2 == 1:
                        nc.scalar.activation(o_sb[0:W, :], ps_o[0:W, :1024], COPY)
                    else:
                        nc.vector.tensor_copy(o_sb[0:W, :], ps_o[0:W, :1024])
                    nc.sync.dma_start(
                        out=outB[it * P : it * P + W, :], in_=o_sb[0:W, :]
                    )

                if tail1 and (HL - 1) not in tt_store:
                    tt_store[HL - 1] = s_tail(HL - 1, tail1[0][0])
                emit_fill_to(n_units)
                aprev = a_pos[0]
                pa = psa.tile([P, PSW], f32, tag="att", name="pa")
                for jt in range(NTL):
                    emit_a_to(aprev + ((nA - aprev) * (jt + 1)) // NTL)
                    pv_wide(HL - 1, w0, jt, t_store[HL - 1][jt], pa)
                    if jt == 2 and tail1 and (HL - 2) in tt_store:
                        offt = tail1[0][0]
                        apt_cur = app.tile([P, P], bf, tag="apt", name="apt")
                        p_tail(HL - 2, offt, tt_store.pop(HL - 2), apt_cur)
                    if jt == 4 and tail1 and (HL - 1) in tt_store:
                        offt = tail1[0][0]
                        p_tail(HL - 1, offt, tt_store.pop(HL - 1), apt_cur)
                        transpose_pair(NP - 1, offt, 1, apt_cur)
                    if jt == 0 and tail1:
                        # tail i-tile outA rows (pairs 0-1 only): PE work to
                        # cover the window-boundary normalize wait
                        offt = tail1[0][0]
                        emit_half(offt, 0, 0)
                        emit_half(offt, 1, 0)

                corr_wide(HL - 1, off0, w0, pa)
                emit_a_to(nA)
                # pipelined pair-3 finish: normalize slot -> transpose ->
                # previous tile's whole-row B chunk (hides eviction latency)
                p0e = ((HL - 1) % 2) * 64
                r_sb = sp.tile([P, 8], f32, tag="r", name="r_e")
                pa3 = pa.rearrange("p (t c) -> p t c", t=PSW // P, c=P)
                nc.vector.reciprocal(r_sb[:, :w0], pa3[:, :w0, DH])
                for it in range(w0):
                    nc.vector.tensor_scalar_mul(
                        ap_cur[:, it * P + p0e : it * P + p0e + DH],
                        pa[:, it * P : it * P + DH],
                        r_sb[:, it : it + 1],
                    )
                    ps_tr = pqk.tile([P, P], bf, tag="qk", name="ps_tr")
                    nc.tensor.transpose(
                        ps_tr[:, :], ap_cur[:, it * P : (it + 1) * P], ident
                    )
                    nc.vector.tensor_copy(
                        attT[
                            :,
                            (NP - 1) * NLP + (off0 + it) * P : (NP - 1) * NLP
                            + (off0 + it + 1) * P,
                        ],
                        ps_tr[:, :],
                    )
                    if it >= 1:
                        emit_outB_big(off0 + it - 1, split=(it == w0 - 1))
                emit_outB_big(off0 + w0 - 1, split=True)
                if tail1:
                    # tail i-tile outB last: its 24-row eviction/DMAs drain
                    # far faster than a full 128-row tile's
                    emit_outB_big(tail1[0][0], split=True)

                # ---- slow generic path for wide tail spans (NTL > 9) ----
                for offt, wt in slow_tails:
                    for pair in range(NP):
                        for hh in range(2):
                            h = 2 * pair + hh
                            pa = psa.tile([P, PSW], f32, tag="att", name="pa_w")
                            tw = [sc_wide(h, offt, wt, jt) for jt in range(NTL)]
                            for jt in range(NTL):
                                pv_wide(h, wt, jt, tw[jt], pa)
                            corr_wide(h, offt, wt, pa)
                            if hh == 0:
                                ap_t = app.tile([P, PSW], bf, tag="ap", name="ap_w")
                            norm_wide(h, wt, pa, ap_t)
                        transpose_pair(pair, offt, wt, ap_t)
                    for it in range(offt, offt + wt):
                        for ch in range(2):
                            emit_half(it, ch, 0)

                # B halves for slow-tail tiles (already emitted for the rest)
                for offt, wt in slow_tails:
                    for it in range(offt, offt + wt):
                        for ch in range(2):
                            emit_half(it, ch, 1)

    nc.compile()
    return nc


def _shard_inputs(x, w_qkv, w_out, b_out, mask):
    """Host-side live-token gather + per-core layout prep."""
    bf = ml_dtypes.bfloat16
    x = np.asarray(x, dtype=np.float32)
    w_qkv = np.asarray(w_qkv, dtype=np.float32)
    w_out = np.asarray(w_out, dtype=np.float32)
    mask = np.asarray(mask)

    NLs = [int(mask[b].sum()) for b in range(B)]
    NLP = int(np.ceil((max(NLs) + 1) / 8) * 8)
    NTL = (NLP + P - 1) // P

    w3 = w_qkv.reshape(DM, H, 3, DH)
    in_maps = []
    for c in range(NCORES):
        b, hg = c // HG, c % HG
        nl = NLs[b]
        live = np.nonzero(mask[b])[0]
        dead = np.nonzero(mask[b] == 0)[0]

        wq = w3[:, hg * HL : (hg + 1) * HL, 0, :].reshape(DM, FV) * SCALE
        wk = w3[:, hg * HL : (hg + 1) * HL, 1, :].reshape(DM, FV)
        # pair-major column layout: [q_p0 | k_p0 | q_p1 | k_p1 | ...]
        wqk_c = np.empty((DM, FQK), np.float32)
        for p in range(NP):
            wqk_c[:, p * 256 : p * 256 + 128] = wq[:, p * 128 : (p + 1) * 128]
            wqk_c[:, p * 256 + 128 : (p + 1) * 256] = wk[:, p * 128 : (p + 1) * 128]
        wv_c = np.ascontiguousarray(
            w3[:, hg * HL : (hg + 1) * HL, 2, :].reshape(DM, FV)
        )
        wout_c = np.ascontiguousarray(w_out[hg * FV : (hg + 1) * FV, :])

        xTl_c = np.zeros((DM, NLP), np.float32)
        xTl_c[:, :nl] = x[b].T[:, live]

        padrow_c = np.zeros((1, NLP), np.float32)
        padrow_c[0, nl:] = 1.0
        lind_c = np.zeros(NTL * P, np.float32)
        lind_c[:nl] = 1.0
        lind_c = np.ascontiguousarray(lind_c.reshape(NTL, P).T)

        # dvec: per head [sum_dead v_h | N_dead]
        xs = x[b][dead].sum(axis=0) if len(dead) else np.zeros(DM, np.float32)
        dv = (xs @ wv_c).reshape(HL, DH)
        dvec_c = np.empty((1, VROW), np.float32)
        for h in range(HL):
            dvec_c[0, h * VW : h * VW + DH] = dv[h]
            dvec_c[0, h * VW + DH] = float(len(dead))

        in_maps.append(
            {
                "xTl": xTl_c.astype(bf),
                "wqk": wqk_c.astype(bf),
                "wv": wv_c.astype(bf),
                "wout": wout_c.astype(bf),
                "padrow": padrow_c.astype(bf),
                "lind": lind_c.astype(np.float32),
                "dvec": dvec_c.astype(bf),
            }
        )
    return in_maps, NLP, NLs


def kernel(x, w_qkv, w_out, b_out, mask):
    from concourse.bass_utils import run_bass_kernel_spmd

    in_maps, NLP, NLs = _shard_inputs(x, w_qkv, w_out, b_out, mask)
    if NLP not in _CACHE:
        _CACHE[NLP] = _build_program(NLP)
    nc = _CACHE[NLP]

    res = run_bass_kernel_spmd(nc, in_maps, list(range(NCORES))).results

    mask = np.asarray(mask)
    b_out = np.asarray(b_out, dtype=np.float32)
    out = np.empty((B, N, DM), np.float32)
    for b in range(B):
        nl = NLs[b]
        live = np.nonzero(mask[b])[0]
        dead = np.nonzero(mask[b] == 0)[0]
        part = (
            res[HG * b]["outA"]
            + res[HG * b]["outB"].astype(np.float32)
            + res[HG * b + 1]["outA"]
            + res[HG * b + 1]["outB"].astype(np.float32)
        )
        out[b, live] = part[:nl]
        if len(dead):
            out[b, dead] = part[nl]
        out[b] += b_out[None, :]
    return out
